# revision 44
# baseline (speedup 1.0000x reference)
"""Trainium2 Bass kernel for a GPT-2 style transformer block.

Problem: B=4, T=2048, C=1024, H=16 heads (hd=64), MLP hidden 4096, fp32 I/O.

Sharding: zero-collective 8-way data parallel. Core c handles batch b=c//2;
s=c%2 selects its query set: s=0 owns the OUTER sequence quarters (blocks
0-3 and 12-15 of 128 tokens), s=1 the MIDDLE half (blocks 4-11). This makes
the causal-attention work symmetric across the pair: a uniform 24-tile
slot schedule per head covers both cores' needs, with per-core host-side
mask / gate tables providing the divergence. K/V are computed locally for
all 2048 tokens in own-first order.

Precision: all big weight matmuls (QKV, V, proj, FC, out) run in fp8e4
DoubleRow perf mode (2 contraction rows per PE cell per cycle): weights are
host-prescaled by 64 (W_out by 256) so N(0, 0.02)-scale values land in
e4m3's normal range; the scale is removed in the PSUM evacuation ops.
Attention q/k/v/exp-weights are fp8e4 as well (no DoubleRow; contraction is
only 64/128 deep), scores accumulate in f32 PSUM and softmax runs in f32 on
ScalarE. LayerNorm is f32 (bn_stats), residuals bf16/f32.

Layouts:
  x / residuals / final out: token-major [tok(P), C]
  x_ln transposed to feature-major [feat(P), chunk, tok] fp8 via PE
  transposes (bf16) + DVE convert-copy
  Q [feat(P), TQ], K [feat(P), T2] fp8; head pair hp lives in one tile
  (rows 0-63 head 2hp, 64-127 head 2hp+1) so score matmuls of a pair are
  emitted adjacently and run CONCURRENTLY in distinct PE row-groups
  V token-major [tok(P), h*65] fp8 with a built-in ones column per head
  (softmax row sums ride the AV matmul); softmax needs no max-subtraction
  (scores bounded ~|s|<4) and no transposes anywhere in attention
  normalization + v-bias deferred to after AV via a tiny K=2 f32r selector
  matmul that partition-broadcasts 16/sums
"""

import os
import sys
import types

import numpy as np
import ml_dtypes

for _p in ("/opt/trn_rl_repo", "/root/.axon_site/_ro/trn_rl_repo"):
    if os.path.isdir(_p) and _p not in sys.path:
        sys.path.append(_p)

# antenv.axon_hooks is absent in this image; bass_utils imports it when
# tracing under axon. Provide the trivial get/set holder it expects.
if "antenv.axon_hooks" not in sys.modules:
    try:
        import antenv

        _m = types.ModuleType("antenv.axon_hooks")
        _m._hook = None

        def _set_hook(h):
            _m._hook = h

        def _get_hook():
            return _m._hook

        _m.set_axon_ntff_profile_hook = _set_hook
        _m.get_axon_ntff_profile_hook = _get_hook
        sys.modules["antenv.axon_hooks"] = _m
        antenv.axon_hooks = _m
    except ImportError:
        pass

import concourse.bacc as bacc
import concourse.tile as tile
from concourse import mybir
from concourse.masks import make_identity

P = 128
B, T, C = 4, 2048, 1024
H, HD = 16, 64
F = 4096
T2 = T  # tokens per core for K/V (full sequence of one batch element)
TQ = T // 2  # own query tokens per core
CC = C // P  # 8 C-chunks
NPR = CC // 2  # 4 DoubleRow contraction pair-chunks (256 each)
N = 512  # moving free dim per matmul

SW = 64.0  # fp8 weight prescale (qkv/v/proj/fc)
SWO = 256.0  # fp8 weight prescale for W_out
SY = 16.0  # attention-output prescale into fp8

F32 = mybir.dt.float32
F32R = mybir.dt.float32r
BF16 = mybir.dt.bfloat16
FP8 = mybir.dt.float8e4
AF = mybir.ActivationFunctionType
OP = mybir.AluOpType
DR = mybir.MatmulPerfMode.DoubleRow

PROFILE = False
SIM_GELU = False  # CoreSim lacks the Gelu LUT; emulate with Tanh + DVE ops
last_exec_time_ns = None

_CACHE = {}

# per-(g,kt) attention slot schedule, uniform across cores.
# kinds: 'diag' (mask m), 'full', 'gate2' (live iff s==1), 'gate3' (iff s==0)
SLOTS0 = [(0, "diag", 0), (1, "diag", 1), (2, "diag", 2), (3, "diag", 3),
          (8, "gate2", 0), (9, "gate2", 0), (10, "gate2", 0), (11, "gate2", 0)]
SLOTS1 = [(0, "full", 0), (1, "full", 0), (2, "full", 0), (3, "full", 0),
          (4, "diag", 0), (5, "diag", 1), (6, "diag", 2), (7, "diag", 3),
          (8, "full", 0), (9, "full", 0), (10, "full", 0), (11, "full", 0),
          (12, "gate3", 0), (13, "gate3", 0), (14, "gate3", 0), (15, "gate3", 0)]
SLOTS = (SLOTS0, SLOTS1)


def _build_nc(apply_lnwb: bool = True):
    nc = bacc.Bacc("TRN2", target_bir_lowering=False, debug=False, num_devices=8)

    x_seq = nc.dram_tensor("x_seq", [T2, C], F32, kind="ExternalInput")
    w_qk3 = nc.dram_tensor("w_qk3", [P, NPR, 2, 2 * C], FP8, kind="ExternalInput")
    w_v3 = nc.dram_tensor("w_v3", [P, NPR, 2, C], FP8, kind="ExternalInput")
    w_pj3 = nc.dram_tensor("w_pj3", [P, NPR, 2, C], FP8, kind="ExternalInput")
    w_fc3 = nc.dram_tensor("w_fc3", [P, NPR, 2, F], FP8, kind="ExternalInput")
    w_ot3 = nc.dram_tensor("w_ot3", [P, F // 256, 2, C], FP8, kind="ExternalInput")
    ln1w = nc.dram_tensor("ln1w", [P, C], F32, kind="ExternalInput")
    ln1b = nc.dram_tensor("ln1b", [P, C], F32, kind="ExternalInput")
    ln2w = nc.dram_tensor("ln2w", [P, C], F32, kind="ExternalInput")
    ln2b = nc.dram_tensor("ln2b", [P, C], F32, kind="ExternalInput")
    b_q = nc.dram_tensor("b_q", [P, CC], F32, kind="ExternalInput")  # x64
    b_k = nc.dram_tensor("b_k", [P, CC], F32, kind="ExternalInput")  # x64
    b_v = nc.dram_tensor("b_v", [P, CC], F32, kind="ExternalInput")  # x16, col per chunk
    b_pj = nc.dram_tensor("b_pj", [P, C], F32, kind="ExternalInput")
    b_fc = nc.dram_tensor("b_fc", [P, F // P], F32, kind="ExternalInput")
    b_ot = nc.dram_tensor("b_ot", [P, C], F32, kind="ExternalInput")
    mask4 = nc.dram_tensor("mask4", [P, 4, 2, N], BF16, kind="ExternalInput")
    gate2 = nc.dram_tensor("gate2", [P, 1], F32, kind="ExternalInput")
    gate3 = nc.dram_tensor("gate3", [P, 1], F32, kind="ExternalInput")
    sel2 = nc.dram_tensor("sel2", [33, P], F32R, kind="ExternalInput")
    ones33 = nc.dram_tensor("ones33", [33, N], F32R, kind="ExternalInput")

    out_d = nc.dram_tensor("out", [TQ, C], F32, kind="ExternalOutput")

    from contextlib import ExitStack

    with tile.TileContext(nc) as tc, ExitStack() as ctx:
        # pool enter order = reverse of close order (pool stack is LIFO);
        # SBUF is reserved from first tile creation to pool close
        const = ctx.enter_context(tc.tile_pool(name="const", bufs=1))
        p_big = ctx.enter_context(tc.tile_pool(name="p_big", bufs=1))
        es_mid = ctx.enter_context(ExitStack())
        es_x2 = ctx.enter_context(ExitStack())
        es_wpj = ctx.enter_context(ExitStack())
        es_y8 = ctx.enter_context(ExitStack())
        es_qk = ctx.enter_context(ExitStack())
        es_v = ctx.enter_context(ExitStack())
        es_wv = ctx.enter_context(ExitStack())

        ident = const.tile([P, P], BF16, tag="ident", name="ident")
        make_identity(nc, ident)
        eps_sb = const.tile([P, 1], F32, tag="eps", name="eps")
        nc.vector.memset(eps_sb[:], 1e-5)
        mask_sb = const.tile([P, 4, 2, N], BF16, tag="mask", name="mask")
        nc.sync.dma_start(out=mask_sb[:], in_=mask4[:])
        gate2_sb = const.tile([P, 1], F32, tag="g2", name="g2")
        nc.sync.dma_start(out=gate2_sb[:], in_=gate2[:])
        gate3_sb = const.tile([P, 1], F32, tag="g3", name="g3")
        nc.sync.dma_start(out=gate3_sb[:], in_=gate3[:])
        sel_sb = const.tile([33, P], F32R, tag="sel", name="sel")
        nc.sync.dma_start(out=sel_sb[:], in_=sel2[:])
        # reciprocal softmax sums land on partitions 0 / 32 of s2 (legal
        # engine write bases); other rows stay 1.0 so sel's zeros see no NaNs
        s2_sb = const.tile([33, N], F32R, tag="s2", name="s2")
        nc.sync.dma_start(out=s2_sb[:], in_=ones33[:])
        bq_sb = const.tile([P, CC], F32, tag="bq", name="bq")
        nc.sync.dma_start(out=bq_sb[:], in_=b_q[:])
        bk_sb = const.tile([P, CC], F32, tag="bk", name="bk")
        nc.sync.dma_start(out=bk_sb[:], in_=b_k[:])
        bv_sb = const.tile([P, CC], F32, tag="bv", name="bv")
        nc.sync.dma_start(out=bv_sb[:], in_=b_v[:])
        bfc_sb = const.tile([P, F // P], F32, tag="bfc", name="bfc")
        nc.sync.dma_start(out=bfc_sb[:], in_=b_fc[:])
        bpj_sb = const.tile([P, C], F32, tag="bpj", name="bpj")
        nc.sync.dma_start(out=bpj_sb[:], in_=b_pj[:])
        bot_sb = const.tile([P, C], F32, tag="bot", name="bot")
        nc.sync.dma_start(out=bot_sb[:], in_=b_ot[:])

        p_mid = es_mid.enter_context(tc.tile_pool(name="p_mid", bufs=1))
        p_x2 = es_x2.enter_context(tc.tile_pool(name="p_x2", bufs=1))
        p_wpj = es_wpj.enter_context(tc.tile_pool(name="p_wpj", bufs=1))
        p_y8 = es_y8.enter_context(tc.tile_pool(name="p_y8", bufs=1))
        p_qk = es_qk.enter_context(tc.tile_pool(name="p_qk", bufs=1))
        p_v = es_v.enter_context(tc.tile_pool(name="p_v", bufs=1))
        p_wv = es_wv.enter_context(tc.tile_pool(name="p_wv", bufs=1))

        # one 32 KiB/partition fp8 buffer triple-aliased across disjoint
        # lifetimes: [xlnT | wqk] (phases 1-3) then h8 (phases 7-8)
        buf32 = p_big.tile([P, 2 * CC * T2], FP8, tag="buf32", name="buf32")
        xlnT = buf32.rearrange("p (a c t) -> p a c t", a=2, t=T2)[:, 0]
        wqk_sb = buf32.rearrange("p (a pr hf f) -> p a pr hf f",
                                 a=2, pr=NPR, hf=2)[:, 1]
        h8 = buf32.rearrange("p (f t) -> p f t", t=TQ)
        wv_sb = p_wv.tile([P, NPR, 2, C], FP8, tag="wv", name="wv")
        wpj_sb = p_wpj.tile([P, NPR, 2, C], FP8, tag="wpj", name="wpj")

        # ---------------- Phase 1: LN1 + transpose + K (fused) --------------
        # K matmuls for token-block pairs are emitted as soon as their
        # transposes land, filling the PE during the DVE/ACT-bound LN loop.
        # Weight-stationary over 2 blocks so each 256-col DoubleRow weight
        # load amortizes over 2 matmuls; evacuations ((psum+64b)/64 -> bf16)
        # run on ScalarE (idle here) via the free affine: ps/64 + b_true.
        nc.sync.dma_start(out=wqk_sb[:], in_=w_qk3[:])
        nc.sync.dma_start(out=wv_sb[:], in_=w_v3[:])
        nc.sync.dma_start(out=wpj_sb[:], in_=w_pj3[:])
        q_sb = [p_qk.tile([P, TQ], BF16, tag=f"q{i}", name=f"q{i}") for i in range(CC)]
        k_sb = [p_qk.tile([P, T2], BF16, tag=f"k{i}", name=f"k{i}") for i in range(CC)]

        def emit_k(bp, kps):
            for fc in range(CC):
                psK = [kps.tile([P, N], F32, tag="k_ps", name="k_ps") for _ in range(2)]
                for pr in range(NPR):
                    lk = wqk_sb[:, pr, :, C + fc * P:C + (fc + 1) * P]
                    for bi in range(2):
                        blk = 2 * bp + bi
                        nc.tensor.matmul(psK[bi][:], lhsT=lk,
                                         rhs=xlnT[:, 2 * pr:2 * pr + 2, blk * N:(blk + 1) * N],
                                         start=(pr == 0), stop=(pr == NPR - 1),
                                         perf_mode=DR)
                for bi in range(2):
                    blk = 2 * bp + bi
                    nc.scalar.activation(k_sb[fc][:, blk * N:(blk + 1) * N],
                                         psK[bi][:], AF.Identity,
                                         bias=bk_sb[:, fc:fc + 1], scale=1.0 / SW)

        with tc.tile_pool(name="qk_ps", bufs=4, space="PSUM") as kps:
            with tc.tile_pool(name="ln1_sp", bufs=3) as sp, \
                 tc.tile_pool(name="ln1_cp", bufs=1) as cp, \
                 tc.tile_pool(name="ln1_st", bufs=6) as st, \
                 tc.tile_pool(name="ln1_ps", bufs=3, space="PSUM") as tps:
                if apply_lnwb:
                    w1 = cp.tile([P, C], F32, tag="w1", name="w1")
                    nc.sync.dma_start(out=w1[:], in_=ln1w[:])
                    b1 = cp.tile([P, C], F32, tag="b1", name="b1")
                    nc.sync.dma_start(out=b1[:], in_=ln1b[:])
                for tt in range(T2 // P):
                    xt = sp.tile([P, C], F32, tag="xs", name="xs")
                    nc.sync.dma_start(out=xt[:], in_=x_seq[tt * P:(tt + 1) * P, :])
                    stats = st.tile([P, 2, 6], F32, tag="st", name="st")
                    for g in range(2):
                        nc.vector.bn_stats(out=stats[:, g, :], in_=xt[:, g * 512:(g + 1) * 512])
                    mv = st.tile([P, 2], F32, tag="mv", name="mv")
                    nc.vector.bn_aggr(out=mv[:], in_=stats[:])
                    rstd = st.tile([P, 1], F32, tag="rstd", name="rstd")
                    nc.scalar.activation(rstd[:], mv[:, 1:2], AF.Sqrt, bias=eps_sb[:], scale=1.0)
                    nc.vector.reciprocal(out=rstd[:], in_=rstd[:])
                    xb = sp.tile([P, C], BF16, tag="xb", name="xb")
                    nmr = st.tile([P, 1], F32, tag="nmr", name="nmr")
                    nc.vector.tensor_scalar(nmr[:], mv[:, 0:1], rstd[:], -1.0,
                                            OP.mult, OP.mult)
                    if apply_lnwb:
                        xc = sp.tile([P, C], F32, tag="xc", name="xc")
                        nc.scalar.activation(xc[:], xt[:], AF.Identity,
                                             bias=nmr[:], scale=rstd[:])
                        xw = sp.tile([P, C], F32, tag="xw", name="xw")
                        nc.vector.tensor_tensor(out=xw[:], in0=xc[:], in1=w1[:], op=OP.mult)
                        nc.vector.tensor_tensor(out=xb[:], in0=xw[:], in1=b1[:], op=OP.add)
                    else:
                        nc.scalar.activation(xb[:], xt[:], AF.Identity,
                                             bias=nmr[:], scale=rstd[:])
                    for gr in range(2):
                        pst = tps.tile([P, 4, P], BF16, tag="trp", name="trp")
                        for j in range(4):
                            cc = gr * 4 + j
                            nc.tensor.transpose(pst[:, j, :],
                                                xb[:, cc * P:(cc + 1) * P], ident[:])
                        nc.vector.tensor_copy(
                            out=xlnT[:, gr * 4:(gr + 1) * 4, tt * P:(tt + 1) * P],
                            in_=pst[:])
                    if tt == 7:
                        emit_k(0, kps)
            emit_k(1, kps)

        # ---------------- Phase 2: Q projection (DoubleRow fp8) -------------
        with tc.tile_pool(name="q_ps", bufs=4, space="PSUM") as qps:
            for fc in range(CC):
                psQ = [qps.tile([P, N], F32, tag="q_ps", name="q_ps") for _ in range(2)]
                for pr in range(NPR):
                    lq = wqk_sb[:, pr, :, fc * P:(fc + 1) * P]
                    for blk in range(2):
                        nc.tensor.matmul(psQ[blk][:], lhsT=lq,
                                         rhs=xlnT[:, 2 * pr:2 * pr + 2, blk * N:(blk + 1) * N],
                                         start=(pr == 0), stop=(pr == NPR - 1),
                                         perf_mode=DR)
                for blk in range(2):
                    nc.scalar.activation(q_sb[fc][:, blk * N:(blk + 1) * N],
                                         psQ[blk][:], AF.Identity,
                                         bias=bq_sb[:, fc:fc + 1], scale=1.0 / SW)

        # ---------------- Phase 2b: V projection (DoubleRow fp8) ------------
        v_sb = [p_v.tile([P, H * 65], BF16, tag=f"v{i}", name=f"v{i}")
                for i in range(T2 // P)]
        for kt in range(T2 // P):
            nc.gpsimd.memset(
                v_sb[kt].rearrange("p (h d) -> p h d", d=65)[:, :, 64:65], 1.0)
        with tc.tile_pool(name="v_ps", bufs=4, space="PSUM") as vps:
            for kt in range(T2 // P):
                ps = [vps.tile([P, N], F32, tag="v_ps", name="v_ps") for _ in range(2)]
                for pr in range(NPR):
                    lv = xlnT[:, 2 * pr:2 * pr + 2, kt * P:(kt + 1) * P]
                    for vg in range(2):
                        nc.tensor.matmul(ps[vg][:], lhsT=lv,
                                         rhs=wv_sb[:, pr, :, vg * N:(vg + 1) * N],
                                         start=(pr == 0), stop=(pr == NPR - 1),
                                         perf_mode=DR)
                for vg in range(2):
                    out_ap = v_sb[kt].rearrange("p (h d) -> p h d", d=65)[
                        :, vg * 8:(vg + 1) * 8, 0:64]
                    in_ap = ps[vg].rearrange("p (h d) -> p h d", d=64)[:, :, :]
                    nc.scalar.activation(out_ap, in_ap, AF.Identity,
                                         bias=0.0, scale=1.0 / SW)

        # ---------------- Phase 3: attention + fused normalize --------------
        # Per head-pair: 24 causal slots/qb-group; score matmul pair runs
        # concurrently in PE row-groups 0-63/64-127. After each pair's AV,
        # its softmax sums roundtrip through DRAM (partition repack) and the
        # yps tiles are normalized straight into fp8 y8 (x SY, + SY*b_v),
        # overlapping the next pair's exp-bound stream.
        y8 = p_y8.tile([P, CC, TQ], FP8, tag="y8", name="y8")
        with tc.tile_pool(name="att_at", bufs=1) as ap_pool, \
             tc.tile_pool(name="att_rp", bufs=2) as rp, \
             tc.tile_pool(name="att_sps", bufs=2, space="PSUM") as sps_pool, \
             tc.tile_pool(name="att_yps", bufs=2, space="PSUM") as yps_pool:
            for hp in range(H // 2):
                for qb in (0, 1):
                    slots = SLOTS[qb]
                    yps = [yps_pool.tile([65, N], F32, tag="yps", name="yps")
                           for _ in range(2)]
                    last = len(slots) - 1
                    for i, (kt, kind, m) in enumerate(slots):
                        sp = sps_pool.tile([P, 2, N], F32, tag="sps", name="sps")
                        for j in range(2):
                            ro = j * 64
                            nc.tensor.matmul(
                                sp[:, j, :],
                                lhsT=k_sb[hp][ro:ro + 64, kt * P:(kt + 1) * P],
                                rhs=q_sb[hp][ro:ro + 64, qb * N:(qb + 1) * N],
                                start=True, stop=True)
                        at = ap_pool.tile([P, 2, N], BF16, tag="at", name="at", bufs=6)
                        bias = {"diag": 0.0, "full": 0.0,
                                "gate2": gate2_sb[:, 0:1],
                                "gate3": gate3_sb[:, 0:1]}[kind]
                        nc.scalar.activation(at[:, 0:2, :], sp[:, 0:2, :],
                                             AF.Exp, bias=bias, scale=0.125)
                        if kind == "diag":
                            nc.gpsimd.tensor_tensor(
                                out=at[:, 0:2, :], in0=at[:, 0:2, :],
                                in1=mask_sb[:, m, :, :], op=OP.mult)
                        for j in range(2):
                            h = 2 * hp + j
                            nc.tensor.matmul(yps[j][:],
                                             lhsT=v_sb[kt][:, h * 65:(h + 1) * 65],
                                             rhs=at[:, j, :],
                                             start=(i == 0), stop=(i == last))
                    # 1/sums onto partitions 0/32, then the SY-scaled 33-row
                    # selector matmul broadcasts SY/sums to each head's rows
                    with nc.allow_low_precision(reason="f32r view of f32 recip"):
                        for j in range(2):
                            nc.vector.reciprocal(out=s2_sb[32 * j:32 * j + 1, :],
                                                 in_=yps[j][64:65, :])
                    rps = yps_pool.tile([P, N], F32, tag="rps", name="rps")
                    nc.tensor.matmul(rps[:], lhsT=sel_sb[:], rhs=s2_sb[:],
                                     start=True, stop=True)
                    rps_sb = rp.tile([P, N], F32, tag="rps_sb", name="rps_sb")
                    nc.vector.tensor_copy(out=rps_sb[:], in_=rps[:])
                    for j in range(2):
                        rr = slice(j * 64, (j + 1) * 64)
                        nc.vector.tensor_tensor(
                            out=y8[rr, hp, qb * N:(qb + 1) * N],
                            in0=yps[j][0:64, :], in1=rps_sb[rr, :], op=OP.mult)
                        nc.vector.tensor_scalar(
                            y8[rr, hp, qb * N:(qb + 1) * N],
                            y8[rr, hp, qb * N:(qb + 1) * N],
                            bv_sb[rr, hp:hp + 1], None, OP.add)

        es_wv.close()
        es_v.close()
        es_qk.close()

        # ---------------- Phase 5: proj + residual + LN2 (fused) ------------
        # proj evacuation, residual add, LN2 and its transposes all run
        # per-128-token tile so PE pipelines the next tile's proj matmuls
        # under this tile's DVE/ACT work.
        x_mid = [p_mid.tile([P, C], BF16, tag=f"xm{i}", name=f"xm{i}")
                 for i in range(TQ // P)]
        xln2T = p_x2.tile([P, CC, TQ], FP8, tag="xln2T", name="xln2T")
        with tc.tile_pool(name="pj_sp", bufs=3) as sp, \
             tc.tile_pool(name="pj_cp", bufs=1) as cp, \
             tc.tile_pool(name="pj_st", bufs=6) as st, \
             tc.tile_pool(name="pj_ps", bufs=2, space="PSUM") as pps, \
             tc.tile_pool(name="ln2_ps", bufs=3, space="PSUM") as tps:
            if apply_lnwb:
                w2 = cp.tile([P, C], F32, tag="w2", name="w2")
                nc.sync.dma_start(out=w2[:], in_=ln2w[:])
                b2 = cp.tile([P, C], F32, tag="b2", name="b2")
                nc.sync.dma_start(out=b2[:], in_=ln2b[:])
            for t8 in range(TQ // P):
                xo = sp.tile([P, C], F32, tag="xo", name="xo")
                nc.sync.dma_start(out=xo[:], in_=x_seq[t8 * P:(t8 + 1) * P, :])
                ps2 = pps.tile([P, 2, N], F32, tag="pj_ps", name="pj_ps")
                for pr in range(NPR):
                    ly = y8[:, 2 * pr:2 * pr + 2, t8 * P:(t8 + 1) * P]
                    for ft in range(2):
                        nc.tensor.matmul(ps2[:, ft, :], lhsT=ly,
                                         rhs=wpj_sb[:, pr, :, ft * N:(ft + 1) * N],
                                         start=(pr == 0), stop=(pr == NPR - 1),
                                         perf_mode=DR)
                xt = x_mid[t8]
                nc.scalar.activation(xt[:], ps2[:, 0:2, :], AF.Identity,
                                     bias=0.0, scale=1.0 / (SW * SY))
                nc.vector.tensor_tensor(out=xt[:], in0=xt[:], in1=xo[:], op=OP.add)
                nc.vector.tensor_tensor(out=xt[:], in0=xt[:], in1=bpj_sb[:], op=OP.add)
                # --- LN2 for this token tile ---
                stats = st.tile([P, 2, 6], F32, tag="st2", name="st2")
                for g in range(2):
                    nc.vector.bn_stats(out=stats[:, g, :], in_=xt[:, g * 512:(g + 1) * 512])
                mv = st.tile([P, 2], F32, tag="mv2", name="mv2")
                nc.vector.bn_aggr(out=mv[:], in_=stats[:])
                rstd = st.tile([P, 1], F32, tag="rstd2", name="rstd2")
                nc.scalar.activation(rstd[:], mv[:, 1:2], AF.Sqrt, bias=eps_sb[:], scale=1.0)
                nc.vector.reciprocal(out=rstd[:], in_=rstd[:])
                xb = sp.tile([P, C], BF16, tag="xb2", name="xb2")
                nmr = st.tile([P, 1], F32, tag="nmr2", name="nmr2")
                nc.vector.tensor_scalar(nmr[:], mv[:, 0:1], rstd[:], -1.0,
                                        OP.mult, OP.mult)
                if apply_lnwb:
                    xc = sp.tile([P, C], F32, tag="xc2", name="xc2")
                    nc.scalar.activation(xc[:], xt[:], AF.Identity,
                                         bias=nmr[:], scale=rstd[:])
                    xw = sp.tile([P, C], F32, tag="xw2", name="xw2")
                    nc.vector.tensor_tensor(out=xw[:], in0=xc[:], in1=w2[:], op=OP.mult)
                    nc.vector.tensor_tensor(out=xb[:], in0=xw[:], in1=b2[:], op=OP.add)
                else:
                    nc.scalar.activation(xb[:], xt[:], AF.Identity,
                                         bias=nmr[:], scale=rstd[:])
                for gr in range(2):
                    pst = tps.tile([P, 4, P], BF16, tag="trp2", name="trp2")
                    for j in range(4):
                        cc = gr * 4 + j
                        nc.tensor.transpose(pst[:, j, :],
                                            xb[:, cc * P:(cc + 1) * P], ident[:])
                    nc.vector.tensor_copy(
                        out=xln2T[:, gr * 4:(gr + 1) * 4, t8 * P:(t8 + 1) * P],
                        in_=pst[:])

        es_y8.close()
        es_wpj.close()

        # ---------------- Phase 7: FC + gelu (DoubleRow fp8) ---------------
        with tc.tile_pool(name="fc_w", bufs=3) as wp, \
             tc.tile_pool(name="fc_ps", bufs=3, space="PSUM") as fps:
            for hg in range(F // N):
                wt = wp.tile([P, NPR, 2, N], FP8, tag="wfc", name="wfc")
                nc.sync.dma_start(out=wt[:], in_=w_fc3[:, :, :, hg * N:(hg + 1) * N])
                for hs in range(4):
                    hf = hg * 4 + hs
                    ps2 = fps.tile([P, 2, N], F32, tag="fc_ps", name="fc_ps")
                    for pr in range(NPR):
                        lw = wt[:, pr, :, hs * P:(hs + 1) * P]
                        for tt in range(2):
                            nc.tensor.matmul(ps2[:, tt, :], lhsT=lw,
                                             rhs=xln2T[:, 2 * pr:2 * pr + 2, tt * N:(tt + 1) * N],
                                             start=(pr == 0), stop=(pr == NPR - 1),
                                             perf_mode=DR)
                    if not SIM_GELU:
                        nc.scalar.activation(h8[:, hf, :], ps2[:, 0:2, :],
                                             AF.Gelu_apprx_tanh,
                                             bias=bfc_sb[:, hf:hf + 1], scale=1.0 / SW)
                    else:
                        import math
                        cst = math.sqrt(2.0 / math.pi)
                        u = wp.tile([P, 2, N], F32, tag="g_u", name="g_u")
                        nc.scalar.activation(u[:], ps2[:, 0:2, :], AF.Identity,
                                             bias=bfc_sb[:, hf:hf + 1], scale=1.0 / SW)
                        u3 = wp.tile([P, 2, N], F32, tag="g_u3", name="g_u3")
                        nc.scalar.activation(u3[:], u[:], AF.Square, bias=0.0, scale=1.0)
                        nc.vector.tensor_tensor(out=u3[:], in0=u3[:], in1=u[:], op=OP.mult)
                        nc.vector.tensor_scalar(u3[:], u3[:], 0.044715, None, OP.mult)
                        nc.vector.tensor_tensor(out=u3[:], in0=u3[:], in1=u[:], op=OP.add)
                        tqh = wp.tile([P, 2, N], F32, tag="g_t", name="g_t")
                        nc.scalar.activation(tqh[:], u3[:], AF.Tanh, bias=0.0, scale=cst)
                        nc.vector.tensor_scalar(tqh[:], tqh[:], 1.0, None, OP.add)
                        nc.vector.tensor_tensor(out=tqh[:], in0=tqh[:], in1=u[:], op=OP.mult)
                        nc.vector.tensor_scalar(h8[:, hf, :], tqh[:], 0.5, None, OP.mult)

        es_x2.close()

        # ---------------- Phase 8: out matmul + residual (DoubleRow fp8) ---
        with tc.tile_pool(name="ot_w", bufs=6) as wp, \
             tc.tile_pool(name="ot_sp", bufs=3) as sp, \
             tc.tile_pool(name="ot_ps", bufs=8, space="PSUM") as ops_pool:
            for half in range(2):
                opss = [ops_pool.tile([P, N], F32, tag="ot_ps", name="ot_ps")
                        for _ in range(8)]
                for pr in range(F // 256):
                    wt = wp.tile([P, 2, C], FP8, tag="wot", name="wot")
                    nc.sync.dma_start(out=wt[:], in_=w_ot3[:, pr, :, :])
                    for tc4 in range(4):
                        t8 = half * 4 + tc4
                        lh = h8[:, 2 * pr:2 * pr + 2, t8 * P:(t8 + 1) * P]
                        for ft in range(2):
                            nc.tensor.matmul(opss[tc4 * 2 + ft][:], lhsT=lh,
                                             rhs=wt[:, :, ft * N:(ft + 1) * N],
                                             start=(pr == 0), stop=(pr == F // 256 - 1),
                                             perf_mode=DR)
                for tc4 in range(4):
                    t8 = half * 4 + tc4
                    ot = sp.tile([P, C], F32, tag="ot", name="ot")
                    for ft in range(2):
                        nc.vector.tensor_scalar(ot[:, ft * N:(ft + 1) * N],
                                                opss[tc4 * 2 + ft][:],
                                                1.0 / SWO, None, OP.mult)
                    nc.vector.tensor_tensor(out=ot[:], in0=ot[:],
                                            in1=x_mid[t8][:], op=OP.add)
                    nc.vector.tensor_tensor(out=ot[:], in0=ot[:], in1=bot_sb[:], op=OP.add)
                    nc.sync.dma_start(out=out_d[t8 * P:(t8 + 1) * P, :], in_=ot[:])

    nc.finalize()
    return nc


def _own_blocks(s):
    return [0, 1, 2, 3, 12, 13, 14, 15] if s == 0 else list(range(4, 12))


def _prep_shared(inputs):
    f8 = ml_dtypes.float8_e4m3

    def pack_dr(wT, npr, scale):
        # wT: [K, M] (contraction-major); -> [P, npr, 2, M] with
        # [p, pr, hf, m] = scale * wT[pr*256 + hf*128 + p, m]
        K, M = wT.shape
        assert K == npr * 256
        a = (wT * scale).reshape(npr, 2, P, M).transpose(2, 0, 1, 3)
        return np.ascontiguousarray(a).astype(f8)

    W_attn = np.asarray(inputs["W_attn"], np.float32)
    shared = {
        "w_qk3": pack_dr(np.ascontiguousarray(W_attn[:2 * C].T), NPR, SW),
        "w_v3": pack_dr(np.ascontiguousarray(W_attn[2 * C:].T), NPR, SW),
        "w_pj3": pack_dr(np.ascontiguousarray(np.asarray(inputs["W_proj"], np.float32).T), NPR, SW),
        "w_fc3": pack_dr(np.ascontiguousarray(np.asarray(inputs["W_fc"], np.float32).T), NPR, SW),
        "w_ot3": pack_dr(np.ascontiguousarray(np.asarray(inputs["W_out"], np.float32).T), F // 256, SWO),
        "ln1w": np.ascontiguousarray(np.broadcast_to(np.asarray(inputs["ln1_w"], np.float32), (P, C))),
        "ln1b": np.ascontiguousarray(np.broadcast_to(np.asarray(inputs["ln1_b"], np.float32), (P, C))),
        "ln2w": np.ascontiguousarray(np.broadcast_to(np.asarray(inputs["ln2_w"], np.float32), (P, C))),
        "ln2b": np.ascontiguousarray(np.broadcast_to(np.asarray(inputs["ln2_b"], np.float32), (P, C))),
        "b_q": np.ascontiguousarray(np.asarray(inputs["b_attn"], np.float32)[:C].reshape(CC, P).T),
        "b_k": np.ascontiguousarray(np.asarray(inputs["b_attn"], np.float32)[C:2 * C].reshape(CC, P).T),
        "b_v": np.ascontiguousarray(np.asarray(inputs["b_attn"], np.float32)[2 * C:].reshape(CC, P).T) * SY,
        "b_pj": np.ascontiguousarray(np.broadcast_to(np.asarray(inputs["b_proj"], np.float32), (P, C))),
        "b_fc": np.ascontiguousarray(np.asarray(inputs["b_fc"], np.float32).reshape(F // P, P).T),
        "b_ot": np.ascontiguousarray(np.broadcast_to(np.asarray(inputs["b_out"], np.float32), (P, C))),
    }
    # mask4[p, m*N + qf] = 1 if qf >= m*128 + p else 0
    pp = np.arange(P)[:, None]
    qf = np.arange(N)[None, :]
    mask = np.zeros((P, 4, 2, N), np.float32)
    for m in range(4):
        mask[:, m, 0, :] = (qf >= m * P + pp)
        mask[:, m, 1, :] = mask[:, m, 0, :]
    shared["mask4"] = mask.astype(ml_dtypes.bfloat16)
    sel = np.zeros((33, P), np.float32)
    sel[0, :64] = SY
    sel[32, 64:] = SY
    shared["sel2"] = sel
    shared["ones33"] = np.ones((33, N), np.float32)
    return shared


def _make_in_maps(inputs):
    x = np.asarray(inputs["x"], np.float32)
    shared = _prep_shared(inputs)
    in_maps = []
    for c in range(8):
        b, s = c // 2, c % 2
        own = _own_blocks(s)
        other = _own_blocks(1 - s)
        xb = x[b].reshape(16, P, C)
        m = dict(shared)
        m["x_seq"] = np.ascontiguousarray(
            np.concatenate([xb[own], xb[other]], axis=0).reshape(T2, C))
        m["gate2"] = np.full((P, 1), 0.0 if s == 1 else -1e30, np.float32)
        m["gate3"] = np.full((P, 1), 0.0 if s == 0 else -1e30, np.float32)
        in_maps.append(m)
    return in_maps


def _get_nc(apply_lnwb=True):
    key = ("nc", apply_lnwb, SIM_GELU)
    if key not in _CACHE:
        _CACHE[key] = _build_nc(apply_lnwb)
    return _CACHE[key]


def run_cores(inputs, profile=False):
    """Run the SPMD program; returns list of per-core result dicts."""
    global last_exec_time_ns
    apply_lnwb = not (
        np.allclose(np.asarray(inputs["ln1_w"]), 1.0)
        and np.allclose(np.asarray(inputs["ln1_b"]), 0.0)
        and np.allclose(np.asarray(inputs["ln2_w"]), 1.0)
        and np.allclose(np.asarray(inputs["ln2_b"]), 0.0))
    nc = _get_nc(apply_lnwb)
    in_maps = _make_in_maps(inputs)
    if profile:
        import concourse.bass_utils as bass_utils
        bass_utils.upload_artifacts = lambda tmpdir: "local://" + tmpdir
        try:
            from trn_agent_boot.trn_boot import _ntff_profile_via_ctypes
            import antenv.axon_hooks as hooks
            if hooks.get_axon_ntff_profile_hook() is None:
                hooks.set_axon_ntff_profile_hook(
                    _ntff_profile_via_ctypes("/opt/axon/libaxon_pjrt.so"))
        except Exception:
            pass
        res = bass_utils.run_bass_kernel_spmd(nc, in_maps, list(range(8)), trace=True)
        last_exec_time_ns = res.exec_time_ns
        return res.results
    return _cached_runner(nc)(in_maps)


def _cached_runner(nc):
    """Per-process cached jit of the SPMD executable so repeated kernel()
    calls don't recompile (mirrors bass2jax.run_bass_via_pjrt's multi-core
    branch)."""
    key = ("runner", id(nc))
    if key in _CACHE:
        return _CACHE[key]
    import jax
    import numpy as _np
    from jax.sharding import Mesh, PartitionSpec
    from jax.experimental.shard_map import shard_map
    from concourse import bass2jax, mybir as _mybir
    bass2jax.install_neuronx_cc_hook()

    part_name = nc.partition_id_tensor.name if nc.partition_id_tensor else None
    in_names, out_names, out_avals, zero_outs = [], [], [], []
    for alloc in nc.m.functions[0].allocations:
        if not isinstance(alloc, _mybir.MemoryLocationSet):
            continue
        name = alloc.memorylocations[0].name
        if alloc.kind == "ExternalInput":
            if name != part_name:
                in_names.append(name)
        elif alloc.kind == "ExternalOutput":
            out_names.append(name)
            shape = tuple(alloc.tensor_shape)
            dtype = _mybir.dt.np(alloc.dtype)
            out_avals.append(jax.core.ShapedArray(shape, dtype))
            zero_outs.append(_np.zeros(shape, dtype))
    n_params = len(in_names)
    all_names = in_names + out_names
    if part_name is not None:
        all_names = all_names + [part_name]
    donate = tuple(range(n_params, n_params + len(out_names)))
    if jax.default_backend() == "cpu":
        donate = ()  # cpu sim path can't alias donated outputs

    def _body(*args):
        operands = list(args)
        if part_name is not None:
            operands.append(bass2jax.partition_id_tensor())
        outs = bass2jax._bass_exec_p.bind(
            *operands, out_avals=tuple(out_avals), in_names=tuple(all_names),
            out_names=tuple(out_names), lowering_input_output_aliases=(),
            sim_require_finite=True, sim_require_nnan=True, nc=nc)
        return tuple(outs)

    devices = jax.devices()[:8]
    mesh = Mesh(_np.asarray(devices), ("core",))
    spec = (PartitionSpec("core"),) * (n_params + len(out_names))
    sharded = jax.jit(
        shard_map(_body, mesh=mesh, in_specs=spec,
                  out_specs=(PartitionSpec("core"),) * len(out_names),
                  check_rep=False),
        donate_argnums=donate, keep_unused=True)

    def run(in_maps):
        concat_in = [
            _np.concatenate([_np.asarray(in_maps[c][nm]) for c in range(8)], axis=0)
            for nm in in_names]
        concat_zero = [_np.zeros((8 * z.shape[0], *z.shape[1:]), z.dtype)
                       for z in zero_outs]
        out_arrs = sharded(*concat_in, *concat_zero)
        return [
            {nm: _np.asarray(out_arrs[i]).reshape(8, *out_avals[i].shape)[c]
             for i, nm in enumerate(out_names)}
            for c in range(8)]

    _CACHE[key] = run
    return run


def kernel(**inputs) -> np.ndarray:
    results = run_cores(inputs, profile=PROFILE)
    out = np.empty((B, T, C), np.float32)
    for c in range(8):
        b, s = c // 2, c % 2
        res = results[c]["out"]
        for j, blk in enumerate(_own_blocks(s)):
            out[b, blk * P:(blk + 1) * P, :] = res[j * P:(j + 1) * P]
    return out


# revision 47
# speedup vs baseline: 1.2159x; 1.2159x over previous
"""Trainium2 Bass kernel for a GPT-2 style transformer block.

Problem: B=4, T=2048, C=1024, H=16 heads (hd=64), MLP hidden 4096, fp32 I/O.

Sharding: zero-collective 8-way data parallel. Core c handles batch b=c//2;
s=c%2 selects its query set: s=0 owns the OUTER sequence quarters (blocks
0-3 and 12-15 of 128 tokens), s=1 the MIDDLE half (blocks 4-11). This makes
the causal-attention work symmetric across the pair: a uniform 24-tile
slot schedule per head covers both cores' needs, with per-core host-side
mask / gate tables providing the divergence. K/V are computed locally for
all 2048 tokens in own-first order.

Precision: all big weight matmuls (QKV, V, proj, FC, out) run in fp8e4
DoubleRow perf mode (2 contraction rows per PE cell per cycle): weights are
host-prescaled by 64 (W_out by 256) so N(0, 0.02)-scale values land in
e4m3's normal range; the scale is removed in the PSUM evacuation ops.
Attention q/k/v/exp-weights are fp8e4 as well (no DoubleRow; contraction is
only 64/128 deep), scores accumulate in f32 PSUM and softmax runs in f32 on
ScalarE. LayerNorm is f32 (bn_stats), residuals bf16/f32.

Layouts:
  x / residuals / final out: token-major [tok(P), C]
  x_ln transposed to feature-major [feat(P), chunk, tok] fp8 via PE
  transposes (bf16) + DVE convert-copy
  Q [feat(P), TQ], K [feat(P), T2] fp8; head pair hp lives in one tile
  (rows 0-63 head 2hp, 64-127 head 2hp+1) so score matmuls of a pair are
  emitted adjacently and run CONCURRENTLY in distinct PE row-groups
  V token-major [tok(P), h*65] fp8 with a built-in ones column per head
  (softmax row sums ride the AV matmul); softmax needs no max-subtraction
  (scores bounded ~|s|<4) and no transposes anywhere in attention
  normalization + v-bias deferred to after AV via a tiny K=2 f32r selector
  matmul that partition-broadcasts 16/sums
"""

import os
import sys
import types

import numpy as np
import ml_dtypes

for _p in ("/opt/trn_rl_repo", "/root/.axon_site/_ro/trn_rl_repo"):
    if os.path.isdir(_p) and _p not in sys.path:
        sys.path.append(_p)

# antenv.axon_hooks is absent in this image; bass_utils imports it when
# tracing under axon. Provide the trivial get/set holder it expects.
if "antenv.axon_hooks" not in sys.modules:
    try:
        import antenv

        _m = types.ModuleType("antenv.axon_hooks")
        _m._hook = None

        def _set_hook(h):
            _m._hook = h

        def _get_hook():
            return _m._hook

        _m.set_axon_ntff_profile_hook = _set_hook
        _m.get_axon_ntff_profile_hook = _get_hook
        sys.modules["antenv.axon_hooks"] = _m
        antenv.axon_hooks = _m
    except ImportError:
        pass

import concourse.bacc as bacc
import concourse.tile as tile
from concourse import mybir
from concourse.masks import make_identity

P = 128
B, T, C = 4, 2048, 1024
H, HD = 16, 64
F = 4096
T2 = T  # tokens per core for K/V (full sequence of one batch element)
TQ = T // 2  # own query tokens per core
CC = C // P  # 8 C-chunks
NPR = CC // 2  # 4 DoubleRow contraction pair-chunks (256 each)
N = 512  # moving free dim per matmul

SW = 64.0  # fp8 weight prescale (qkv/v/proj/fc)
SWO = 256.0  # fp8 weight prescale for W_out
SY = 16.0  # attention-output prescale into fp8

F32 = mybir.dt.float32
F32R = mybir.dt.float32r
BF16 = mybir.dt.bfloat16
FP8 = mybir.dt.float8e4
AF = mybir.ActivationFunctionType
OP = mybir.AluOpType
DR = mybir.MatmulPerfMode.DoubleRow

PROFILE = False
SIM_GELU = False  # CoreSim lacks the Gelu LUT; emulate with Tanh + DVE ops
last_exec_time_ns = None

_CACHE = {}

# per-(g,kt) attention slot schedule, uniform across cores.
# kinds: 'diag' (mask m), 'full', 'gate2' (live iff s==1), 'gate3' (iff s==0)
SLOTS0 = [(8, "gate2", 0), (9, "gate2", 0), (10, "gate2", 0), (11, "gate2", 0),
          (0, "diag", 0), (1, "diag", 1), (2, "diag", 2), (3, "diag", 3)]
SLOTS1 = [(0, "full", 0), (1, "full", 0), (2, "full", 0), (3, "full", 0),
          (4, "diag", 0), (5, "diag", 1), (6, "diag", 2), (7, "diag", 3),
          (8, "full", 0), (9, "full", 0), (10, "full", 0), (11, "full", 0),
          (12, "gate3", 0), (13, "gate3", 0), (14, "gate3", 0), (15, "gate3", 0)]
SLOTS = (SLOTS0, SLOTS1)


def _build_nc(apply_lnwb: bool = True):
    nc = bacc.Bacc("TRN2", target_bir_lowering=False, debug=False, num_devices=8)

    x_seq = nc.dram_tensor("x_seq", [T2, C], F32, kind="ExternalInput")
    w_qk3 = nc.dram_tensor("w_qk3", [P, NPR, 2, 2 * C], FP8, kind="ExternalInput")
    w_v3 = nc.dram_tensor("w_v3", [P, NPR, 2, C], FP8, kind="ExternalInput")
    w_pj3 = nc.dram_tensor("w_pj3", [P, NPR, 2, C], FP8, kind="ExternalInput")
    w_fc3 = nc.dram_tensor("w_fc3", [P, NPR, 2, F], FP8, kind="ExternalInput")
    w_ot3 = nc.dram_tensor("w_ot3", [P, F // 256, 2, C], FP8, kind="ExternalInput")
    ln1w = nc.dram_tensor("ln1w", [P, C], F32, kind="ExternalInput")
    ln1b = nc.dram_tensor("ln1b", [P, C], F32, kind="ExternalInput")
    ln2w = nc.dram_tensor("ln2w", [P, C], F32, kind="ExternalInput")
    ln2b = nc.dram_tensor("ln2b", [P, C], F32, kind="ExternalInput")
    b_q = nc.dram_tensor("b_q", [P, CC], F32, kind="ExternalInput")  # x64
    b_k = nc.dram_tensor("b_k", [P, CC], F32, kind="ExternalInput")  # x64
    b_v = nc.dram_tensor("b_v", [P, CC], F32, kind="ExternalInput")  # x16, col per chunk
    b_pj = nc.dram_tensor("b_pj", [P, C], F32, kind="ExternalInput")
    b_fc = nc.dram_tensor("b_fc", [P, F // P], F32, kind="ExternalInput")
    b_ot = nc.dram_tensor("b_ot", [P, C], F32, kind="ExternalInput")
    mask4 = nc.dram_tensor("mask4", [P, 4, 2, N], BF16, kind="ExternalInput")
    gate2 = nc.dram_tensor("gate2", [P, 1], F32, kind="ExternalInput")
    gate3 = nc.dram_tensor("gate3", [P, 1], F32, kind="ExternalInput")
    sel2 = nc.dram_tensor("sel2", [2, P], F32R, kind="ExternalInput")

    out_d = nc.dram_tensor("out", [TQ, C], F32, kind="ExternalOutput")
    sums_d = nc.dram_tensor("sums_scratch", [16, TQ], F32)

    from contextlib import ExitStack

    with tile.TileContext(nc) as tc, ExitStack() as ctx:
        # pool enter order = reverse of close order (pool stack is LIFO);
        # SBUF is reserved from first tile creation to pool close
        const = ctx.enter_context(tc.tile_pool(name="const", bufs=1))
        p_big = ctx.enter_context(tc.tile_pool(name="p_big", bufs=1))
        es_mid = ctx.enter_context(ExitStack())
        es_x2 = ctx.enter_context(ExitStack())
        es_wpj = ctx.enter_context(ExitStack())
        es_y8 = ctx.enter_context(ExitStack())
        es_y = ctx.enter_context(ExitStack())
        es_qk = ctx.enter_context(ExitStack())
        es_v = ctx.enter_context(ExitStack())
        es_wv = ctx.enter_context(ExitStack())

        ident = const.tile([P, P], BF16, tag="ident", name="ident")
        make_identity(nc, ident)
        eps_sb = const.tile([P, 1], F32, tag="eps", name="eps")
        nc.vector.memset(eps_sb[:], 1e-5)
        mask_sb = const.tile([P, 4, 2, N], BF16, tag="mask", name="mask")
        nc.sync.dma_start(out=mask_sb[:], in_=mask4[:])
        gate2_sb = const.tile([P, 1], F32, tag="g2", name="g2")
        nc.sync.dma_start(out=gate2_sb[:], in_=gate2[:])
        gate3_sb = const.tile([P, 1], F32, tag="g3", name="g3")
        nc.sync.dma_start(out=gate3_sb[:], in_=gate3[:])
        sel_sb = const.tile([2, P], F32R, tag="sel", name="sel")
        nc.sync.dma_start(out=sel_sb[:], in_=sel2[:])
        bq_sb = const.tile([P, CC], F32, tag="bq", name="bq")
        nc.sync.dma_start(out=bq_sb[:], in_=b_q[:])
        bk_sb = const.tile([P, CC], F32, tag="bk", name="bk")
        nc.sync.dma_start(out=bk_sb[:], in_=b_k[:])
        bv_sb = const.tile([P, CC], F32, tag="bv", name="bv")
        nc.sync.dma_start(out=bv_sb[:], in_=b_v[:])
        bfc_sb = const.tile([P, F // P], F32, tag="bfc", name="bfc")
        nc.sync.dma_start(out=bfc_sb[:], in_=b_fc[:])
        bpj_sb = const.tile([P, C], F32, tag="bpj", name="bpj")
        nc.sync.dma_start(out=bpj_sb[:], in_=b_pj[:])
        bot_sb = const.tile([P, C], F32, tag="bot", name="bot")
        nc.sync.dma_start(out=bot_sb[:], in_=b_ot[:])

        p_mid = es_mid.enter_context(tc.tile_pool(name="p_mid", bufs=1))
        p_x2 = es_x2.enter_context(tc.tile_pool(name="p_x2", bufs=1))
        p_wpj = es_wpj.enter_context(tc.tile_pool(name="p_wpj", bufs=1))
        p_y8 = es_y8.enter_context(tc.tile_pool(name="p_y8", bufs=1))
        p_y = es_y.enter_context(tc.tile_pool(name="p_y", bufs=1))
        p_qk = es_qk.enter_context(tc.tile_pool(name="p_qk", bufs=1))
        p_v = es_v.enter_context(tc.tile_pool(name="p_v", bufs=1))
        p_wv = es_wv.enter_context(tc.tile_pool(name="p_wv", bufs=1))

        # one 32 KiB/partition fp8 buffer triple-aliased across disjoint
        # lifetimes: [xlnT | wqk] (phases 1-3) then h8 (phases 7-8)
        buf32 = p_big.tile([P, 2 * CC * T2], FP8, tag="buf32", name="buf32")
        xlnT = buf32.rearrange("p (a c t) -> p a c t", a=2, t=T2)[:, 0]
        wqk_sb = buf32.rearrange("p (a pr hf f) -> p a pr hf f",
                                 a=2, pr=NPR, hf=2)[:, 1]
        h8 = buf32.rearrange("p (f t) -> p f t", t=TQ)
        wv_sb = p_wv.tile([P, NPR, 2, C], FP8, tag="wv", name="wv")
        wpj_sb = p_wpj.tile([P, NPR, 2, C], FP8, tag="wpj", name="wpj")

        # ---------------- Phase 1: LN1 + transpose + K (fused) --------------
        # K matmuls for token-block pairs are emitted as soon as their
        # transposes land, filling the PE during the DVE/ACT-bound LN loop.
        # Weight-stationary over 2 blocks so each 256-col DoubleRow weight
        # load amortizes over 2 matmuls; evacuations ((psum+64b)/64 -> bf16)
        # run on ScalarE (idle here) via the free affine: ps/64 + b_true.
        nc.sync.dma_start(out=wqk_sb[:], in_=w_qk3[:])
        nc.sync.dma_start(out=wv_sb[:], in_=w_v3[:])
        nc.sync.dma_start(out=wpj_sb[:], in_=w_pj3[:])
        q_sb = [p_qk.tile([P, TQ], BF16, tag=f"q{i}", name=f"q{i}") for i in range(CC)]
        k_sb = [p_qk.tile([P, T2], BF16, tag=f"k{i}", name=f"k{i}") for i in range(CC)]

        def emit_k(bp, kps):
            for fc in range(CC):
                psK = [kps.tile([P, N], F32, tag="k_ps", name="k_ps") for _ in range(2)]
                for pr in range(NPR):
                    lk = wqk_sb[:, pr, :, C + fc * P:C + (fc + 1) * P]
                    for bi in range(2):
                        blk = 2 * bp + bi
                        nc.tensor.matmul(psK[bi][:], lhsT=lk,
                                         rhs=xlnT[:, 2 * pr:2 * pr + 2, blk * N:(blk + 1) * N],
                                         start=(pr == 0), stop=(pr == NPR - 1),
                                         perf_mode=DR)
                for bi in range(2):
                    blk = 2 * bp + bi
                    nc.scalar.activation(k_sb[fc][:, blk * N:(blk + 1) * N],
                                         psK[bi][:], AF.Identity,
                                         bias=bk_sb[:, fc:fc + 1], scale=1.0 / SW)

        with tc.tile_pool(name="qk_ps", bufs=4, space="PSUM") as kps:
            with tc.tile_pool(name="ln1_sp", bufs=3) as sp, \
                 tc.tile_pool(name="ln1_cp", bufs=1) as cp, \
                 tc.tile_pool(name="ln1_st", bufs=6) as st, \
                 tc.tile_pool(name="ln1_ps", bufs=3, space="PSUM") as tps:
                if apply_lnwb:
                    w1 = cp.tile([P, C], F32, tag="w1", name="w1")
                    nc.sync.dma_start(out=w1[:], in_=ln1w[:])
                    b1 = cp.tile([P, C], F32, tag="b1", name="b1")
                    nc.sync.dma_start(out=b1[:], in_=ln1b[:])
                for tt in range(T2 // P):
                    xt = sp.tile([P, C], F32, tag="xs", name="xs")
                    nc.sync.dma_start(out=xt[:], in_=x_seq[tt * P:(tt + 1) * P, :])
                    stats = st.tile([P, 2, 6], F32, tag="st", name="st")
                    for g in range(2):
                        nc.vector.bn_stats(out=stats[:, g, :], in_=xt[:, g * 512:(g + 1) * 512])
                    mv = st.tile([P, 2], F32, tag="mv", name="mv")
                    nc.vector.bn_aggr(out=mv[:], in_=stats[:])
                    rstd = st.tile([P, 1], F32, tag="rstd", name="rstd")
                    nc.scalar.activation(rstd[:], mv[:, 1:2], AF.Sqrt, bias=eps_sb[:], scale=1.0)
                    nc.vector.reciprocal(out=rstd[:], in_=rstd[:])
                    xb = sp.tile([P, C], BF16, tag="xb", name="xb")
                    nmr = st.tile([P, 1], F32, tag="nmr", name="nmr")
                    nc.vector.tensor_scalar(nmr[:], mv[:, 0:1], rstd[:], -1.0,
                                            OP.mult, OP.mult)
                    if apply_lnwb:
                        xc = sp.tile([P, C], F32, tag="xc", name="xc")
                        nc.scalar.activation(xc[:], xt[:], AF.Identity,
                                             bias=nmr[:], scale=rstd[:])
                        xw = sp.tile([P, C], F32, tag="xw", name="xw")
                        nc.vector.tensor_tensor(out=xw[:], in0=xc[:], in1=w1[:], op=OP.mult)
                        nc.vector.tensor_tensor(out=xb[:], in0=xw[:], in1=b1[:], op=OP.add)
                    else:
                        nc.scalar.activation(xb[:], xt[:], AF.Identity,
                                             bias=nmr[:], scale=rstd[:])
                    for gr in range(2):
                        pst = tps.tile([P, 4, P], BF16, tag="trp", name="trp")
                        for j in range(4):
                            cc = gr * 4 + j
                            nc.tensor.transpose(pst[:, j, :],
                                                xb[:, cc * P:(cc + 1) * P], ident[:])
                        nc.vector.tensor_copy(
                            out=xlnT[:, gr * 4:(gr + 1) * 4, tt * P:(tt + 1) * P],
                            in_=pst[:])
                    if tt == 7:
                        emit_k(0, kps)
            emit_k(1, kps)

        # ---------------- Phase 2: Q projection (DoubleRow fp8) -------------
        with tc.tile_pool(name="q_ps", bufs=4, space="PSUM") as qps:
            for fc in range(CC):
                psQ = [qps.tile([P, N], F32, tag="q_ps", name="q_ps") for _ in range(2)]
                for pr in range(NPR):
                    lq = wqk_sb[:, pr, :, fc * P:(fc + 1) * P]
                    for blk in range(2):
                        nc.tensor.matmul(psQ[blk][:], lhsT=lq,
                                         rhs=xlnT[:, 2 * pr:2 * pr + 2, blk * N:(blk + 1) * N],
                                         start=(pr == 0), stop=(pr == NPR - 1),
                                         perf_mode=DR)
                for blk in range(2):
                    nc.scalar.activation(q_sb[fc][:, blk * N:(blk + 1) * N],
                                         psQ[blk][:], AF.Identity,
                                         bias=bq_sb[:, fc:fc + 1], scale=1.0 / SW)

        # ---------------- Phase 2b: V projection (DoubleRow fp8) ------------
        v_sb = [p_v.tile([P, H * 65], FP8, tag=f"v{i}", name=f"v{i}")
                for i in range(T2 // P)]
        for kt in range(T2 // P):
            nc.gpsimd.memset(
                v_sb[kt].rearrange("p (h d) -> p h d", d=65)[:, :, 64:65], 1.0)
        with tc.tile_pool(name="v_ps", bufs=4, space="PSUM") as vps:
            for kt in range(T2 // P):
                ps = [vps.tile([P, N], F32, tag="v_ps", name="v_ps") for _ in range(2)]
                for pr in range(NPR):
                    lv = xlnT[:, 2 * pr:2 * pr + 2, kt * P:(kt + 1) * P]
                    for vg in range(2):
                        nc.tensor.matmul(ps[vg][:], lhsT=lv,
                                         rhs=wv_sb[:, pr, :, vg * N:(vg + 1) * N],
                                         start=(pr == 0), stop=(pr == NPR - 1),
                                         perf_mode=DR)
                for vg in range(2):
                    out_ap = v_sb[kt].rearrange("p (h d) -> p h d", d=65)[
                        :, vg * 8:(vg + 1) * 8, 0:64]
                    in_ap = ps[vg].rearrange("p (h d) -> p h d", d=64)[:, :, :]
                    nc.scalar.activation(out_ap, in_ap, AF.Identity,
                                         bias=0.0, scale=1.0 / SW)

        # ---------------- Phase 3: attention -------------------------------
        # Per head-pair: 24 causal slots/qb-group; score matmul pair runs
        # concurrently in PE row-groups 0-63/64-127. Each qb group leads with
        # its maskless slots so the previous group's DVE evacuation tail
        # drains before the first causal-mask multiply is needed.
        y_fm = [p_y.tile([P, TQ], BF16, tag=f"y{i}", name=f"y{i}") for i in range(CC)]
        with tc.tile_pool(name="att_at", bufs=1) as ap_pool, \
             tc.tile_pool(name="att_sps", bufs=2, space="PSUM") as sps_pool, \
             tc.tile_pool(name="att_yps", bufs=2, space="PSUM") as yps_pool:
            for hp in range(H // 2):
                for qb in (0, 1):
                    slots = SLOTS[qb]
                    yps = [yps_pool.tile([65, N], F32, tag="yps", name="yps")
                           for _ in range(2)]
                    last = len(slots) - 1
                    for i, (kt, kind, m) in enumerate(slots):
                        sp = sps_pool.tile([P, 2, N], F32, tag="sps", name="sps")
                        for j in range(2):
                            ro = j * 64
                            nc.tensor.matmul(
                                sp[:, j, :],
                                lhsT=k_sb[hp][ro:ro + 64, kt * P:(kt + 1) * P],
                                rhs=q_sb[hp][ro:ro + 64, qb * N:(qb + 1) * N],
                                start=True, stop=True)
                        at = ap_pool.tile([P, 2, N], FP8, tag="at", name="at", bufs=8)
                        bias = {"diag": 0.0, "full": 0.0,
                                "gate2": gate2_sb[:, 0:1],
                                "gate3": gate3_sb[:, 0:1]}[kind]
                        nc.scalar.activation(at[:, 0:2, :], sp[:, 0:2, :],
                                             AF.Exp, bias=bias, scale=0.125)
                        if kind == "diag":
                            nc.vector.tensor_tensor(
                                out=at[:, 0:2, :], in0=at[:, 0:2, :],
                                in1=mask_sb[:, m, :, :], op=OP.mult)
                        for j in range(2):
                            h = 2 * hp + j
                            nc.tensor.matmul(yps[j][:],
                                             lhsT=v_sb[kt][:, h * 65:(h + 1) * 65],
                                             rhs=at[:, j, :],
                                             start=(i == 0), stop=(i == last))
                    for j in range(2):
                        nc.vector.tensor_copy(
                            out=y_fm[hp][j * 64:(j + 1) * 64, qb * N:(qb + 1) * N],
                            in_=yps[j][0:64, :])
                        s1 = ap_pool.tile([1, N], F32, tag="s1", name="s1", bufs=4)
                        nc.vector.tensor_copy(out=s1[:], in_=yps[j][64:65, :])
                        nc.sync.dma_start(
                            out=sums_d[2 * hp + j:2 * hp + j + 1, qb * N:(qb + 1) * N],
                            in_=s1[:])

        # ---------------- Phase 4: normalize y -> y8 (x SY, + SY*b_v) ------
        # one lane-parallel reciprocal over all 16 heads' sums; partition
        # repack via DRAM roundtrip (engine partition bases are 0/32/64 only)
        y8 = p_y8.tile([P, CC, TQ], FP8, tag="y8", name="y8")
        with tc.tile_pool(name="att_rp", bufs=2) as rp, \
             tc.tile_pool(name="att_rps", bufs=2, space="PSUM") as rps_pool:
            s16 = rp.tile([16, TQ], F32, tag="s16", name="s16", bufs=1)
            nc.sync.dma_start(out=s16[:], in_=sums_d[:])
            recip16 = rp.tile([16, TQ], F32, tag="recip16", name="recip16", bufs=1)
            nc.vector.reciprocal(out=recip16[:], in_=s16[:])
            reciprr = rp.tile([16, TQ], F32R, tag="reciprr", name="reciprr", bufs=1)
            with nc.allow_low_precision(reason="f32r view of f32 recip"):
                nc.vector.tensor_scalar(reciprr[:], recip16[:], SY, None, OP.mult)
            for yt in range(CC):
                recip_r = rp.tile([2, TQ], F32R, tag="recipr", name="recipr", bufs=4)
                nc.sync.dma_start(out=recip_r[:], in_=reciprr[2 * yt:2 * yt + 2, :])
                for tt in range(2):
                    rps = rps_pool.tile([P, N], F32, tag="rps", name="rps")
                    nc.tensor.matmul(rps[:], lhsT=sel_sb[:],
                                     rhs=recip_r[:, tt * N:(tt + 1) * N],
                                     start=True, stop=True)
                    nc.vector.tensor_tensor(out=y8[:, yt, tt * N:(tt + 1) * N],
                                            in0=y_fm[yt][:, tt * N:(tt + 1) * N],
                                            in1=rps[:], op=OP.mult)
                    nc.vector.tensor_scalar(y8[:, yt, tt * N:(tt + 1) * N],
                                            y8[:, yt, tt * N:(tt + 1) * N],
                                            bv_sb[:, yt:yt + 1], None, OP.add)

        es_wv.close()
        es_v.close()
        es_qk.close()
        es_y.close()

        # ---------------- Phase 5: proj + residual + LN2 (fused) ------------
        # proj evacuation, residual add, LN2 and its transposes all run
        # per-128-token tile so PE pipelines the next tile's proj matmuls
        # under this tile's DVE/ACT work.
        x_mid = [p_mid.tile([P, C], BF16, tag=f"xm{i}", name=f"xm{i}")
                 for i in range(TQ // P)]
        xln2T = p_x2.tile([P, CC, TQ], FP8, tag="xln2T", name="xln2T")
        with tc.tile_pool(name="pj_sp", bufs=3) as sp, \
             tc.tile_pool(name="pj_cp", bufs=1) as cp, \
             tc.tile_pool(name="pj_st", bufs=6) as st, \
             tc.tile_pool(name="pj_ps", bufs=2, space="PSUM") as pps, \
             tc.tile_pool(name="ln2_ps", bufs=3, space="PSUM") as tps:
            if apply_lnwb:
                w2 = cp.tile([P, C], F32, tag="w2", name="w2")
                nc.sync.dma_start(out=w2[:], in_=ln2w[:])
                b2 = cp.tile([P, C], F32, tag="b2", name="b2")
                nc.sync.dma_start(out=b2[:], in_=ln2b[:])
            for t8 in range(TQ // P):
                xo = sp.tile([P, C], F32, tag="xo", name="xo")
                nc.sync.dma_start(out=xo[:], in_=x_seq[t8 * P:(t8 + 1) * P, :])
                ps2 = pps.tile([P, 2, N], F32, tag="pj_ps", name="pj_ps")
                for pr in range(NPR):
                    ly = y8[:, 2 * pr:2 * pr + 2, t8 * P:(t8 + 1) * P]
                    for ft in range(2):
                        nc.tensor.matmul(ps2[:, ft, :], lhsT=ly,
                                         rhs=wpj_sb[:, pr, :, ft * N:(ft + 1) * N],
                                         start=(pr == 0), stop=(pr == NPR - 1),
                                         perf_mode=DR)
                xt = x_mid[t8]
                nc.scalar.activation(xt[:], ps2[:, 0:2, :], AF.Identity,
                                     bias=0.0, scale=1.0 / (SW * SY))
                nc.vector.tensor_tensor(out=xt[:], in0=xt[:], in1=xo[:], op=OP.add)
                nc.vector.tensor_tensor(out=xt[:], in0=xt[:], in1=bpj_sb[:], op=OP.add)
                # --- LN2 for this token tile ---
                stats = st.tile([P, 2, 6], F32, tag="st2", name="st2")
                for g in range(2):
                    nc.vector.bn_stats(out=stats[:, g, :], in_=xt[:, g * 512:(g + 1) * 512])
                mv = st.tile([P, 2], F32, tag="mv2", name="mv2")
                nc.vector.bn_aggr(out=mv[:], in_=stats[:])
                rstd = st.tile([P, 1], F32, tag="rstd2", name="rstd2")
                nc.scalar.activation(rstd[:], mv[:, 1:2], AF.Sqrt, bias=eps_sb[:], scale=1.0)
                nc.vector.reciprocal(out=rstd[:], in_=rstd[:])
                xb = sp.tile([P, C], BF16, tag="xb2", name="xb2")
                nmr = st.tile([P, 1], F32, tag="nmr2", name="nmr2")
                nc.vector.tensor_scalar(nmr[:], mv[:, 0:1], rstd[:], -1.0,
                                        OP.mult, OP.mult)
                if apply_lnwb:
                    xc = sp.tile([P, C], F32, tag="xc2", name="xc2")
                    nc.scalar.activation(xc[:], xt[:], AF.Identity,
                                         bias=nmr[:], scale=rstd[:])
                    xw = sp.tile([P, C], F32, tag="xw2", name="xw2")
                    nc.vector.tensor_tensor(out=xw[:], in0=xc[:], in1=w2[:], op=OP.mult)
                    nc.vector.tensor_tensor(out=xb[:], in0=xw[:], in1=b2[:], op=OP.add)
                else:
                    nc.scalar.activation(xb[:], xt[:], AF.Identity,
                                         bias=nmr[:], scale=rstd[:])
                for gr in range(2):
                    pst = tps.tile([P, 4, P], BF16, tag="trp2", name="trp2")
                    for j in range(4):
                        cc = gr * 4 + j
                        nc.tensor.transpose(pst[:, j, :],
                                            xb[:, cc * P:(cc + 1) * P], ident[:])
                    nc.vector.tensor_copy(
                        out=xln2T[:, gr * 4:(gr + 1) * 4, t8 * P:(t8 + 1) * P],
                        in_=pst[:])

        es_y8.close()
        es_wpj.close()

        # ---------------- Phase 7: FC + gelu (DoubleRow fp8) ---------------
        with tc.tile_pool(name="fc_w", bufs=3) as wp, \
             tc.tile_pool(name="fc_ps", bufs=3, space="PSUM") as fps:
            for hg in range(F // N):
                wt = wp.tile([P, NPR, 2, N], FP8, tag="wfc", name="wfc")
                nc.sync.dma_start(out=wt[:], in_=w_fc3[:, :, :, hg * N:(hg + 1) * N])
                for hs in range(4):
                    hf = hg * 4 + hs
                    ps2 = fps.tile([P, 2, N], F32, tag="fc_ps", name="fc_ps")
                    for pr in range(NPR):
                        lw = wt[:, pr, :, hs * P:(hs + 1) * P]
                        for tt in range(2):
                            nc.tensor.matmul(ps2[:, tt, :], lhsT=lw,
                                             rhs=xln2T[:, 2 * pr:2 * pr + 2, tt * N:(tt + 1) * N],
                                             start=(pr == 0), stop=(pr == NPR - 1),
                                             perf_mode=DR)
                    if not SIM_GELU:
                        nc.scalar.activation(h8[:, hf, :], ps2[:, 0:2, :],
                                             AF.Gelu_apprx_tanh,
                                             bias=bfc_sb[:, hf:hf + 1], scale=1.0 / SW)
                    else:
                        import math
                        cst = math.sqrt(2.0 / math.pi)
                        u = wp.tile([P, 2, N], F32, tag="g_u", name="g_u")
                        nc.scalar.activation(u[:], ps2[:, 0:2, :], AF.Identity,
                                             bias=bfc_sb[:, hf:hf + 1], scale=1.0 / SW)
                        u3 = wp.tile([P, 2, N], F32, tag="g_u3", name="g_u3")
                        nc.scalar.activation(u3[:], u[:], AF.Square, bias=0.0, scale=1.0)
                        nc.vector.tensor_tensor(out=u3[:], in0=u3[:], in1=u[:], op=OP.mult)
                        nc.vector.tensor_scalar(u3[:], u3[:], 0.044715, None, OP.mult)
                        nc.vector.tensor_tensor(out=u3[:], in0=u3[:], in1=u[:], op=OP.add)
                        tqh = wp.tile([P, 2, N], F32, tag="g_t", name="g_t")
                        nc.scalar.activation(tqh[:], u3[:], AF.Tanh, bias=0.0, scale=cst)
                        nc.vector.tensor_scalar(tqh[:], tqh[:], 1.0, None, OP.add)
                        nc.vector.tensor_tensor(out=tqh[:], in0=tqh[:], in1=u[:], op=OP.mult)
                        nc.vector.tensor_scalar(h8[:, hf, :], tqh[:], 0.5, None, OP.mult)

        es_x2.close()

        # ---------------- Phase 8: out matmul + residual (DoubleRow fp8) ---
        with tc.tile_pool(name="ot_w", bufs=6) as wp, \
             tc.tile_pool(name="ot_sp", bufs=3) as sp, \
             tc.tile_pool(name="ot_ps", bufs=8, space="PSUM") as ops_pool:
            for half in range(2):
                opss = [ops_pool.tile([P, N], F32, tag="ot_ps", name="ot_ps")
                        for _ in range(8)]
                for pr in range(F // 256):
                    wt = wp.tile([P, 2, C], FP8, tag="wot", name="wot")
                    nc.sync.dma_start(out=wt[:], in_=w_ot3[:, pr, :, :])
                    for tc4 in range(4):
                        t8 = half * 4 + tc4
                        lh = h8[:, 2 * pr:2 * pr + 2, t8 * P:(t8 + 1) * P]
                        for ft in range(2):
                            nc.tensor.matmul(opss[tc4 * 2 + ft][:], lhsT=lh,
                                             rhs=wt[:, :, ft * N:(ft + 1) * N],
                                             start=(pr == 0), stop=(pr == F // 256 - 1),
                                             perf_mode=DR)
                for tc4 in range(4):
                    t8 = half * 4 + tc4
                    ot = sp.tile([P, C], F32, tag="ot", name="ot")
                    for ft in range(2):
                        nc.vector.tensor_scalar(ot[:, ft * N:(ft + 1) * N],
                                                opss[tc4 * 2 + ft][:],
                                                1.0 / SWO, None, OP.mult)
                    nc.vector.tensor_tensor(out=ot[:], in0=ot[:],
                                            in1=x_mid[t8][:], op=OP.add)
                    nc.vector.tensor_tensor(out=ot[:], in0=ot[:], in1=bot_sb[:], op=OP.add)
                    nc.sync.dma_start(out=out_d[t8 * P:(t8 + 1) * P, :], in_=ot[:])

    nc.finalize()
    return nc


def _own_blocks(s):
    return [0, 1, 2, 3, 12, 13, 14, 15] if s == 0 else list(range(4, 12))


def _prep_shared(inputs):
    f8 = ml_dtypes.float8_e4m3

    def pack_dr(wT, npr, scale):
        # wT: [K, M] (contraction-major); -> [P, npr, 2, M] with
        # [p, pr, hf, m] = scale * wT[pr*256 + hf*128 + p, m]
        K, M = wT.shape
        assert K == npr * 256
        a = (wT * scale).reshape(npr, 2, P, M).transpose(2, 0, 1, 3)
        return np.ascontiguousarray(a).astype(f8)

    W_attn = np.asarray(inputs["W_attn"], np.float32)
    shared = {
        "w_qk3": pack_dr(np.ascontiguousarray(W_attn[:2 * C].T), NPR, SW),
        "w_v3": pack_dr(np.ascontiguousarray(W_attn[2 * C:].T), NPR, SW),
        "w_pj3": pack_dr(np.ascontiguousarray(np.asarray(inputs["W_proj"], np.float32).T), NPR, SW),
        "w_fc3": pack_dr(np.ascontiguousarray(np.asarray(inputs["W_fc"], np.float32).T), NPR, SW),
        "w_ot3": pack_dr(np.ascontiguousarray(np.asarray(inputs["W_out"], np.float32).T), F // 256, SWO),
        "ln1w": np.ascontiguousarray(np.broadcast_to(np.asarray(inputs["ln1_w"], np.float32), (P, C))),
        "ln1b": np.ascontiguousarray(np.broadcast_to(np.asarray(inputs["ln1_b"], np.float32), (P, C))),
        "ln2w": np.ascontiguousarray(np.broadcast_to(np.asarray(inputs["ln2_w"], np.float32), (P, C))),
        "ln2b": np.ascontiguousarray(np.broadcast_to(np.asarray(inputs["ln2_b"], np.float32), (P, C))),
        "b_q": np.ascontiguousarray(np.asarray(inputs["b_attn"], np.float32)[:C].reshape(CC, P).T),
        "b_k": np.ascontiguousarray(np.asarray(inputs["b_attn"], np.float32)[C:2 * C].reshape(CC, P).T),
        "b_v": np.ascontiguousarray(np.asarray(inputs["b_attn"], np.float32)[2 * C:].reshape(CC, P).T) * SY,
        "b_pj": np.ascontiguousarray(np.broadcast_to(np.asarray(inputs["b_proj"], np.float32), (P, C))),
        "b_fc": np.ascontiguousarray(np.asarray(inputs["b_fc"], np.float32).reshape(F // P, P).T),
        "b_ot": np.ascontiguousarray(np.broadcast_to(np.asarray(inputs["b_out"], np.float32), (P, C))),
    }
    # mask4[p, m*N + qf] = 1 if qf >= m*128 + p else 0
    pp = np.arange(P)[:, None]
    qf = np.arange(N)[None, :]
    mask = np.zeros((P, 4, 2, N), np.float32)
    for m in range(4):
        mask[:, m, 0, :] = (qf >= m * P + pp)
        mask[:, m, 1, :] = mask[:, m, 0, :]
    shared["mask4"] = mask.astype(ml_dtypes.bfloat16)
    sel = np.zeros((2, P), np.float32)
    sel[0, :64] = 1.0
    sel[1, 64:] = 1.0
    shared["sel2"] = sel
    return shared


def _make_in_maps(inputs):
    x = np.asarray(inputs["x"], np.float32)
    shared = _prep_shared(inputs)
    in_maps = []
    for c in range(8):
        b, s = c // 2, c % 2
        own = _own_blocks(s)
        other = _own_blocks(1 - s)
        xb = x[b].reshape(16, P, C)
        m = dict(shared)
        m["x_seq"] = np.ascontiguousarray(
            np.concatenate([xb[own], xb[other]], axis=0).reshape(T2, C))
        m["gate2"] = np.full((P, 1), 0.0 if s == 1 else -1e30, np.float32)
        m["gate3"] = np.full((P, 1), 0.0 if s == 0 else -1e30, np.float32)
        in_maps.append(m)
    return in_maps


def _get_nc(apply_lnwb=True):
    key = ("nc", apply_lnwb, SIM_GELU)
    if key not in _CACHE:
        _CACHE[key] = _build_nc(apply_lnwb)
    return _CACHE[key]


def run_cores(inputs, profile=False):
    """Run the SPMD program; returns list of per-core result dicts."""
    global last_exec_time_ns
    apply_lnwb = not (
        np.allclose(np.asarray(inputs["ln1_w"]), 1.0)
        and np.allclose(np.asarray(inputs["ln1_b"]), 0.0)
        and np.allclose(np.asarray(inputs["ln2_w"]), 1.0)
        and np.allclose(np.asarray(inputs["ln2_b"]), 0.0))
    nc = _get_nc(apply_lnwb)
    in_maps = _make_in_maps(inputs)
    if profile:
        import concourse.bass_utils as bass_utils
        bass_utils.upload_artifacts = lambda tmpdir: "local://" + tmpdir
        try:
            from trn_agent_boot.trn_boot import _ntff_profile_via_ctypes
            import antenv.axon_hooks as hooks
            if hooks.get_axon_ntff_profile_hook() is None:
                hooks.set_axon_ntff_profile_hook(
                    _ntff_profile_via_ctypes("/opt/axon/libaxon_pjrt.so"))
        except Exception:
            pass
        res = bass_utils.run_bass_kernel_spmd(nc, in_maps, list(range(8)), trace=True)
        last_exec_time_ns = res.exec_time_ns
        return res.results
    return _cached_runner(nc)(in_maps)


def _cached_runner(nc):
    """Per-process cached jit of the SPMD executable so repeated kernel()
    calls don't recompile (mirrors bass2jax.run_bass_via_pjrt's multi-core
    branch)."""
    key = ("runner", id(nc))
    if key in _CACHE:
        return _CACHE[key]
    import jax
    import numpy as _np
    from jax.sharding import Mesh, PartitionSpec
    from jax.experimental.shard_map import shard_map
    from concourse import bass2jax, mybir as _mybir
    bass2jax.install_neuronx_cc_hook()

    part_name = nc.partition_id_tensor.name if nc.partition_id_tensor else None
    in_names, out_names, out_avals, zero_outs = [], [], [], []
    for alloc in nc.m.functions[0].allocations:
        if not isinstance(alloc, _mybir.MemoryLocationSet):
            continue
        name = alloc.memorylocations[0].name
        if alloc.kind == "ExternalInput":
            if name != part_name:
                in_names.append(name)
        elif alloc.kind == "ExternalOutput":
            out_names.append(name)
            shape = tuple(alloc.tensor_shape)
            dtype = _mybir.dt.np(alloc.dtype)
            out_avals.append(jax.core.ShapedArray(shape, dtype))
            zero_outs.append(_np.zeros(shape, dtype))
    n_params = len(in_names)
    all_names = in_names + out_names
    if part_name is not None:
        all_names = all_names + [part_name]
    donate = tuple(range(n_params, n_params + len(out_names)))
    if jax.default_backend() == "cpu":
        donate = ()  # cpu sim path can't alias donated outputs

    def _body(*args):
        operands = list(args)
        if part_name is not None:
            operands.append(bass2jax.partition_id_tensor())
        outs = bass2jax._bass_exec_p.bind(
            *operands, out_avals=tuple(out_avals), in_names=tuple(all_names),
            out_names=tuple(out_names), lowering_input_output_aliases=(),
            sim_require_finite=True, sim_require_nnan=True, nc=nc)
        return tuple(outs)

    devices = jax.devices()[:8]
    mesh = Mesh(_np.asarray(devices), ("core",))
    spec = (PartitionSpec("core"),) * (n_params + len(out_names))
    sharded = jax.jit(
        shard_map(_body, mesh=mesh, in_specs=spec,
                  out_specs=(PartitionSpec("core"),) * len(out_names),
                  check_rep=False),
        donate_argnums=donate, keep_unused=True)

    def run(in_maps):
        concat_in = [
            _np.concatenate([_np.asarray(in_maps[c][nm]) for c in range(8)], axis=0)
            for nm in in_names]
        concat_zero = [_np.zeros((8 * z.shape[0], *z.shape[1:]), z.dtype)
                       for z in zero_outs]
        out_arrs = sharded(*concat_in, *concat_zero)
        return [
            {nm: _np.asarray(out_arrs[i]).reshape(8, *out_avals[i].shape)[c]
             for i, nm in enumerate(out_names)}
            for c in range(8)]

    _CACHE[key] = run
    return run


def kernel(**inputs) -> np.ndarray:
    results = run_cores(inputs, profile=PROFILE)
    out = np.empty((B, T, C), np.float32)
    for c in range(8):
        b, s = c // 2, c % 2
        res = results[c]["out"]
        for j, blk in enumerate(_own_blocks(s)):
            out[b, blk * P:(blk + 1) * P, :] = res[j * P:(j + 1) * P]
    return out


# revision 50
# speedup vs baseline: 1.2372x; 1.0175x over previous
"""Trainium2 Bass kernel for a GPT-2 style transformer block.

Problem: B=4, T=2048, C=1024, H=16 heads (hd=64), MLP hidden 4096, fp32 I/O.

Sharding: zero-collective 8-way data parallel. Core c handles batch b=c//2;
s=c%2 selects its query set: s=0 owns the OUTER sequence quarters (blocks
0-3 and 12-15 of 128 tokens), s=1 the MIDDLE half (blocks 4-11). This makes
the causal-attention work symmetric across the pair: a uniform 24-tile
slot schedule per head covers both cores' needs, with per-core host-side
mask / gate tables providing the divergence. K/V are computed locally for
all 2048 tokens in own-first order.

Precision: all big weight matmuls (QKV, V, proj, FC, out) run in fp8e4
DoubleRow perf mode (2 contraction rows per PE cell per cycle): weights are
host-prescaled by 64 (W_out by 256) so N(0, 0.02)-scale values land in
e4m3's normal range; the scale is removed in the PSUM evacuation ops.
Attention q/k/v/exp-weights are fp8e4 as well (no DoubleRow; contraction is
only 64/128 deep), scores accumulate in f32 PSUM and softmax runs in f32 on
ScalarE. LayerNorm is f32 (bn_stats), residuals bf16/f32.

Layouts:
  x / residuals / final out: token-major [tok(P), C]
  x_ln transposed to feature-major [feat(P), chunk, tok] fp8 via PE
  transposes (bf16) + DVE convert-copy
  Q [feat(P), TQ], K [feat(P), T2] fp8; head pair hp lives in one tile
  (rows 0-63 head 2hp, 64-127 head 2hp+1) so score matmuls of a pair are
  emitted adjacently and run CONCURRENTLY in distinct PE row-groups
  V token-major [tok(P), h*65] fp8 with a built-in ones column per head
  (softmax row sums ride the AV matmul); softmax needs no max-subtraction
  (scores bounded ~|s|<4) and no transposes anywhere in attention
  normalization + v-bias deferred to after AV via a tiny K=2 f32r selector
  matmul that partition-broadcasts 16/sums
"""

import os
import sys
import types

import numpy as np
import ml_dtypes

for _p in ("/opt/trn_rl_repo", "/root/.axon_site/_ro/trn_rl_repo"):
    if os.path.isdir(_p) and _p not in sys.path:
        sys.path.append(_p)

# antenv.axon_hooks is absent in this image; bass_utils imports it when
# tracing under axon. Provide the trivial get/set holder it expects.
if "antenv.axon_hooks" not in sys.modules:
    try:
        import antenv

        _m = types.ModuleType("antenv.axon_hooks")
        _m._hook = None

        def _set_hook(h):
            _m._hook = h

        def _get_hook():
            return _m._hook

        _m.set_axon_ntff_profile_hook = _set_hook
        _m.get_axon_ntff_profile_hook = _get_hook
        sys.modules["antenv.axon_hooks"] = _m
        antenv.axon_hooks = _m
    except ImportError:
        pass

import concourse.bacc as bacc
import concourse.tile as tile
from concourse import mybir
from concourse.masks import make_identity

P = 128
B, T, C = 4, 2048, 1024
H, HD = 16, 64
F = 4096
T2 = T  # tokens per core for K/V (full sequence of one batch element)
TQ = T // 2  # own query tokens per core
CC = C // P  # 8 C-chunks
NPR = CC // 2  # 4 DoubleRow contraction pair-chunks (256 each)
N = 512  # moving free dim per matmul

SW = 64.0  # fp8 weight prescale (qkv/v/proj/fc)
SWO = 256.0  # fp8 weight prescale for W_out
SY = 16.0  # attention-output prescale into fp8

F32 = mybir.dt.float32
F32R = mybir.dt.float32r
BF16 = mybir.dt.bfloat16
FP8 = mybir.dt.float8e4
AF = mybir.ActivationFunctionType
OP = mybir.AluOpType
DR = mybir.MatmulPerfMode.DoubleRow

PROFILE = False
SIM_GELU = False  # CoreSim lacks the Gelu LUT; emulate with Tanh + DVE ops
last_exec_time_ns = None

_CACHE = {}

# per-(g,kt) attention slot schedule, uniform across cores.
# kinds: 'diag' (mask m), 'full', 'gate2' (live iff s==1), 'gate3' (iff s==0)
SLOTS0 = [(8, "gate2", 0), (9, "gate2", 0), (10, "gate2", 0), (11, "gate2", 0),
          (0, "diag", 0), (1, "diag", 1), (2, "diag", 2), (3, "diag", 3)]
SLOTS1 = [(0, "full", 0), (1, "full", 0), (2, "full", 0), (3, "full", 0),
          (4, "diag", 0), (5, "diag", 1), (6, "diag", 2), (7, "diag", 3),
          (8, "full", 0), (9, "full", 0), (10, "full", 0), (11, "full", 0),
          (12, "gate3", 0), (13, "gate3", 0), (14, "gate3", 0), (15, "gate3", 0)]
SLOTS = (SLOTS0, SLOTS1)


def _build_nc(apply_lnwb: bool = True):
    nc = bacc.Bacc("TRN2", target_bir_lowering=False, debug=False, num_devices=8)

    x_seq = nc.dram_tensor("x_seq", [T2, C], F32, kind="ExternalInput")
    w_qk3 = nc.dram_tensor("w_qk3", [P, NPR, 2, 2 * C], FP8, kind="ExternalInput")
    w_v3 = nc.dram_tensor("w_v3", [P, NPR, 2, C], FP8, kind="ExternalInput")
    w_pj3 = nc.dram_tensor("w_pj3", [P, NPR, 2, C], FP8, kind="ExternalInput")
    w_fc3 = nc.dram_tensor("w_fc3", [P, NPR, 2, F], FP8, kind="ExternalInput")
    w_ot3 = nc.dram_tensor("w_ot3", [P, F // 256, 2, C], FP8, kind="ExternalInput")
    ln1w = nc.dram_tensor("ln1w", [P, C], F32, kind="ExternalInput")
    ln1b = nc.dram_tensor("ln1b", [P, C], F32, kind="ExternalInput")
    ln2w = nc.dram_tensor("ln2w", [P, C], F32, kind="ExternalInput")
    ln2b = nc.dram_tensor("ln2b", [P, C], F32, kind="ExternalInput")
    b_q = nc.dram_tensor("b_q", [P, CC], F32, kind="ExternalInput")  # x64
    b_k = nc.dram_tensor("b_k", [P, CC], F32, kind="ExternalInput")  # x64
    b_v = nc.dram_tensor("b_v", [P, CC], F32, kind="ExternalInput")  # x16, col per chunk
    b_pj = nc.dram_tensor("b_pj", [P, C], F32, kind="ExternalInput")
    b_fc = nc.dram_tensor("b_fc", [P, F // P], F32, kind="ExternalInput")
    b_ot = nc.dram_tensor("b_ot", [P, C], F32, kind="ExternalInput")
    mask4 = nc.dram_tensor("mask4", [P, 4, 2, N], BF16, kind="ExternalInput")
    gate2 = nc.dram_tensor("gate2", [P, 1], F32, kind="ExternalInput")
    gate3 = nc.dram_tensor("gate3", [P, 1], F32, kind="ExternalInput")
    sel2 = nc.dram_tensor("sel2", [2, P], F32R, kind="ExternalInput")

    out_d = nc.dram_tensor("out", [TQ, C], F32, kind="ExternalOutput")
    sums_d = nc.dram_tensor("sums_scratch", [16, TQ], F32)

    from contextlib import ExitStack

    with tile.TileContext(nc) as tc, ExitStack() as ctx:
        # pool enter order = reverse of close order (pool stack is LIFO);
        # SBUF is reserved from first tile creation to pool close
        const = ctx.enter_context(tc.tile_pool(name="const", bufs=1))
        p_big = ctx.enter_context(tc.tile_pool(name="p_big", bufs=1))
        es_mid = ctx.enter_context(ExitStack())
        es_x2 = ctx.enter_context(ExitStack())
        es_wpj = ctx.enter_context(ExitStack())
        es_y8 = ctx.enter_context(ExitStack())
        es_y = ctx.enter_context(ExitStack())
        es_qk = ctx.enter_context(ExitStack())
        es_v = ctx.enter_context(ExitStack())
        es_wv = ctx.enter_context(ExitStack())

        ident = const.tile([P, P], BF16, tag="ident", name="ident")
        make_identity(nc, ident)
        eps_sb = const.tile([P, 1], F32, tag="eps", name="eps")
        nc.vector.memset(eps_sb[:], 1e-5)
        mask_sb = const.tile([P, 4, 2, N], BF16, tag="mask", name="mask")
        nc.sync.dma_start(out=mask_sb[:], in_=mask4[:])
        gate2_sb = const.tile([P, 1], F32, tag="g2", name="g2")
        nc.sync.dma_start(out=gate2_sb[:], in_=gate2[:])
        gate3_sb = const.tile([P, 1], F32, tag="g3", name="g3")
        nc.sync.dma_start(out=gate3_sb[:], in_=gate3[:])
        sel_sb = const.tile([2, P], F32R, tag="sel", name="sel")
        nc.sync.dma_start(out=sel_sb[:], in_=sel2[:])
        bq_sb = const.tile([P, CC], F32, tag="bq", name="bq")
        nc.sync.dma_start(out=bq_sb[:], in_=b_q[:])
        bk_sb = const.tile([P, CC], F32, tag="bk", name="bk")
        nc.sync.dma_start(out=bk_sb[:], in_=b_k[:])
        bv_sb = const.tile([P, CC], F32, tag="bv", name="bv")
        nc.sync.dma_start(out=bv_sb[:], in_=b_v[:])
        bfc_sb = const.tile([P, F // P], F32, tag="bfc", name="bfc")
        nc.sync.dma_start(out=bfc_sb[:], in_=b_fc[:])
        bpj_sb = const.tile([P, C], F32, tag="bpj", name="bpj")
        nc.sync.dma_start(out=bpj_sb[:], in_=b_pj[:])
        bot_sb = const.tile([P, C], F32, tag="bot", name="bot")
        nc.sync.dma_start(out=bot_sb[:], in_=b_ot[:])

        p_mid = es_mid.enter_context(tc.tile_pool(name="p_mid", bufs=1))
        p_x2 = es_x2.enter_context(tc.tile_pool(name="p_x2", bufs=1))
        p_wpj = es_wpj.enter_context(tc.tile_pool(name="p_wpj", bufs=1))
        p_y8 = es_y8.enter_context(tc.tile_pool(name="p_y8", bufs=1))
        p_y = es_y.enter_context(tc.tile_pool(name="p_y", bufs=1))
        p_qk = es_qk.enter_context(tc.tile_pool(name="p_qk", bufs=1))
        p_v = es_v.enter_context(tc.tile_pool(name="p_v", bufs=1))
        p_wv = es_wv.enter_context(tc.tile_pool(name="p_wv", bufs=1))

        # one 32 KiB/partition fp8 buffer triple-aliased across disjoint
        # lifetimes: [xlnT | wqk] (phases 1-3) then h8 (phases 7-8)
        buf32 = p_big.tile([P, 2 * CC * T2], FP8, tag="buf32", name="buf32")
        xlnT = buf32.rearrange("p (a c t) -> p a c t", a=2, t=T2)[:, 0]
        wqk_sb = buf32.rearrange("p (a pr hf f) -> p a pr hf f",
                                 a=2, pr=NPR, hf=2)[:, 1]
        h8 = buf32.rearrange("p (f t) -> p f t", t=TQ)
        wv_sb = p_wv.tile([P, NPR, 2, C], FP8, tag="wv", name="wv")
        wpj_sb = p_wpj.tile([P, NPR, 2, C], FP8, tag="wpj", name="wpj")

        # ---------------- Phase 1: LN1 + transpose + K (fused) --------------
        # K matmuls for token-block pairs are emitted as soon as their
        # transposes land, filling the PE during the DVE/ACT-bound LN loop.
        # Weight-stationary over 2 blocks so each 256-col DoubleRow weight
        # load amortizes over 2 matmuls; evacuations ((psum+64b)/64 -> bf16)
        # run on ScalarE (idle here) via the free affine: ps/64 + b_true.
        nc.sync.dma_start(out=wqk_sb[:], in_=w_qk3[:])
        nc.sync.dma_start(out=wv_sb[:], in_=w_v3[:])
        nc.sync.dma_start(out=wpj_sb[:], in_=w_pj3[:])
        q_sb = [p_qk.tile([P, TQ], BF16, tag=f"q{i}", name=f"q{i}") for i in range(CC)]
        k_sb = [p_qk.tile([P, T2], BF16, tag=f"k{i}", name=f"k{i}") for i in range(CC)]

        def emit_k(blk, kps):
            for fc in range(CC):
                psK = kps.tile([P, N], F32, tag="k_ps", name="k_ps")
                for pr in range(NPR):
                    lk = wqk_sb[:, pr, :, C + fc * P:C + (fc + 1) * P]
                    nc.tensor.matmul(psK[:], lhsT=lk,
                                     rhs=xlnT[:, 2 * pr:2 * pr + 2, blk * N:(blk + 1) * N],
                                     start=(pr == 0), stop=(pr == NPR - 1),
                                     perf_mode=DR)
                nc.scalar.activation(k_sb[fc][:, blk * N:(blk + 1) * N],
                                     psK[:], AF.Identity,
                                     bias=bk_sb[:, fc:fc + 1], scale=1.0 / SW)

        with tc.tile_pool(name="qk_ps", bufs=4, space="PSUM") as kps:
            with tc.tile_pool(name="ln1_sp", bufs=3) as sp, \
                 tc.tile_pool(name="ln1_cp", bufs=1) as cp, \
                 tc.tile_pool(name="ln1_st", bufs=6) as st, \
                 tc.tile_pool(name="ln1_ps", bufs=3, space="PSUM") as tps:
                if apply_lnwb:
                    w1 = cp.tile([P, C], F32, tag="w1", name="w1")
                    nc.sync.dma_start(out=w1[:], in_=ln1w[:])
                    b1 = cp.tile([P, C], F32, tag="b1", name="b1")
                    nc.sync.dma_start(out=b1[:], in_=ln1b[:])
                for tt in range(T2 // P):
                    xt = sp.tile([P, C], F32, tag="xs", name="xs")
                    nc.sync.dma_start(out=xt[:], in_=x_seq[tt * P:(tt + 1) * P, :])
                    stats = st.tile([P, 2, 6], F32, tag="st", name="st")
                    for g in range(2):
                        nc.vector.bn_stats(out=stats[:, g, :], in_=xt[:, g * 512:(g + 1) * 512])
                    mv = st.tile([P, 2], F32, tag="mv", name="mv")
                    nc.vector.bn_aggr(out=mv[:], in_=stats[:])
                    rstd = st.tile([P, 1], F32, tag="rstd", name="rstd")
                    nc.scalar.activation(rstd[:], mv[:, 1:2], AF.Sqrt, bias=eps_sb[:], scale=1.0)
                    nc.vector.reciprocal(out=rstd[:], in_=rstd[:])
                    xb = sp.tile([P, C], BF16, tag="xb", name="xb")
                    nmr = st.tile([P, 1], F32, tag="nmr", name="nmr")
                    nc.vector.tensor_scalar(nmr[:], mv[:, 0:1], rstd[:], -1.0,
                                            OP.mult, OP.mult)
                    if apply_lnwb:
                        xc = sp.tile([P, C], F32, tag="xc", name="xc")
                        nc.scalar.activation(xc[:], xt[:], AF.Identity,
                                             bias=nmr[:], scale=rstd[:])
                        xw = sp.tile([P, C], F32, tag="xw", name="xw")
                        nc.vector.tensor_tensor(out=xw[:], in0=xc[:], in1=w1[:], op=OP.mult)
                        nc.vector.tensor_tensor(out=xb[:], in0=xw[:], in1=b1[:], op=OP.add)
                    else:
                        nc.scalar.activation(xb[:], xt[:], AF.Identity,
                                             bias=nmr[:], scale=rstd[:])
                    pst = tps.tile([P, CC, P], BF16, tag="trp", name="trp")
                    for cc in range(CC):
                        nc.tensor.transpose(pst[:, cc, :],
                                            xb[:, cc * P:(cc + 1) * P], ident[:])
                    nc.vector.tensor_copy(
                        out=xlnT[:, :, tt * P:(tt + 1) * P], in_=pst[:])
                    if tt % 4 == 3 and tt < 15:
                        emit_k(tt // 4, kps)
            emit_k(3, kps)

        # ---------------- Phase 2: Q projection (DoubleRow fp8) -------------
        with tc.tile_pool(name="q_ps", bufs=4, space="PSUM") as qps:
            for fc in range(CC):
                psQ = [qps.tile([P, N], F32, tag="q_ps", name="q_ps") for _ in range(2)]
                for pr in range(NPR):
                    lq = wqk_sb[:, pr, :, fc * P:(fc + 1) * P]
                    for blk in range(2):
                        nc.tensor.matmul(psQ[blk][:], lhsT=lq,
                                         rhs=xlnT[:, 2 * pr:2 * pr + 2, blk * N:(blk + 1) * N],
                                         start=(pr == 0), stop=(pr == NPR - 1),
                                         perf_mode=DR)
                for blk in range(2):
                    nc.scalar.activation(q_sb[fc][:, blk * N:(blk + 1) * N],
                                         psQ[blk][:], AF.Identity,
                                         bias=bq_sb[:, fc:fc + 1], scale=1.0 / SW)

        # ---------------- Phase 2b: V projection (DoubleRow fp8) ------------
        v_sb = [p_v.tile([P, H * 65], FP8, tag=f"v{i}", name=f"v{i}")
                for i in range(T2 // P)]
        for kt in range(T2 // P):
            nc.gpsimd.memset(
                v_sb[kt].rearrange("p (h d) -> p h d", d=65)[:, :, 64:65], 1.0)
        def emit_v(kt, vps):
            ps = [vps.tile([P, N], F32, tag="v_ps", name="v_ps") for _ in range(2)]
            for pr in range(NPR):
                lv = xlnT[:, 2 * pr:2 * pr + 2, kt * P:(kt + 1) * P]
                for vg in range(2):
                    nc.tensor.matmul(ps[vg][:], lhsT=lv,
                                     rhs=wv_sb[:, pr, :, vg * N:(vg + 1) * N],
                                     start=(pr == 0), stop=(pr == NPR - 1),
                                     perf_mode=DR)
            for vg in range(2):
                out_ap = v_sb[kt].rearrange("p (h d) -> p h d", d=65)[
                    :, vg * 8:(vg + 1) * 8, 0:64]
                in_ap = ps[vg].rearrange("p (h d) -> p h d", d=64)[:, :, :]
                nc.scalar.activation(out_ap, in_ap, AF.Identity,
                                     bias=0.0, scale=1.0 / SW)

        with tc.tile_pool(name="v_ps", bufs=4, space="PSUM") as vps:
            for kt in (8, 9, 10, 11, 0, 1, 2, 3):
                emit_v(kt, vps)
        v_queue = [4, 5, 6, 7, 12, 13, 14, 15]

        # ---------------- Phase 3: attention -------------------------------
        # Per head-pair: 24 causal slots/qb-group; score matmul pair runs
        # concurrently in PE row-groups 0-63/64-127. Each qb group leads with
        # its maskless slots so the previous group's DVE evacuation tail
        # drains before the first causal-mask multiply is needed.
        y_fm = [p_y.tile([P, TQ], BF16, tag=f"y{i}", name=f"y{i}") for i in range(CC)]
        with tc.tile_pool(name="att_at", bufs=1) as ap_pool, \
             tc.tile_pool(name="att_sps", bufs=2, space="PSUM") as sps_pool, \
             tc.tile_pool(name="att_yps", bufs=2, space="PSUM") as yps_pool, \
             tc.tile_pool(name="att_vps", bufs=2, space="PSUM") as avps:
            for hp in range(H // 2):
                for qb in (0, 1):
                    slots = SLOTS[qb]
                    yps = [yps_pool.tile([65, N], F32, tag="yps", name="yps")
                           for _ in range(2)]
                    last = len(slots) - 1
                    for i, (kt, kind, m) in enumerate(slots):
                        sp = sps_pool.tile([P, 2, N], F32, tag="sps", name="sps")
                        for j in range(2):
                            ro = j * 64
                            nc.tensor.matmul(
                                sp[:, j, :],
                                lhsT=k_sb[hp][ro:ro + 64, kt * P:(kt + 1) * P],
                                rhs=q_sb[hp][ro:ro + 64, qb * N:(qb + 1) * N],
                                start=True, stop=True)
                        if v_queue and hp == 0 and i % 2 == 1:
                            emit_v(v_queue.pop(0), avps)
                        at = ap_pool.tile([P, 2, N], FP8, tag="at", name="at", bufs=8)
                        bias = {"diag": 0.0, "full": 0.0,
                                "gate2": gate2_sb[:, 0:1],
                                "gate3": gate3_sb[:, 0:1]}[kind]
                        nc.scalar.activation(at[:, 0:2, :], sp[:, 0:2, :],
                                             AF.Exp, bias=bias, scale=0.125)
                        if kind == "diag":
                            nc.vector.tensor_tensor(
                                out=at[:, 0:2, :], in0=at[:, 0:2, :],
                                in1=mask_sb[:, m, :, :], op=OP.mult)
                        for j in range(2):
                            h = 2 * hp + j
                            nc.tensor.matmul(yps[j][:],
                                             lhsT=v_sb[kt][:, h * 65:(h + 1) * 65],
                                             rhs=at[:, j, :],
                                             start=(i == 0), stop=(i == last))
                    for j in range(2):
                        nc.vector.tensor_copy(
                            out=y_fm[hp][j * 64:(j + 1) * 64, qb * N:(qb + 1) * N],
                            in_=yps[j][0:64, :])
                        s1 = ap_pool.tile([1, N], F32, tag="s1", name="s1", bufs=4)
                        nc.vector.tensor_copy(out=s1[:], in_=yps[j][64:65, :])
                        nc.sync.dma_start(
                            out=sums_d[2 * hp + j:2 * hp + j + 1, qb * N:(qb + 1) * N],
                            in_=s1[:])

        # ---------------- Phase 4: normalize y -> y8 (x SY, + SY*b_v) ------
        # one lane-parallel reciprocal over all 16 heads' sums; partition
        # repack via DRAM roundtrip (engine partition bases are 0/32/64 only)
        y8 = p_y8.tile([P, CC, TQ], FP8, tag="y8", name="y8")
        with tc.tile_pool(name="att_rp", bufs=2) as rp, \
             tc.tile_pool(name="att_rps", bufs=2, space="PSUM") as rps_pool:
            s16 = rp.tile([16, TQ], F32, tag="s16", name="s16", bufs=1)
            nc.sync.dma_start(out=s16[:], in_=sums_d[:])
            recip16 = rp.tile([16, TQ], F32, tag="recip16", name="recip16", bufs=1)
            nc.vector.reciprocal(out=recip16[:], in_=s16[:])
            reciprr = rp.tile([16, TQ], F32R, tag="reciprr", name="reciprr", bufs=1)
            with nc.allow_low_precision(reason="f32r view of f32 recip"):
                nc.vector.tensor_scalar(reciprr[:], recip16[:], SY, None, OP.mult)
            for yt in range(CC):
                recip_r = rp.tile([2, TQ], F32R, tag="recipr", name="recipr", bufs=4)
                nc.sync.dma_start(out=recip_r[:], in_=reciprr[2 * yt:2 * yt + 2, :])
                for tt in range(2):
                    rps = rps_pool.tile([P, N], F32, tag="rps", name="rps")
                    nc.tensor.matmul(rps[:], lhsT=sel_sb[:],
                                     rhs=recip_r[:, tt * N:(tt + 1) * N],
                                     start=True, stop=True)
                    nc.vector.tensor_tensor(out=y8[:, yt, tt * N:(tt + 1) * N],
                                            in0=y_fm[yt][:, tt * N:(tt + 1) * N],
                                            in1=rps[:], op=OP.mult)
                    nc.vector.tensor_scalar(y8[:, yt, tt * N:(tt + 1) * N],
                                            y8[:, yt, tt * N:(tt + 1) * N],
                                            bv_sb[:, yt:yt + 1], None, OP.add)

        es_wv.close()
        es_v.close()
        es_qk.close()
        es_y.close()

        # ---------------- Phase 5: proj + residual + LN2 (fused) ------------
        # proj evacuation, residual add, LN2 and its transposes all run
        # per-128-token tile so PE pipelines the next tile's proj matmuls
        # under this tile's DVE/ACT work.
        x_mid = [p_mid.tile([P, C], BF16, tag=f"xm{i}", name=f"xm{i}")
                 for i in range(TQ // P)]
        xln2T = p_x2.tile([P, CC, TQ], FP8, tag="xln2T", name="xln2T")
        with tc.tile_pool(name="pj_sp", bufs=3) as sp, \
             tc.tile_pool(name="pj_cp", bufs=1) as cp, \
             tc.tile_pool(name="pj_st", bufs=6) as st, \
             tc.tile_pool(name="pj_ps", bufs=2, space="PSUM") as pps, \
             tc.tile_pool(name="ln2_ps", bufs=3, space="PSUM") as tps:
            if apply_lnwb:
                w2 = cp.tile([P, C], F32, tag="w2", name="w2")
                nc.sync.dma_start(out=w2[:], in_=ln2w[:])
                b2 = cp.tile([P, C], F32, tag="b2", name="b2")
                nc.sync.dma_start(out=b2[:], in_=ln2b[:])
            for t8 in range(TQ // P):
                xo = sp.tile([P, C], F32, tag="xo", name="xo")
                nc.sync.dma_start(out=xo[:], in_=x_seq[t8 * P:(t8 + 1) * P, :])
                ps2 = pps.tile([P, 2, N], F32, tag="pj_ps", name="pj_ps")
                for pr in range(NPR):
                    ly = y8[:, 2 * pr:2 * pr + 2, t8 * P:(t8 + 1) * P]
                    for ft in range(2):
                        nc.tensor.matmul(ps2[:, ft, :], lhsT=ly,
                                         rhs=wpj_sb[:, pr, :, ft * N:(ft + 1) * N],
                                         start=(pr == 0), stop=(pr == NPR - 1),
                                         perf_mode=DR)
                xt = x_mid[t8]
                nc.scalar.activation(xt[:], ps2[:, 0:2, :], AF.Identity,
                                     bias=0.0, scale=1.0 / (SW * SY))
                nc.vector.tensor_tensor(out=xt[:], in0=xt[:], in1=xo[:], op=OP.add)
                nc.gpsimd.tensor_tensor(out=xt[:], in0=xt[:], in1=bpj_sb[:], op=OP.add)
                # --- LN2 for this token tile ---
                stats = st.tile([P, 2, 6], F32, tag="st2", name="st2")
                for g in range(2):
                    nc.vector.bn_stats(out=stats[:, g, :], in_=xt[:, g * 512:(g + 1) * 512])
                mv = st.tile([P, 2], F32, tag="mv2", name="mv2")
                nc.vector.bn_aggr(out=mv[:], in_=stats[:])
                rstd = st.tile([P, 1], F32, tag="rstd2", name="rstd2")
                nc.scalar.activation(rstd[:], mv[:, 1:2], AF.Sqrt, bias=eps_sb[:], scale=1.0)
                nc.vector.reciprocal(out=rstd[:], in_=rstd[:])
                xb = sp.tile([P, C], BF16, tag="xb2", name="xb2")
                nmr = st.tile([P, 1], F32, tag="nmr2", name="nmr2")
                nc.vector.tensor_scalar(nmr[:], mv[:, 0:1], rstd[:], -1.0,
                                        OP.mult, OP.mult)
                if apply_lnwb:
                    xc = sp.tile([P, C], F32, tag="xc2", name="xc2")
                    nc.scalar.activation(xc[:], xt[:], AF.Identity,
                                         bias=nmr[:], scale=rstd[:])
                    xw = sp.tile([P, C], F32, tag="xw2", name="xw2")
                    nc.vector.tensor_tensor(out=xw[:], in0=xc[:], in1=w2[:], op=OP.mult)
                    nc.vector.tensor_tensor(out=xb[:], in0=xw[:], in1=b2[:], op=OP.add)
                else:
                    nc.scalar.activation(xb[:], xt[:], AF.Identity,
                                         bias=nmr[:], scale=rstd[:])
                pst = tps.tile([P, CC, P], BF16, tag="trp2", name="trp2")
                for cc in range(CC):
                    nc.tensor.transpose(pst[:, cc, :],
                                        xb[:, cc * P:(cc + 1) * P], ident[:])
                nc.vector.tensor_copy(
                    out=xln2T[:, :, t8 * P:(t8 + 1) * P], in_=pst[:])

        es_y8.close()
        es_wpj.close()

        # ---------------- Phase 7: FC + gelu (DoubleRow fp8) ---------------
        with tc.tile_pool(name="fc_w", bufs=3) as wp, \
             tc.tile_pool(name="fc_ps", bufs=3, space="PSUM") as fps:
            for hg in range(F // N):
                wt = wp.tile([P, NPR, 2, N], FP8, tag="wfc", name="wfc")
                nc.sync.dma_start(out=wt[:], in_=w_fc3[:, :, :, hg * N:(hg + 1) * N])
                for hs in range(4):
                    hf = hg * 4 + hs
                    ps2 = fps.tile([P, 2, N], F32, tag="fc_ps", name="fc_ps")
                    for pr in range(NPR):
                        lw = wt[:, pr, :, hs * P:(hs + 1) * P]
                        for tt in range(2):
                            nc.tensor.matmul(ps2[:, tt, :], lhsT=lw,
                                             rhs=xln2T[:, 2 * pr:2 * pr + 2, tt * N:(tt + 1) * N],
                                             start=(pr == 0), stop=(pr == NPR - 1),
                                             perf_mode=DR)
                    if not SIM_GELU:
                        nc.scalar.activation(h8[:, hf, :], ps2[:, 0:2, :],
                                             AF.Gelu_apprx_tanh,
                                             bias=bfc_sb[:, hf:hf + 1], scale=1.0 / SW)
                    else:
                        import math
                        cst = math.sqrt(2.0 / math.pi)
                        u = wp.tile([P, 2, N], F32, tag="g_u", name="g_u")
                        nc.scalar.activation(u[:], ps2[:, 0:2, :], AF.Identity,
                                             bias=bfc_sb[:, hf:hf + 1], scale=1.0 / SW)
                        u3 = wp.tile([P, 2, N], F32, tag="g_u3", name="g_u3")
                        nc.scalar.activation(u3[:], u[:], AF.Square, bias=0.0, scale=1.0)
                        nc.vector.tensor_tensor(out=u3[:], in0=u3[:], in1=u[:], op=OP.mult)
                        nc.vector.tensor_scalar(u3[:], u3[:], 0.044715, None, OP.mult)
                        nc.vector.tensor_tensor(out=u3[:], in0=u3[:], in1=u[:], op=OP.add)
                        tqh = wp.tile([P, 2, N], F32, tag="g_t", name="g_t")
                        nc.scalar.activation(tqh[:], u3[:], AF.Tanh, bias=0.0, scale=cst)
                        nc.vector.tensor_scalar(tqh[:], tqh[:], 1.0, None, OP.add)
                        nc.vector.tensor_tensor(out=tqh[:], in0=tqh[:], in1=u[:], op=OP.mult)
                        nc.vector.tensor_scalar(h8[:, hf, :], tqh[:], 0.5, None, OP.mult)

        es_x2.close()

        # ---------------- Phase 8: out matmul + residual (DoubleRow fp8) ---
        with tc.tile_pool(name="ot_w", bufs=6) as wp, \
             tc.tile_pool(name="ot_sp", bufs=3) as sp, \
             tc.tile_pool(name="ot_ps", bufs=8, space="PSUM") as ops_pool:
            for half in range(2):
                opss = [ops_pool.tile([P, N], F32, tag="ot_ps", name="ot_ps")
                        for _ in range(8)]
                for pr in range(F // 256):
                    wt = wp.tile([P, 2, C], FP8, tag="wot", name="wot")
                    nc.sync.dma_start(out=wt[:], in_=w_ot3[:, pr, :, :])
                    for tc4 in range(4):
                        t8 = half * 4 + tc4
                        lh = h8[:, 2 * pr:2 * pr + 2, t8 * P:(t8 + 1) * P]
                        for ft in range(2):
                            nc.tensor.matmul(opss[tc4 * 2 + ft][:], lhsT=lh,
                                             rhs=wt[:, :, ft * N:(ft + 1) * N],
                                             start=(pr == 0), stop=(pr == F // 256 - 1),
                                             perf_mode=DR)
                for tc4 in range(4):
                    t8 = half * 4 + tc4
                    ot = sp.tile([P, C], F32, tag="ot", name="ot")
                    for ft in range(2):
                        nc.vector.tensor_scalar(ot[:, ft * N:(ft + 1) * N],
                                                opss[tc4 * 2 + ft][:],
                                                1.0 / SWO, None, OP.mult)
                    nc.vector.tensor_tensor(out=ot[:], in0=ot[:],
                                            in1=x_mid[t8][:], op=OP.add)
                    nc.vector.tensor_tensor(out=ot[:], in0=ot[:], in1=bot_sb[:], op=OP.add)
                    nc.sync.dma_start(out=out_d[t8 * P:(t8 + 1) * P, :], in_=ot[:])

    nc.finalize()
    return nc


def _own_blocks(s):
    return [0, 1, 2, 3, 12, 13, 14, 15] if s == 0 else list(range(4, 12))


def _prep_shared(inputs):
    f8 = ml_dtypes.float8_e4m3

    def pack_dr(wT, npr, scale):
        # wT: [K, M] (contraction-major); -> [P, npr, 2, M] with
        # [p, pr, hf, m] = scale * wT[pr*256 + hf*128 + p, m]
        K, M = wT.shape
        assert K == npr * 256
        a = (wT * scale).reshape(npr, 2, P, M).transpose(2, 0, 1, 3)
        return np.ascontiguousarray(a).astype(f8)

    W_attn = np.asarray(inputs["W_attn"], np.float32)
    shared = {
        "w_qk3": pack_dr(np.ascontiguousarray(W_attn[:2 * C].T), NPR, SW),
        "w_v3": pack_dr(np.ascontiguousarray(W_attn[2 * C:].T), NPR, SW),
        "w_pj3": pack_dr(np.ascontiguousarray(np.asarray(inputs["W_proj"], np.float32).T), NPR, SW),
        "w_fc3": pack_dr(np.ascontiguousarray(np.asarray(inputs["W_fc"], np.float32).T), NPR, SW),
        "w_ot3": pack_dr(np.ascontiguousarray(np.asarray(inputs["W_out"], np.float32).T), F // 256, SWO),
        "ln1w": np.ascontiguousarray(np.broadcast_to(np.asarray(inputs["ln1_w"], np.float32), (P, C))),
        "ln1b": np.ascontiguousarray(np.broadcast_to(np.asarray(inputs["ln1_b"], np.float32), (P, C))),
        "ln2w": np.ascontiguousarray(np.broadcast_to(np.asarray(inputs["ln2_w"], np.float32), (P, C))),
        "ln2b": np.ascontiguousarray(np.broadcast_to(np.asarray(inputs["ln2_b"], np.float32), (P, C))),
        "b_q": np.ascontiguousarray(np.asarray(inputs["b_attn"], np.float32)[:C].reshape(CC, P).T),
        "b_k": np.ascontiguousarray(np.asarray(inputs["b_attn"], np.float32)[C:2 * C].reshape(CC, P).T),
        "b_v": np.ascontiguousarray(np.asarray(inputs["b_attn"], np.float32)[2 * C:].reshape(CC, P).T) * SY,
        "b_pj": np.ascontiguousarray(np.broadcast_to(np.asarray(inputs["b_proj"], np.float32), (P, C))),
        "b_fc": np.ascontiguousarray(np.asarray(inputs["b_fc"], np.float32).reshape(F // P, P).T),
        "b_ot": np.ascontiguousarray(np.broadcast_to(np.asarray(inputs["b_out"], np.float32), (P, C))),
    }
    # mask4[p, m*N + qf] = 1 if qf >= m*128 + p else 0
    pp = np.arange(P)[:, None]
    qf = np.arange(N)[None, :]
    mask = np.zeros((P, 4, 2, N), np.float32)
    for m in range(4):
        mask[:, m, 0, :] = (qf >= m * P + pp)
        mask[:, m, 1, :] = mask[:, m, 0, :]
    shared["mask4"] = mask.astype(ml_dtypes.bfloat16)
    sel = np.zeros((2, P), np.float32)
    sel[0, :64] = 1.0
    sel[1, 64:] = 1.0
    shared["sel2"] = sel
    return shared


def _make_in_maps(inputs):
    x = np.asarray(inputs["x"], np.float32)
    shared = _prep_shared(inputs)
    in_maps = []
    for c in range(8):
        b, s = c // 2, c % 2
        own = _own_blocks(s)
        other = _own_blocks(1 - s)
        xb = x[b].reshape(16, P, C)
        m = dict(shared)
        m["x_seq"] = np.ascontiguousarray(
            np.concatenate([xb[own], xb[other]], axis=0).reshape(T2, C))
        m["gate2"] = np.full((P, 1), 0.0 if s == 1 else -1e30, np.float32)
        m["gate3"] = np.full((P, 1), 0.0 if s == 0 else -1e30, np.float32)
        in_maps.append(m)
    return in_maps


def _get_nc(apply_lnwb=True):
    key = ("nc", apply_lnwb, SIM_GELU)
    if key not in _CACHE:
        _CACHE[key] = _build_nc(apply_lnwb)
    return _CACHE[key]


def run_cores(inputs, profile=False):
    """Run the SPMD program; returns list of per-core result dicts."""
    global last_exec_time_ns
    apply_lnwb = not (
        np.allclose(np.asarray(inputs["ln1_w"]), 1.0)
        and np.allclose(np.asarray(inputs["ln1_b"]), 0.0)
        and np.allclose(np.asarray(inputs["ln2_w"]), 1.0)
        and np.allclose(np.asarray(inputs["ln2_b"]), 0.0))
    nc = _get_nc(apply_lnwb)
    in_maps = _make_in_maps(inputs)
    if profile:
        import concourse.bass_utils as bass_utils
        bass_utils.upload_artifacts = lambda tmpdir: "local://" + tmpdir
        try:
            from trn_agent_boot.trn_boot import _ntff_profile_via_ctypes
            import antenv.axon_hooks as hooks
            if hooks.get_axon_ntff_profile_hook() is None:
                hooks.set_axon_ntff_profile_hook(
                    _ntff_profile_via_ctypes("/opt/axon/libaxon_pjrt.so"))
        except Exception:
            pass
        res = bass_utils.run_bass_kernel_spmd(nc, in_maps, list(range(8)), trace=True)
        last_exec_time_ns = res.exec_time_ns
        return res.results
    return _cached_runner(nc)(in_maps)


def _cached_runner(nc):
    """Per-process cached jit of the SPMD executable so repeated kernel()
    calls don't recompile (mirrors bass2jax.run_bass_via_pjrt's multi-core
    branch)."""
    key = ("runner", id(nc))
    if key in _CACHE:
        return _CACHE[key]
    import jax
    import numpy as _np
    from jax.sharding import Mesh, PartitionSpec
    from jax.experimental.shard_map import shard_map
    from concourse import bass2jax, mybir as _mybir
    bass2jax.install_neuronx_cc_hook()

    part_name = nc.partition_id_tensor.name if nc.partition_id_tensor else None
    in_names, out_names, out_avals, zero_outs = [], [], [], []
    for alloc in nc.m.functions[0].allocations:
        if not isinstance(alloc, _mybir.MemoryLocationSet):
            continue
        name = alloc.memorylocations[0].name
        if alloc.kind == "ExternalInput":
            if name != part_name:
                in_names.append(name)
        elif alloc.kind == "ExternalOutput":
            out_names.append(name)
            shape = tuple(alloc.tensor_shape)
            dtype = _mybir.dt.np(alloc.dtype)
            out_avals.append(jax.core.ShapedArray(shape, dtype))
            zero_outs.append(_np.zeros(shape, dtype))
    n_params = len(in_names)
    all_names = in_names + out_names
    if part_name is not None:
        all_names = all_names + [part_name]
    donate = tuple(range(n_params, n_params + len(out_names)))
    if jax.default_backend() == "cpu":
        donate = ()  # cpu sim path can't alias donated outputs

    def _body(*args):
        operands = list(args)
        if part_name is not None:
            operands.append(bass2jax.partition_id_tensor())
        outs = bass2jax._bass_exec_p.bind(
            *operands, out_avals=tuple(out_avals), in_names=tuple(all_names),
            out_names=tuple(out_names), lowering_input_output_aliases=(),
            sim_require_finite=True, sim_require_nnan=True, nc=nc)
        return tuple(outs)

    devices = jax.devices()[:8]
    mesh = Mesh(_np.asarray(devices), ("core",))
    spec = (PartitionSpec("core"),) * (n_params + len(out_names))
    sharded = jax.jit(
        shard_map(_body, mesh=mesh, in_specs=spec,
                  out_specs=(PartitionSpec("core"),) * len(out_names),
                  check_rep=False),
        donate_argnums=donate, keep_unused=True)

    def run(in_maps):
        concat_in = [
            _np.concatenate([_np.asarray(in_maps[c][nm]) for c in range(8)], axis=0)
            for nm in in_names]
        concat_zero = [_np.zeros((8 * z.shape[0], *z.shape[1:]), z.dtype)
                       for z in zero_outs]
        out_arrs = sharded(*concat_in, *concat_zero)
        return [
            {nm: _np.asarray(out_arrs[i]).reshape(8, *out_avals[i].shape)[c]
             for i, nm in enumerate(out_names)}
            for c in range(8)]

    _CACHE[key] = run
    return run


def kernel(**inputs) -> np.ndarray:
    results = run_cores(inputs, profile=PROFILE)
    out = np.empty((B, T, C), np.float32)
    for c in range(8):
        b, s = c // 2, c % 2
        res = results[c]["out"]
        for j, blk in enumerate(_own_blocks(s)):
            out[b, blk * P:(blk + 1) * P, :] = res[j * P:(j + 1) * P]
    return out


# revision 51
# speedup vs baseline: 1.2534x; 1.0131x over previous
"""Trainium2 Bass kernel for a GPT-2 style transformer block.

Problem: B=4, T=2048, C=1024, H=16 heads (hd=64), MLP hidden 4096, fp32 I/O.

Sharding: zero-collective 8-way data parallel. Core c handles batch b=c//2;
s=c%2 selects its query set: s=0 owns the OUTER sequence quarters (blocks
0-3 and 12-15 of 128 tokens), s=1 the MIDDLE half (blocks 4-11). This makes
the causal-attention work symmetric across the pair: a uniform 24-tile
slot schedule per head covers both cores' needs, with per-core host-side
mask / gate tables providing the divergence. K/V are computed locally for
all 2048 tokens in own-first order.

Precision: all big weight matmuls (QKV, V, proj, FC, out) run in fp8e4
DoubleRow perf mode (2 contraction rows per PE cell per cycle): weights are
host-prescaled by 64 (W_out by 256) so N(0, 0.02)-scale values land in
e4m3's normal range; the scale is removed in the PSUM evacuation ops.
Attention q/k/v/exp-weights are fp8e4 as well (no DoubleRow; contraction is
only 64/128 deep), scores accumulate in f32 PSUM and softmax runs in f32 on
ScalarE. LayerNorm is f32 (bn_stats), residuals bf16/f32.

Layouts:
  x / residuals / final out: token-major [tok(P), C]
  x_ln transposed to feature-major [feat(P), chunk, tok] fp8 via PE
  transposes (bf16) + DVE convert-copy
  Q [feat(P), TQ], K [feat(P), T2] fp8; head pair hp lives in one tile
  (rows 0-63 head 2hp, 64-127 head 2hp+1) so score matmuls of a pair are
  emitted adjacently and run CONCURRENTLY in distinct PE row-groups
  V token-major [tok(P), h*65] fp8 with a built-in ones column per head
  (softmax row sums ride the AV matmul); softmax needs no max-subtraction
  (scores bounded ~|s|<4) and no transposes anywhere in attention
  normalization + v-bias deferred to after AV via a tiny K=2 f32r selector
  matmul that partition-broadcasts 16/sums
"""

import os
import sys
import types

import numpy as np
import ml_dtypes

for _p in ("/opt/trn_rl_repo", "/root/.axon_site/_ro/trn_rl_repo"):
    if os.path.isdir(_p) and _p not in sys.path:
        sys.path.append(_p)

# antenv.axon_hooks is absent in this image; bass_utils imports it when
# tracing under axon. Provide the trivial get/set holder it expects.
if "antenv.axon_hooks" not in sys.modules:
    try:
        import antenv

        _m = types.ModuleType("antenv.axon_hooks")
        _m._hook = None

        def _set_hook(h):
            _m._hook = h

        def _get_hook():
            return _m._hook

        _m.set_axon_ntff_profile_hook = _set_hook
        _m.get_axon_ntff_profile_hook = _get_hook
        sys.modules["antenv.axon_hooks"] = _m
        antenv.axon_hooks = _m
    except ImportError:
        pass

import concourse.bacc as bacc
import concourse.tile as tile
from concourse import mybir
from concourse.masks import make_identity

P = 128
B, T, C = 4, 2048, 1024
H, HD = 16, 64
F = 4096
T2 = T  # tokens per core for K/V (full sequence of one batch element)
TQ = T // 2  # own query tokens per core
CC = C // P  # 8 C-chunks
NPR = CC // 2  # 4 DoubleRow contraction pair-chunks (256 each)
N = 512  # moving free dim per matmul

SW = 64.0  # fp8 weight prescale (qkv/v/proj/fc)
SWO = 256.0  # fp8 weight prescale for W_out
SY = 16.0  # attention-output prescale into fp8

F32 = mybir.dt.float32
F32R = mybir.dt.float32r
BF16 = mybir.dt.bfloat16
FP8 = mybir.dt.float8e4
AF = mybir.ActivationFunctionType
OP = mybir.AluOpType
DR = mybir.MatmulPerfMode.DoubleRow

PROFILE = False
SIM_GELU = False  # CoreSim lacks the Gelu LUT; emulate with Tanh + DVE ops
last_exec_time_ns = None

_CACHE = {}

# per-(g,kt) attention slot schedule, uniform across cores.
# kinds: 'diag' (mask m), 'full', 'gate2' (live iff s==1), 'gate3' (iff s==0)
SLOTS0 = [(8, "gate2", 0), (9, "gate2", 0), (10, "gate2", 0), (11, "gate2", 0),
          (0, "diag", 0), (1, "diag", 1), (2, "diag", 2), (3, "diag", 3)]
SLOTS1 = [(0, "full", 0), (1, "full", 0), (2, "full", 0), (3, "full", 0),
          (4, "diag", 0), (5, "diag", 1), (6, "diag", 2), (7, "diag", 3),
          (8, "full", 0), (9, "full", 0), (10, "full", 0), (11, "full", 0),
          (12, "gate3", 0), (13, "gate3", 0), (14, "gate3", 0), (15, "gate3", 0)]
SLOTS = (SLOTS0, SLOTS1)


def _build_nc(apply_lnwb: bool = True):
    nc = bacc.Bacc("TRN2", target_bir_lowering=False, debug=False, num_devices=8)

    x_seq = nc.dram_tensor("x_seq", [T2, C], F32, kind="ExternalInput")
    w_qk3 = nc.dram_tensor("w_qk3", [P, NPR, 2, 2 * C], FP8, kind="ExternalInput")
    w_v3 = nc.dram_tensor("w_v3", [P, NPR, 2, C], FP8, kind="ExternalInput")
    w_pj3 = nc.dram_tensor("w_pj3", [P, NPR, 2, C], FP8, kind="ExternalInput")
    w_fc3 = nc.dram_tensor("w_fc3", [P, NPR, 2, F], FP8, kind="ExternalInput")
    w_ot3 = nc.dram_tensor("w_ot3", [P, F // 256, 2, C], FP8, kind="ExternalInput")
    ln1w = nc.dram_tensor("ln1w", [P, C], F32, kind="ExternalInput")
    ln1b = nc.dram_tensor("ln1b", [P, C], F32, kind="ExternalInput")
    ln2w = nc.dram_tensor("ln2w", [P, C], F32, kind="ExternalInput")
    ln2b = nc.dram_tensor("ln2b", [P, C], F32, kind="ExternalInput")
    b_q = nc.dram_tensor("b_q", [P, CC], F32, kind="ExternalInput")  # x64
    b_k = nc.dram_tensor("b_k", [P, CC], F32, kind="ExternalInput")  # x64
    b_v = nc.dram_tensor("b_v", [P, CC], F32, kind="ExternalInput")  # x16, col per chunk
    b_pj = nc.dram_tensor("b_pj", [P, C], F32, kind="ExternalInput")
    b_fc = nc.dram_tensor("b_fc", [P, F // P], F32, kind="ExternalInput")
    b_ot = nc.dram_tensor("b_ot", [P, C], F32, kind="ExternalInput")
    mask4 = nc.dram_tensor("mask4", [P, 4, 2, N], BF16, kind="ExternalInput")
    gate2 = nc.dram_tensor("gate2", [P, 1], F32, kind="ExternalInput")
    gate3 = nc.dram_tensor("gate3", [P, 1], F32, kind="ExternalInput")
    sel2 = nc.dram_tensor("sel2", [2, P], F32R, kind="ExternalInput")

    out_d = nc.dram_tensor("out", [TQ, C], F32, kind="ExternalOutput")
    sums_d = nc.dram_tensor("sums_scratch", [16, TQ], F32)

    from contextlib import ExitStack

    with tile.TileContext(nc) as tc, ExitStack() as ctx:
        # pool enter order = reverse of close order (pool stack is LIFO);
        # SBUF is reserved from first tile creation to pool close
        const = ctx.enter_context(tc.tile_pool(name="const", bufs=1))
        p_big = ctx.enter_context(tc.tile_pool(name="p_big", bufs=1))
        es_mid = ctx.enter_context(ExitStack())
        es_x2 = ctx.enter_context(ExitStack())
        es_wpj = ctx.enter_context(ExitStack())
        es_y8 = ctx.enter_context(ExitStack())
        es_y = ctx.enter_context(ExitStack())
        es_qk = ctx.enter_context(ExitStack())
        es_v = ctx.enter_context(ExitStack())
        es_wv = ctx.enter_context(ExitStack())

        ident = const.tile([P, P], BF16, tag="ident", name="ident")
        make_identity(nc, ident)
        eps_sb = const.tile([P, 1], F32, tag="eps", name="eps")
        nc.vector.memset(eps_sb[:], 1e-5)
        mask_sb = const.tile([P, 4, 2, N], BF16, tag="mask", name="mask")
        nc.sync.dma_start(out=mask_sb[:], in_=mask4[:])
        gate2_sb = const.tile([P, 1], F32, tag="g2", name="g2")
        nc.sync.dma_start(out=gate2_sb[:], in_=gate2[:])
        gate3_sb = const.tile([P, 1], F32, tag="g3", name="g3")
        nc.sync.dma_start(out=gate3_sb[:], in_=gate3[:])
        sel_sb = const.tile([2, P], F32R, tag="sel", name="sel")
        nc.sync.dma_start(out=sel_sb[:], in_=sel2[:])
        bq_sb = const.tile([P, CC], F32, tag="bq", name="bq")
        nc.sync.dma_start(out=bq_sb[:], in_=b_q[:])
        bk_sb = const.tile([P, CC], F32, tag="bk", name="bk")
        nc.sync.dma_start(out=bk_sb[:], in_=b_k[:])
        bv_sb = const.tile([P, CC], F32, tag="bv", name="bv")
        nc.sync.dma_start(out=bv_sb[:], in_=b_v[:])
        bfc_sb = const.tile([P, F // P], F32, tag="bfc", name="bfc")
        nc.sync.dma_start(out=bfc_sb[:], in_=b_fc[:])
        bpj_sb = const.tile([P, C], F32, tag="bpj", name="bpj")
        nc.sync.dma_start(out=bpj_sb[:], in_=b_pj[:])
        bot_sb = const.tile([P, C], F32, tag="bot", name="bot")
        nc.sync.dma_start(out=bot_sb[:], in_=b_ot[:])

        p_mid = es_mid.enter_context(tc.tile_pool(name="p_mid", bufs=1))
        p_x2 = es_x2.enter_context(tc.tile_pool(name="p_x2", bufs=1))
        p_wpj = es_wpj.enter_context(tc.tile_pool(name="p_wpj", bufs=1))
        p_y8 = es_y8.enter_context(tc.tile_pool(name="p_y8", bufs=1))
        p_y = es_y.enter_context(tc.tile_pool(name="p_y", bufs=1))
        p_qk = es_qk.enter_context(tc.tile_pool(name="p_qk", bufs=1))
        p_v = es_v.enter_context(tc.tile_pool(name="p_v", bufs=1))
        p_wv = es_wv.enter_context(tc.tile_pool(name="p_wv", bufs=1))

        # one 32 KiB/partition fp8 buffer triple-aliased across disjoint
        # lifetimes: [xlnT | wqk] (phases 1-3) then h8 (phases 7-8)
        buf32 = p_big.tile([P, 2 * CC * T2], FP8, tag="buf32", name="buf32")
        xlnT = buf32.rearrange("p (a c t) -> p a c t", a=2, t=T2)[:, 0]
        wqk_sb = buf32.rearrange("p (a pr hf f) -> p a pr hf f",
                                 a=2, pr=NPR, hf=2)[:, 1]
        h8 = buf32.rearrange("p (f t) -> p f t", t=TQ)
        wv_sb = p_wv.tile([P, NPR, 2, C], FP8, tag="wv", name="wv")
        wpj_sb = p_wpj.tile([P, NPR, 2, C], FP8, tag="wpj", name="wpj")

        # ---------------- Phase 1: LN1 + transpose + K (fused) --------------
        # K matmuls for token-block pairs are emitted as soon as their
        # transposes land, filling the PE during the DVE/ACT-bound LN loop.
        # Weight-stationary over 2 blocks so each 256-col DoubleRow weight
        # load amortizes over 2 matmuls; evacuations ((psum+64b)/64 -> bf16)
        # run on ScalarE (idle here) via the free affine: ps/64 + b_true.
        q_sb = [p_qk.tile([P, TQ], BF16, tag=f"q{i}", name=f"q{i}") for i in range(CC)]
        k_sb = [p_qk.tile([P, T2], BF16, tag=f"k{i}", name=f"k{i}") for i in range(CC)]

        def emit_k(blk, kps):
            for fc in range(CC):
                psK = kps.tile([P, N], F32, tag="k_ps", name="k_ps")
                for pr in range(NPR):
                    lk = wqk_sb[:, pr, :, C + fc * P:C + (fc + 1) * P]
                    nc.tensor.matmul(psK[:], lhsT=lk,
                                     rhs=xlnT[:, 2 * pr:2 * pr + 2, blk * N:(blk + 1) * N],
                                     start=(pr == 0), stop=(pr == NPR - 1),
                                     perf_mode=DR)
                nc.scalar.activation(k_sb[fc][:, blk * N:(blk + 1) * N],
                                     psK[:], AF.Identity,
                                     bias=bk_sb[:, fc:fc + 1], scale=1.0 / SW)

        with tc.tile_pool(name="qk_ps", bufs=4, space="PSUM") as kps:
            with tc.tile_pool(name="ln1_sp", bufs=3) as sp, \
                 tc.tile_pool(name="ln1_cp", bufs=1) as cp, \
                 tc.tile_pool(name="ln1_st", bufs=6) as st, \
                 tc.tile_pool(name="ln1_ps", bufs=3, space="PSUM") as tps:
                if apply_lnwb:
                    w1 = cp.tile([P, C], F32, tag="w1", name="w1")
                    nc.sync.dma_start(out=w1[:], in_=ln1w[:])
                    b1 = cp.tile([P, C], F32, tag="b1", name="b1")
                    nc.sync.dma_start(out=b1[:], in_=ln1b[:])
                for tt in range(T2 // P):
                    xt = sp.tile([P, C], F32, tag="xs", name="xs")
                    nc.sync.dma_start(out=xt[:], in_=x_seq[tt * P:(tt + 1) * P, :])
                    if tt == 2:
                        nc.sync.dma_start(out=wqk_sb[:], in_=w_qk3[:])
                        nc.sync.dma_start(out=wv_sb[:], in_=w_v3[:])
                        nc.sync.dma_start(out=wpj_sb[:], in_=w_pj3[:])
                    stats = st.tile([P, 2, 6], F32, tag="st", name="st")
                    for g in range(2):
                        nc.vector.bn_stats(out=stats[:, g, :], in_=xt[:, g * 512:(g + 1) * 512])
                    mv = st.tile([P, 2], F32, tag="mv", name="mv")
                    nc.vector.bn_aggr(out=mv[:], in_=stats[:])
                    rstd = st.tile([P, 1], F32, tag="rstd", name="rstd")
                    nc.scalar.activation(rstd[:], mv[:, 1:2], AF.Sqrt, bias=eps_sb[:], scale=1.0)
                    nc.vector.reciprocal(out=rstd[:], in_=rstd[:])
                    xb = sp.tile([P, C], BF16, tag="xb", name="xb")
                    nmr = st.tile([P, 1], F32, tag="nmr", name="nmr")
                    nc.vector.tensor_scalar(nmr[:], mv[:, 0:1], rstd[:], -1.0,
                                            OP.mult, OP.mult)
                    if apply_lnwb:
                        xc = sp.tile([P, C], F32, tag="xc", name="xc")
                        nc.scalar.activation(xc[:], xt[:], AF.Identity,
                                             bias=nmr[:], scale=rstd[:])
                        xw = sp.tile([P, C], F32, tag="xw", name="xw")
                        nc.vector.tensor_tensor(out=xw[:], in0=xc[:], in1=w1[:], op=OP.mult)
                        nc.vector.tensor_tensor(out=xb[:], in0=xw[:], in1=b1[:], op=OP.add)
                    else:
                        nc.scalar.activation(xb[:], xt[:], AF.Identity,
                                             bias=nmr[:], scale=rstd[:])
                    pst = tps.tile([P, CC, P], BF16, tag="trp", name="trp")
                    for cc in range(CC):
                        nc.tensor.transpose(pst[:, cc, :],
                                            xb[:, cc * P:(cc + 1) * P], ident[:])
                    nc.vector.tensor_copy(
                        out=xlnT[:, :, tt * P:(tt + 1) * P], in_=pst[:])
                    if tt % 4 == 3 and tt < 15:
                        emit_k(tt // 4, kps)
            emit_k(3, kps)

        # ---------------- Phase 2: Q projection (DoubleRow fp8) -------------
        with tc.tile_pool(name="q_ps", bufs=4, space="PSUM") as qps:
            for fc in range(CC):
                psQ = [qps.tile([P, N], F32, tag="q_ps", name="q_ps") for _ in range(2)]
                for pr in range(NPR):
                    lq = wqk_sb[:, pr, :, fc * P:(fc + 1) * P]
                    for blk in range(2):
                        nc.tensor.matmul(psQ[blk][:], lhsT=lq,
                                         rhs=xlnT[:, 2 * pr:2 * pr + 2, blk * N:(blk + 1) * N],
                                         start=(pr == 0), stop=(pr == NPR - 1),
                                         perf_mode=DR)
                for blk in range(2):
                    nc.scalar.activation(q_sb[fc][:, blk * N:(blk + 1) * N],
                                         psQ[blk][:], AF.Identity,
                                         bias=bq_sb[:, fc:fc + 1], scale=1.0 / SW)

        # ---------------- Phase 2b: V projection (DoubleRow fp8) ------------
        v_sb = [p_v.tile([P, H * 65], FP8, tag=f"v{i}", name=f"v{i}")
                for i in range(T2 // P)]
        for kt in range(T2 // P):
            nc.gpsimd.memset(
                v_sb[kt].rearrange("p (h d) -> p h d", d=65)[:, :, 64:65], 1.0)
        def emit_v(kt, vps):
            ps = [vps.tile([P, N], F32, tag="v_ps", name="v_ps") for _ in range(2)]
            for pr in range(NPR):
                lv = xlnT[:, 2 * pr:2 * pr + 2, kt * P:(kt + 1) * P]
                for vg in range(2):
                    nc.tensor.matmul(ps[vg][:], lhsT=lv,
                                     rhs=wv_sb[:, pr, :, vg * N:(vg + 1) * N],
                                     start=(pr == 0), stop=(pr == NPR - 1),
                                     perf_mode=DR)
            for vg in range(2):
                out_ap = v_sb[kt].rearrange("p (h d) -> p h d", d=65)[
                    :, vg * 8:(vg + 1) * 8, 0:64]
                in_ap = ps[vg].rearrange("p (h d) -> p h d", d=64)[:, :, :]
                nc.scalar.activation(out_ap, in_ap, AF.Identity,
                                     bias=0.0, scale=1.0 / SW)

        with tc.tile_pool(name="v_ps", bufs=4, space="PSUM") as vps:
            for kt in (8, 9, 10, 11, 0, 1, 2, 3):
                emit_v(kt, vps)
        v_queue = [4, 5, 6, 7, 12, 13, 14, 15]

        # ---------------- Phase 3: attention -------------------------------
        # Per head-pair: 24 causal slots/qb-group; score matmul pair runs
        # concurrently in PE row-groups 0-63/64-127. Each qb group leads with
        # its maskless slots so the previous group's DVE evacuation tail
        # drains before the first causal-mask multiply is needed.
        y_fm = [p_y.tile([P, TQ], BF16, tag=f"y{i}", name=f"y{i}") for i in range(CC)]
        with tc.tile_pool(name="att_at", bufs=1) as ap_pool, \
             tc.tile_pool(name="att_sps", bufs=2, space="PSUM") as sps_pool, \
             tc.tile_pool(name="att_yps", bufs=2, space="PSUM") as yps_pool, \
             tc.tile_pool(name="att_vps", bufs=2, space="PSUM") as avps:
            for hp in range(H // 2):
                for qb in (0, 1):
                    slots = SLOTS[qb]
                    yps = [yps_pool.tile([65, N], F32, tag="yps", name="yps")
                           for _ in range(2)]
                    last = len(slots) - 1
                    for i, (kt, kind, m) in enumerate(slots):
                        sp = sps_pool.tile([P, 2, N], F32, tag="sps", name="sps")
                        for j in range(2):
                            ro = j * 64
                            nc.tensor.matmul(
                                sp[:, j, :],
                                lhsT=k_sb[hp][ro:ro + 64, kt * P:(kt + 1) * P],
                                rhs=q_sb[hp][ro:ro + 64, qb * N:(qb + 1) * N],
                                start=True, stop=True)
                        if v_queue and hp == 0 and i % 2 == 1:
                            emit_v(v_queue.pop(0), avps)
                        at = ap_pool.tile([P, 2, N], FP8, tag="at", name="at", bufs=8)
                        bias = {"diag": 0.0, "full": 0.0,
                                "gate2": gate2_sb[:, 0:1],
                                "gate3": gate3_sb[:, 0:1]}[kind]
                        nc.scalar.activation(at[:, 0:2, :], sp[:, 0:2, :],
                                             AF.Exp, bias=bias, scale=0.125)
                        if kind == "diag":
                            nc.vector.tensor_tensor(
                                out=at[:, 0:2, :], in0=at[:, 0:2, :],
                                in1=mask_sb[:, m, :, :], op=OP.mult)
                        for j in range(2):
                            h = 2 * hp + j
                            nc.tensor.matmul(yps[j][:],
                                             lhsT=v_sb[kt][:, h * 65:(h + 1) * 65],
                                             rhs=at[:, j, :],
                                             start=(i == 0), stop=(i == last))
                    for j in range(2):
                        nc.vector.tensor_copy(
                            out=y_fm[hp][j * 64:(j + 1) * 64, qb * N:(qb + 1) * N],
                            in_=yps[j][0:64, :])
                        s1 = ap_pool.tile([1, N], F32, tag="s1", name="s1", bufs=4)
                        nc.vector.tensor_copy(out=s1[:], in_=yps[j][64:65, :])
                        nc.sync.dma_start(
                            out=sums_d[2 * hp + j:2 * hp + j + 1, qb * N:(qb + 1) * N],
                            in_=s1[:])

        # ---------------- Phase 4: normalize y -> y8 (x SY, + SY*b_v) ------
        # one lane-parallel reciprocal over all 16 heads' sums; partition
        # repack via DRAM roundtrip (engine partition bases are 0/32/64 only)
        y8 = p_y8.tile([P, CC, TQ], FP8, tag="y8", name="y8")
        with tc.tile_pool(name="att_rp", bufs=2) as rp, \
             tc.tile_pool(name="att_rps", bufs=2, space="PSUM") as rps_pool:
            s16 = rp.tile([16, TQ], F32, tag="s16", name="s16", bufs=1)
            nc.sync.dma_start(out=s16[:], in_=sums_d[:])
            recip16 = rp.tile([16, TQ], F32, tag="recip16", name="recip16", bufs=1)
            nc.vector.reciprocal(out=recip16[:], in_=s16[:])
            reciprr = rp.tile([16, TQ], F32R, tag="reciprr", name="reciprr", bufs=1)
            with nc.allow_low_precision(reason="f32r view of f32 recip"):
                nc.vector.tensor_scalar(reciprr[:], recip16[:], SY, None, OP.mult)
            for yt in range(CC):
                recip_r = rp.tile([2, TQ], F32R, tag="recipr", name="recipr", bufs=4)
                nc.sync.dma_start(out=recip_r[:], in_=reciprr[2 * yt:2 * yt + 2, :])
                for tt in range(2):
                    rps = rps_pool.tile([P, N], F32, tag="rps", name="rps")
                    nc.tensor.matmul(rps[:], lhsT=sel_sb[:],
                                     rhs=recip_r[:, tt * N:(tt + 1) * N],
                                     start=True, stop=True)
                    nc.vector.tensor_tensor(out=y8[:, yt, tt * N:(tt + 1) * N],
                                            in0=y_fm[yt][:, tt * N:(tt + 1) * N],
                                            in1=rps[:], op=OP.mult)
                    nc.vector.tensor_scalar(y8[:, yt, tt * N:(tt + 1) * N],
                                            y8[:, yt, tt * N:(tt + 1) * N],
                                            bv_sb[:, yt:yt + 1], None, OP.add)

        es_wv.close()
        es_v.close()
        es_qk.close()
        es_y.close()

        # ---------------- Phase 5: proj + residual + LN2 (fused) ------------
        # proj evacuation, residual add, LN2 and its transposes all run
        # per-128-token tile so PE pipelines the next tile's proj matmuls
        # under this tile's DVE/ACT work.
        x_mid = [p_mid.tile([P, C], BF16, tag=f"xm{i}", name=f"xm{i}")
                 for i in range(TQ // P)]
        xln2T = p_x2.tile([P, CC, TQ], FP8, tag="xln2T", name="xln2T")
        with tc.tile_pool(name="pj_sp", bufs=3) as sp, \
             tc.tile_pool(name="pj_cp", bufs=1) as cp, \
             tc.tile_pool(name="pj_st", bufs=6) as st, \
             tc.tile_pool(name="pj_ps", bufs=2, space="PSUM") as pps, \
             tc.tile_pool(name="ln2_ps", bufs=3, space="PSUM") as tps:
            if apply_lnwb:
                w2 = cp.tile([P, C], F32, tag="w2", name="w2")
                nc.sync.dma_start(out=w2[:], in_=ln2w[:])
                b2 = cp.tile([P, C], F32, tag="b2", name="b2")
                nc.sync.dma_start(out=b2[:], in_=ln2b[:])
            for t8 in range(TQ // P):
                xo = sp.tile([P, C], F32, tag="xo", name="xo")
                nc.sync.dma_start(out=xo[:], in_=x_seq[t8 * P:(t8 + 1) * P, :])
                ps2 = pps.tile([P, 2, N], F32, tag="pj_ps", name="pj_ps")
                for pr in range(NPR):
                    ly = y8[:, 2 * pr:2 * pr + 2, t8 * P:(t8 + 1) * P]
                    for ft in range(2):
                        nc.tensor.matmul(ps2[:, ft, :], lhsT=ly,
                                         rhs=wpj_sb[:, pr, :, ft * N:(ft + 1) * N],
                                         start=(pr == 0), stop=(pr == NPR - 1),
                                         perf_mode=DR)
                xt = x_mid[t8]
                nc.scalar.activation(xt[:], ps2[:, 0:2, :], AF.Identity,
                                     bias=0.0, scale=1.0 / (SW * SY))
                nc.vector.tensor_tensor(out=xt[:], in0=xt[:], in1=xo[:], op=OP.add)
                nc.gpsimd.tensor_tensor(out=xt[:], in0=xt[:], in1=bpj_sb[:], op=OP.add)
                # --- LN2 for this token tile ---
                stats = st.tile([P, 2, 6], F32, tag="st2", name="st2")
                for g in range(2):
                    nc.vector.bn_stats(out=stats[:, g, :], in_=xt[:, g * 512:(g + 1) * 512])
                mv = st.tile([P, 2], F32, tag="mv2", name="mv2")
                nc.vector.bn_aggr(out=mv[:], in_=stats[:])
                rstd = st.tile([P, 1], F32, tag="rstd2", name="rstd2")
                nc.scalar.activation(rstd[:], mv[:, 1:2], AF.Sqrt, bias=eps_sb[:], scale=1.0)
                nc.vector.reciprocal(out=rstd[:], in_=rstd[:])
                xb = sp.tile([P, C], BF16, tag="xb2", name="xb2")
                nmr = st.tile([P, 1], F32, tag="nmr2", name="nmr2")
                nc.vector.tensor_scalar(nmr[:], mv[:, 0:1], rstd[:], -1.0,
                                        OP.mult, OP.mult)
                if apply_lnwb:
                    xc = sp.tile([P, C], F32, tag="xc2", name="xc2")
                    nc.scalar.activation(xc[:], xt[:], AF.Identity,
                                         bias=nmr[:], scale=rstd[:])
                    xw = sp.tile([P, C], F32, tag="xw2", name="xw2")
                    nc.vector.tensor_tensor(out=xw[:], in0=xc[:], in1=w2[:], op=OP.mult)
                    nc.vector.tensor_tensor(out=xb[:], in0=xw[:], in1=b2[:], op=OP.add)
                else:
                    nc.scalar.activation(xb[:], xt[:], AF.Identity,
                                         bias=nmr[:], scale=rstd[:])
                pst = tps.tile([P, CC, P], BF16, tag="trp2", name="trp2")
                for cc in range(CC):
                    nc.tensor.transpose(pst[:, cc, :],
                                        xb[:, cc * P:(cc + 1) * P], ident[:])
                nc.vector.tensor_copy(
                    out=xln2T[:, :, t8 * P:(t8 + 1) * P], in_=pst[:])

        es_y8.close()
        es_wpj.close()

        # ---------------- Phase 7: FC + gelu (DoubleRow fp8) ---------------
        with tc.tile_pool(name="fc_w", bufs=3) as wp, \
             tc.tile_pool(name="fc_ps", bufs=3, space="PSUM") as fps:
            for hg in range(F // N):
                wt = wp.tile([P, NPR, 2, N], FP8, tag="wfc", name="wfc")
                nc.sync.dma_start(out=wt[:], in_=w_fc3[:, :, :, hg * N:(hg + 1) * N])
                for hs in range(4):
                    hf = hg * 4 + hs
                    ps2 = fps.tile([P, 2, N], F32, tag="fc_ps", name="fc_ps")
                    for pr in range(NPR):
                        lw = wt[:, pr, :, hs * P:(hs + 1) * P]
                        for tt in range(2):
                            nc.tensor.matmul(ps2[:, tt, :], lhsT=lw,
                                             rhs=xln2T[:, 2 * pr:2 * pr + 2, tt * N:(tt + 1) * N],
                                             start=(pr == 0), stop=(pr == NPR - 1),
                                             perf_mode=DR)
                    if not SIM_GELU:
                        nc.scalar.activation(h8[:, hf, :], ps2[:, 0:2, :],
                                             AF.Gelu_apprx_tanh,
                                             bias=bfc_sb[:, hf:hf + 1], scale=1.0 / SW)
                    else:
                        import math
                        cst = math.sqrt(2.0 / math.pi)
                        u = wp.tile([P, 2, N], F32, tag="g_u", name="g_u")
                        nc.scalar.activation(u[:], ps2[:, 0:2, :], AF.Identity,
                                             bias=bfc_sb[:, hf:hf + 1], scale=1.0 / SW)
                        u3 = wp.tile([P, 2, N], F32, tag="g_u3", name="g_u3")
                        nc.scalar.activation(u3[:], u[:], AF.Square, bias=0.0, scale=1.0)
                        nc.vector.tensor_tensor(out=u3[:], in0=u3[:], in1=u[:], op=OP.mult)
                        nc.vector.tensor_scalar(u3[:], u3[:], 0.044715, None, OP.mult)
                        nc.vector.tensor_tensor(out=u3[:], in0=u3[:], in1=u[:], op=OP.add)
                        tqh = wp.tile([P, 2, N], F32, tag="g_t", name="g_t")
                        nc.scalar.activation(tqh[:], u3[:], AF.Tanh, bias=0.0, scale=cst)
                        nc.vector.tensor_scalar(tqh[:], tqh[:], 1.0, None, OP.add)
                        nc.vector.tensor_tensor(out=tqh[:], in0=tqh[:], in1=u[:], op=OP.mult)
                        nc.vector.tensor_scalar(h8[:, hf, :], tqh[:], 0.5, None, OP.mult)

        es_x2.close()

        # ---------------- Phase 8: out matmul + residual (DoubleRow fp8) ---
        with tc.tile_pool(name="ot_w", bufs=6) as wp, \
             tc.tile_pool(name="ot_sp", bufs=3) as sp, \
             tc.tile_pool(name="ot_ps", bufs=8, space="PSUM") as ops_pool:
            for half in range(2):
                opss = [ops_pool.tile([P, N], F32, tag="ot_ps", name="ot_ps")
                        for _ in range(8)]
                for pr in range(F // 256):
                    wt = wp.tile([P, 2, C], FP8, tag="wot", name="wot")
                    nc.sync.dma_start(out=wt[:], in_=w_ot3[:, pr, :, :])
                    for tc4 in range(4):
                        t8 = half * 4 + tc4
                        lh = h8[:, 2 * pr:2 * pr + 2, t8 * P:(t8 + 1) * P]
                        for ft in range(2):
                            nc.tensor.matmul(opss[tc4 * 2 + ft][:], lhsT=lh,
                                             rhs=wt[:, :, ft * N:(ft + 1) * N],
                                             start=(pr == 0), stop=(pr == F // 256 - 1),
                                             perf_mode=DR)
                for tc4 in range(4):
                    t8 = half * 4 + tc4
                    ot = sp.tile([P, C], F32, tag="ot", name="ot")
                    for ft in range(2):
                        nc.scalar.activation(ot[:, ft * N:(ft + 1) * N],
                                             opss[tc4 * 2 + ft][:], AF.Identity,
                                             bias=0.0, scale=1.0 / SWO)
                    nc.vector.tensor_tensor(out=ot[:], in0=ot[:],
                                            in1=x_mid[t8][:], op=OP.add)
                    nc.vector.tensor_tensor(out=ot[:], in0=ot[:], in1=bot_sb[:], op=OP.add)
                    nc.sync.dma_start(out=out_d[t8 * P:(t8 + 1) * P, :], in_=ot[:])

    nc.finalize()
    return nc


def _own_blocks(s):
    return [0, 1, 2, 3, 12, 13, 14, 15] if s == 0 else list(range(4, 12))


def _prep_shared(inputs):
    f8 = ml_dtypes.float8_e4m3

    def pack_dr(wT, npr, scale):
        # wT: [K, M] (contraction-major); -> [P, npr, 2, M] with
        # [p, pr, hf, m] = scale * wT[pr*256 + hf*128 + p, m]
        K, M = wT.shape
        assert K == npr * 256
        a = (wT * scale).reshape(npr, 2, P, M).transpose(2, 0, 1, 3)
        return np.ascontiguousarray(a).astype(f8)

    W_attn = np.asarray(inputs["W_attn"], np.float32)
    shared = {
        "w_qk3": pack_dr(np.ascontiguousarray(W_attn[:2 * C].T), NPR, SW),
        "w_v3": pack_dr(np.ascontiguousarray(W_attn[2 * C:].T), NPR, SW),
        "w_pj3": pack_dr(np.ascontiguousarray(np.asarray(inputs["W_proj"], np.float32).T), NPR, SW),
        "w_fc3": pack_dr(np.ascontiguousarray(np.asarray(inputs["W_fc"], np.float32).T), NPR, SW),
        "w_ot3": pack_dr(np.ascontiguousarray(np.asarray(inputs["W_out"], np.float32).T), F // 256, SWO),
        "ln1w": np.ascontiguousarray(np.broadcast_to(np.asarray(inputs["ln1_w"], np.float32), (P, C))),
        "ln1b": np.ascontiguousarray(np.broadcast_to(np.asarray(inputs["ln1_b"], np.float32), (P, C))),
        "ln2w": np.ascontiguousarray(np.broadcast_to(np.asarray(inputs["ln2_w"], np.float32), (P, C))),
        "ln2b": np.ascontiguousarray(np.broadcast_to(np.asarray(inputs["ln2_b"], np.float32), (P, C))),
        "b_q": np.ascontiguousarray(np.asarray(inputs["b_attn"], np.float32)[:C].reshape(CC, P).T),
        "b_k": np.ascontiguousarray(np.asarray(inputs["b_attn"], np.float32)[C:2 * C].reshape(CC, P).T),
        "b_v": np.ascontiguousarray(np.asarray(inputs["b_attn"], np.float32)[2 * C:].reshape(CC, P).T) * SY,
        "b_pj": np.ascontiguousarray(np.broadcast_to(np.asarray(inputs["b_proj"], np.float32), (P, C))),
        "b_fc": np.ascontiguousarray(np.asarray(inputs["b_fc"], np.float32).reshape(F // P, P).T),
        "b_ot": np.ascontiguousarray(np.broadcast_to(np.asarray(inputs["b_out"], np.float32), (P, C))),
    }
    # mask4[p, m*N + qf] = 1 if qf >= m*128 + p else 0
    pp = np.arange(P)[:, None]
    qf = np.arange(N)[None, :]
    mask = np.zeros((P, 4, 2, N), np.float32)
    for m in range(4):
        mask[:, m, 0, :] = (qf >= m * P + pp)
        mask[:, m, 1, :] = mask[:, m, 0, :]
    shared["mask4"] = mask.astype(ml_dtypes.bfloat16)
    sel = np.zeros((2, P), np.float32)
    sel[0, :64] = 1.0
    sel[1, 64:] = 1.0
    shared["sel2"] = sel
    return shared


def _make_in_maps(inputs):
    x = np.asarray(inputs["x"], np.float32)
    shared = _prep_shared(inputs)
    in_maps = []
    for c in range(8):
        b, s = c // 2, c % 2
        own = _own_blocks(s)
        other = _own_blocks(1 - s)
        xb = x[b].reshape(16, P, C)
        m = dict(shared)
        m["x_seq"] = np.ascontiguousarray(
            np.concatenate([xb[own], xb[other]], axis=0).reshape(T2, C))
        m["gate2"] = np.full((P, 1), 0.0 if s == 1 else -1e30, np.float32)
        m["gate3"] = np.full((P, 1), 0.0 if s == 0 else -1e30, np.float32)
        in_maps.append(m)
    return in_maps


def _get_nc(apply_lnwb=True):
    key = ("nc", apply_lnwb, SIM_GELU)
    if key not in _CACHE:
        _CACHE[key] = _build_nc(apply_lnwb)
    return _CACHE[key]


def run_cores(inputs, profile=False):
    """Run the SPMD program; returns list of per-core result dicts."""
    global last_exec_time_ns
    apply_lnwb = not (
        np.allclose(np.asarray(inputs["ln1_w"]), 1.0)
        and np.allclose(np.asarray(inputs["ln1_b"]), 0.0)
        and np.allclose(np.asarray(inputs["ln2_w"]), 1.0)
        and np.allclose(np.asarray(inputs["ln2_b"]), 0.0))
    nc = _get_nc(apply_lnwb)
    in_maps = _make_in_maps(inputs)
    if profile:
        import concourse.bass_utils as bass_utils
        bass_utils.upload_artifacts = lambda tmpdir: "local://" + tmpdir
        try:
            from trn_agent_boot.trn_boot import _ntff_profile_via_ctypes
            import antenv.axon_hooks as hooks
            if hooks.get_axon_ntff_profile_hook() is None:
                hooks.set_axon_ntff_profile_hook(
                    _ntff_profile_via_ctypes("/opt/axon/libaxon_pjrt.so"))
        except Exception:
            pass
        res = bass_utils.run_bass_kernel_spmd(nc, in_maps, list(range(8)), trace=True)
        last_exec_time_ns = res.exec_time_ns
        return res.results
    return _cached_runner(nc)(in_maps)


def _cached_runner(nc):
    """Per-process cached jit of the SPMD executable so repeated kernel()
    calls don't recompile (mirrors bass2jax.run_bass_via_pjrt's multi-core
    branch)."""
    key = ("runner", id(nc))
    if key in _CACHE:
        return _CACHE[key]
    import jax
    import numpy as _np
    from jax.sharding import Mesh, PartitionSpec
    from jax.experimental.shard_map import shard_map
    from concourse import bass2jax, mybir as _mybir
    bass2jax.install_neuronx_cc_hook()

    part_name = nc.partition_id_tensor.name if nc.partition_id_tensor else None
    in_names, out_names, out_avals, zero_outs = [], [], [], []
    for alloc in nc.m.functions[0].allocations:
        if not isinstance(alloc, _mybir.MemoryLocationSet):
            continue
        name = alloc.memorylocations[0].name
        if alloc.kind == "ExternalInput":
            if name != part_name:
                in_names.append(name)
        elif alloc.kind == "ExternalOutput":
            out_names.append(name)
            shape = tuple(alloc.tensor_shape)
            dtype = _mybir.dt.np(alloc.dtype)
            out_avals.append(jax.core.ShapedArray(shape, dtype))
            zero_outs.append(_np.zeros(shape, dtype))
    n_params = len(in_names)
    all_names = in_names + out_names
    if part_name is not None:
        all_names = all_names + [part_name]
    donate = tuple(range(n_params, n_params + len(out_names)))
    if jax.default_backend() == "cpu":
        donate = ()  # cpu sim path can't alias donated outputs

    def _body(*args):
        operands = list(args)
        if part_name is not None:
            operands.append(bass2jax.partition_id_tensor())
        outs = bass2jax._bass_exec_p.bind(
            *operands, out_avals=tuple(out_avals), in_names=tuple(all_names),
            out_names=tuple(out_names), lowering_input_output_aliases=(),
            sim_require_finite=True, sim_require_nnan=True, nc=nc)
        return tuple(outs)

    devices = jax.devices()[:8]
    mesh = Mesh(_np.asarray(devices), ("core",))
    spec = (PartitionSpec("core"),) * (n_params + len(out_names))
    sharded = jax.jit(
        shard_map(_body, mesh=mesh, in_specs=spec,
                  out_specs=(PartitionSpec("core"),) * len(out_names),
                  check_rep=False),
        donate_argnums=donate, keep_unused=True)

    def run(in_maps):
        concat_in = [
            _np.concatenate([_np.asarray(in_maps[c][nm]) for c in range(8)], axis=0)
            for nm in in_names]
        concat_zero = [_np.zeros((8 * z.shape[0], *z.shape[1:]), z.dtype)
                       for z in zero_outs]
        out_arrs = sharded(*concat_in, *concat_zero)
        return [
            {nm: _np.asarray(out_arrs[i]).reshape(8, *out_avals[i].shape)[c]
             for i, nm in enumerate(out_names)}
            for c in range(8)]

    _CACHE[key] = run
    return run


def kernel(**inputs) -> np.ndarray:
    results = run_cores(inputs, profile=PROFILE)
    out = np.empty((B, T, C), np.float32)
    for c in range(8):
        b, s = c // 2, c % 2
        res = results[c]["out"]
        for j, blk in enumerate(_own_blocks(s)):
            out[b, blk * P:(blk + 1) * P, :] = res[j * P:(j + 1) * P]
    return out


# revision 53
# speedup vs baseline: 1.2958x; 1.0338x over previous
"""Trainium2 Bass kernel for a GPT-2 style transformer block.

Problem: B=4, T=2048, C=1024, H=16 heads (hd=64), MLP hidden 4096, fp32 I/O.

Sharding: zero-collective 8-way data parallel. Core c handles batch b=c//2;
s=c%2 selects its query set: s=0 owns the OUTER sequence quarters (blocks
0-3 and 12-15 of 128 tokens), s=1 the MIDDLE half (blocks 4-11). This makes
the causal-attention work symmetric across the pair: a uniform 24-tile
slot schedule per head covers both cores' needs, with per-core host-side
mask / gate tables providing the divergence. K/V are computed locally for
all 2048 tokens in own-first order.

Precision: all big weight matmuls (QKV, V, proj, FC, out) run in fp8e4
DoubleRow perf mode (2 contraction rows per PE cell per cycle): weights are
host-prescaled by 64 (W_out by 256) so N(0, 0.02)-scale values land in
e4m3's normal range; the scale is removed in the PSUM evacuation ops.
Attention q/k/v/exp-weights are fp8e4 as well (no DoubleRow; contraction is
only 64/128 deep), scores accumulate in f32 PSUM and softmax runs in f32 on
ScalarE. LayerNorm is f32 (bn_stats), residuals bf16/f32.

Layouts:
  x / residuals / final out: token-major [tok(P), C]
  x_ln transposed to feature-major [feat(P), chunk, tok] fp8 via PE
  transposes (bf16) + DVE convert-copy
  Q [feat(P), TQ], K [feat(P), T2] fp8; head pair hp lives in one tile
  (rows 0-63 head 2hp, 64-127 head 2hp+1) so score matmuls of a pair are
  emitted adjacently and run CONCURRENTLY in distinct PE row-groups
  V token-major [tok(P), h*65] fp8 with a built-in ones column per head
  (softmax row sums ride the AV matmul); softmax needs no max-subtraction
  (scores bounded ~|s|<4) and no transposes anywhere in attention
  normalization + v-bias deferred to after AV via a tiny K=2 f32r selector
  matmul that partition-broadcasts 16/sums
"""

import os
import sys
import types

import numpy as np
import ml_dtypes

for _p in ("/opt/trn_rl_repo", "/root/.axon_site/_ro/trn_rl_repo"):
    if os.path.isdir(_p) and _p not in sys.path:
        sys.path.append(_p)

# antenv.axon_hooks is absent in this image; bass_utils imports it when
# tracing under axon. Provide the trivial get/set holder it expects.
if "antenv.axon_hooks" not in sys.modules:
    try:
        import antenv

        _m = types.ModuleType("antenv.axon_hooks")
        _m._hook = None

        def _set_hook(h):
            _m._hook = h

        def _get_hook():
            return _m._hook

        _m.set_axon_ntff_profile_hook = _set_hook
        _m.get_axon_ntff_profile_hook = _get_hook
        sys.modules["antenv.axon_hooks"] = _m
        antenv.axon_hooks = _m
    except ImportError:
        pass

import concourse.bacc as bacc
import concourse.tile as tile
from concourse import mybir
from concourse.masks import make_identity

P = 128
B, T, C = 4, 2048, 1024
H, HD = 16, 64
F = 4096
T2 = T  # tokens per core for K/V (full sequence of one batch element)
TQ = T // 2  # own query tokens per core
CC = C // P  # 8 C-chunks
NPR = CC // 2  # 4 DoubleRow contraction pair-chunks (256 each)
N = 512  # moving free dim per matmul

SW = 64.0  # fp8 weight prescale (qkv/v/proj/fc)
SWO = 256.0  # fp8 weight prescale for W_out
SY = 16.0  # attention-output prescale into fp8

F32 = mybir.dt.float32
F32R = mybir.dt.float32r
BF16 = mybir.dt.bfloat16
FP8 = mybir.dt.float8e4
AF = mybir.ActivationFunctionType
OP = mybir.AluOpType
DR = mybir.MatmulPerfMode.DoubleRow

PROFILE = False
SIM_GELU = False  # CoreSim lacks the Gelu LUT; emulate with Tanh + DVE ops
last_exec_time_ns = None

_CACHE = {}

# per-(g,kt) attention slot schedule, uniform across cores.
# kinds: 'diag' (mask m), 'full', 'gate2' (live iff s==1), 'gate3' (iff s==0)
SLOTS0 = [(8, "gate2", 0), (9, "gate2", 0), (10, "gate2", 0), (11, "gate2", 0),
          (0, "diag", 0), (1, "diag", 1), (2, "diag", 2), (3, "diag", 3)]
SLOTS1 = [(0, "full", 0), (1, "full", 0), (2, "full", 0), (3, "full", 0),
          (4, "diag", 0), (5, "diag", 1), (6, "diag", 2), (7, "diag", 3),
          (8, "full", 0), (9, "full", 0), (10, "full", 0), (11, "full", 0),
          (12, "gate3", 0), (13, "gate3", 0), (14, "gate3", 0), (15, "gate3", 0)]
SLOTS = (SLOTS0, SLOTS1)


def _build_nc(apply_lnwb: bool = True, apply_bias: bool = True):
    nc = bacc.Bacc("TRN2", target_bir_lowering=False, debug=False, num_devices=8)

    x_seq = nc.dram_tensor("x_seq", [T2, C], F32, kind="ExternalInput")
    w_qk3 = nc.dram_tensor("w_qk3", [P, NPR, 2, 2 * C], FP8, kind="ExternalInput")
    w_v3 = nc.dram_tensor("w_v3", [P, NPR, 2, C], FP8, kind="ExternalInput")
    w_pj3 = nc.dram_tensor("w_pj3", [P, NPR, 2, C], FP8, kind="ExternalInput")
    w_fc3 = nc.dram_tensor("w_fc3", [P, NPR, 2, F], FP8, kind="ExternalInput")
    w_ot3 = nc.dram_tensor("w_ot3", [P, F // 256, 2, C], FP8, kind="ExternalInput")
    ln1w = nc.dram_tensor("ln1w", [P, C], F32, kind="ExternalInput")
    ln1b = nc.dram_tensor("ln1b", [P, C], F32, kind="ExternalInput")
    ln2w = nc.dram_tensor("ln2w", [P, C], F32, kind="ExternalInput")
    ln2b = nc.dram_tensor("ln2b", [P, C], F32, kind="ExternalInput")
    b_q = nc.dram_tensor("b_q", [P, CC], F32, kind="ExternalInput")  # x64
    b_k = nc.dram_tensor("b_k", [P, CC], F32, kind="ExternalInput")  # x64
    b_v = nc.dram_tensor("b_v", [P, CC], F32, kind="ExternalInput")  # x16, col per chunk
    b_pj = nc.dram_tensor("b_pj", [P, C], F32, kind="ExternalInput")
    b_fc = nc.dram_tensor("b_fc", [P, F // P], F32, kind="ExternalInput")
    b_ot = nc.dram_tensor("b_ot", [P, C], F32, kind="ExternalInput")
    mask4 = nc.dram_tensor("mask4", [P, 4, 2, N], BF16, kind="ExternalInput")
    gate2 = nc.dram_tensor("gate2", [P, 1], F32, kind="ExternalInput")
    gate3 = nc.dram_tensor("gate3", [P, 1], F32, kind="ExternalInput")
    sel2 = nc.dram_tensor("sel2", [2, P], F32R, kind="ExternalInput")

    out_d = nc.dram_tensor("out", [TQ, C], F32, kind="ExternalOutput")
    sums_d = nc.dram_tensor("sums_scratch", [16, TQ], F32)

    from contextlib import ExitStack

    with tile.TileContext(nc) as tc, ExitStack() as ctx:
        # pool enter order = reverse of close order (pool stack is LIFO);
        # SBUF is reserved from first tile creation to pool close
        const = ctx.enter_context(tc.tile_pool(name="const", bufs=1))
        p_big = ctx.enter_context(tc.tile_pool(name="p_big", bufs=1))
        es_mid = ctx.enter_context(ExitStack())
        es_x2 = ctx.enter_context(ExitStack())
        es_wpj = ctx.enter_context(ExitStack())
        es_y8 = ctx.enter_context(ExitStack())
        es_y = ctx.enter_context(ExitStack())
        es_qk = ctx.enter_context(ExitStack())
        es_v = ctx.enter_context(ExitStack())
        es_wv = ctx.enter_context(ExitStack())

        ident = const.tile([P, P], BF16, tag="ident", name="ident")
        make_identity(nc, ident)
        eps_sb = const.tile([P, 1], F32, tag="eps", name="eps")
        nc.vector.memset(eps_sb[:], 1e-5)
        mask_sb = const.tile([P, 4, 2, N], BF16, tag="mask", name="mask")
        nc.sync.dma_start(out=mask_sb[:], in_=mask4[:])
        gate2_sb = const.tile([P, 1], F32, tag="g2", name="g2")
        nc.sync.dma_start(out=gate2_sb[:], in_=gate2[:])
        gate3_sb = const.tile([P, 1], F32, tag="g3", name="g3")
        nc.sync.dma_start(out=gate3_sb[:], in_=gate3[:])
        sel_sb = const.tile([2, P], F32R, tag="sel", name="sel")
        nc.sync.dma_start(out=sel_sb[:], in_=sel2[:])
        bq_sb = const.tile([P, CC], F32, tag="bq", name="bq")
        nc.sync.dma_start(out=bq_sb[:], in_=b_q[:])
        bk_sb = const.tile([P, CC], F32, tag="bk", name="bk")
        nc.sync.dma_start(out=bk_sb[:], in_=b_k[:])
        bv_sb = const.tile([P, CC], F32, tag="bv", name="bv")
        nc.sync.dma_start(out=bv_sb[:], in_=b_v[:])
        bfc_sb = const.tile([P, F // P], F32, tag="bfc", name="bfc")
        nc.sync.dma_start(out=bfc_sb[:], in_=b_fc[:])
        bpj_sb = const.tile([P, C], F32, tag="bpj", name="bpj")
        nc.sync.dma_start(out=bpj_sb[:], in_=b_pj[:])
        bot_sb = const.tile([P, C], F32, tag="bot", name="bot")
        nc.sync.dma_start(out=bot_sb[:], in_=b_ot[:])

        p_mid = es_mid.enter_context(tc.tile_pool(name="p_mid", bufs=1))
        p_x2 = es_x2.enter_context(tc.tile_pool(name="p_x2", bufs=1))
        p_wpj = es_wpj.enter_context(tc.tile_pool(name="p_wpj", bufs=1))
        p_y8 = es_y8.enter_context(tc.tile_pool(name="p_y8", bufs=1))
        p_y = es_y.enter_context(tc.tile_pool(name="p_y", bufs=1))
        p_qk = es_qk.enter_context(tc.tile_pool(name="p_qk", bufs=1))
        p_v = es_v.enter_context(tc.tile_pool(name="p_v", bufs=1))
        p_wv = es_wv.enter_context(tc.tile_pool(name="p_wv", bufs=1))

        # one 32 KiB/partition fp8 buffer triple-aliased across disjoint
        # lifetimes: [xlnT | wqk] (phases 1-3) then h8 (phases 7-8)
        buf32 = p_big.tile([P, 2 * CC * T2], FP8, tag="buf32", name="buf32")
        xlnT = buf32.rearrange("p (a c t) -> p a c t", a=2, t=T2)[:, 0]
        wqk_sb = buf32.rearrange("p (a pr hf f) -> p a pr hf f",
                                 a=2, pr=NPR, hf=2)[:, 1]
        h8 = buf32.rearrange("p (f t) -> p f t", t=TQ)
        wv_sb = p_wv.tile([P, NPR, 2, C], FP8, tag="wv", name="wv")
        wpj_sb = p_wpj.tile([P, NPR, 2, C], FP8, tag="wpj", name="wpj")

        # ---------------- Phase 1: LN1 + transpose + K (fused) --------------
        # K matmuls for token-block pairs are emitted as soon as their
        # transposes land, filling the PE during the DVE/ACT-bound LN loop.
        # Weight-stationary over 2 blocks so each 256-col DoubleRow weight
        # load amortizes over 2 matmuls; evacuations ((psum+64b)/64 -> bf16)
        # run on ScalarE (idle here) via the free affine: ps/64 + b_true.
        q_sb = [p_qk.tile([P, TQ], BF16, tag=f"q{i}", name=f"q{i}") for i in range(CC)]
        k_sb = [p_qk.tile([P, T2], BF16, tag=f"k{i}", name=f"k{i}") for i in range(CC)]

        def emit_k(blk, kps):
            for fc in range(CC):
                psK = kps.tile([P, N], F32, tag="k_ps", name="k_ps")
                for pr in range(NPR):
                    lk = wqk_sb[:, pr, :, C + fc * P:C + (fc + 1) * P]
                    nc.tensor.matmul(psK[:], lhsT=lk,
                                     rhs=xlnT[:, 2 * pr:2 * pr + 2, blk * N:(blk + 1) * N],
                                     start=(pr == 0), stop=(pr == NPR - 1),
                                     perf_mode=DR)
                nc.scalar.activation(k_sb[fc][:, blk * N:(blk + 1) * N],
                                     psK[:], AF.Identity,
                                     bias=bk_sb[:, fc:fc + 1], scale=1.0 / SW)

        with tc.tile_pool(name="qk_ps", bufs=4, space="PSUM") as kps:
            with tc.tile_pool(name="ln1_sp", bufs=3) as sp, \
                 tc.tile_pool(name="ln1_cp", bufs=1) as cp, \
                 tc.tile_pool(name="ln1_st", bufs=6) as st, \
                 tc.tile_pool(name="ln1_ps", bufs=3, space="PSUM") as tps:
                if apply_lnwb:
                    w1 = cp.tile([P, C], F32, tag="w1", name="w1")
                    nc.sync.dma_start(out=w1[:], in_=ln1w[:])
                    b1 = cp.tile([P, C], F32, tag="b1", name="b1")
                    nc.sync.dma_start(out=b1[:], in_=ln1b[:])
                for tt in range(T2 // P):
                    xt = sp.tile([P, C], F32, tag="xs", name="xs")
                    nc.sync.dma_start(out=xt[:], in_=x_seq[tt * P:(tt + 1) * P, :])
                    if tt == 2:
                        nc.sync.dma_start(out=wqk_sb[:], in_=w_qk3[:])
                        nc.sync.dma_start(out=wv_sb[:], in_=w_v3[:])
                        nc.sync.dma_start(out=wpj_sb[:], in_=w_pj3[:])
                    stats = st.tile([P, 2, 6], F32, tag="st", name="st")
                    for g in range(2):
                        nc.vector.bn_stats(out=stats[:, g, :], in_=xt[:, g * 512:(g + 1) * 512])
                    mv = st.tile([P, 2], F32, tag="mv", name="mv")
                    nc.vector.bn_aggr(out=mv[:], in_=stats[:])
                    rstd = st.tile([P, 1], F32, tag="rstd", name="rstd")
                    nc.scalar.activation(rstd[:], mv[:, 1:2], AF.Sqrt, bias=eps_sb[:], scale=1.0)
                    nc.vector.reciprocal(out=rstd[:], in_=rstd[:])
                    xb = sp.tile([P, C], BF16, tag="xb", name="xb")
                    nmr = st.tile([P, 1], F32, tag="nmr", name="nmr")
                    nc.vector.tensor_scalar(nmr[:], mv[:, 0:1], rstd[:], -1.0,
                                            OP.mult, OP.mult)
                    if apply_lnwb:
                        xc = sp.tile([P, C], F32, tag="xc", name="xc")
                        nc.scalar.activation(xc[:], xt[:], AF.Identity,
                                             bias=nmr[:], scale=rstd[:])
                        xw = sp.tile([P, C], F32, tag="xw", name="xw")
                        nc.vector.tensor_tensor(out=xw[:], in0=xc[:], in1=w1[:], op=OP.mult)
                        nc.vector.tensor_tensor(out=xb[:], in0=xw[:], in1=b1[:], op=OP.add)
                    else:
                        nc.scalar.activation(xb[:], xt[:], AF.Identity,
                                             bias=nmr[:], scale=rstd[:])
                    pst = tps.tile([P, CC, P], BF16, tag="trp", name="trp")
                    for cc in range(CC):
                        nc.tensor.transpose(pst[:, cc, :],
                                            xb[:, cc * P:(cc + 1) * P], ident[:])
                    nc.vector.tensor_copy(
                        out=xlnT[:, :, tt * P:(tt + 1) * P], in_=pst[:])
                    if tt % 4 == 3 and tt < 15:
                        emit_k(tt // 4, kps)
            emit_k(3, kps)

        # ---------------- Phase 2: Q projection, fc=0 only ------------------
        # remaining Q chunks and V tiles are fed into attention idle PE slots
        def emit_q(fc, qps):
            psQ = [qps.tile([P, N], F32, tag="v_ps", name="v_ps") for _ in range(2)]
            for pr in range(NPR):
                lq = wqk_sb[:, pr, :, fc * P:(fc + 1) * P]
                for blk in range(2):
                    nc.tensor.matmul(psQ[blk][:], lhsT=lq,
                                     rhs=xlnT[:, 2 * pr:2 * pr + 2, blk * N:(blk + 1) * N],
                                     start=(pr == 0), stop=(pr == NPR - 1),
                                     perf_mode=DR)
            for blk in range(2):
                nc.vector.tensor_scalar(q_sb[fc][:, blk * N:(blk + 1) * N],
                                        psQ[blk][:], bq_sb[:, fc:fc + 1],
                                        1.0 / SW, OP.add, OP.mult)

        def emit_v_dve(kt, vps):
            ps = [vps.tile([P, N], F32, tag="v_ps", name="v_ps") for _ in range(2)]
            for pr in range(NPR):
                lv = xlnT[:, 2 * pr:2 * pr + 2, kt * P:(kt + 1) * P]
                for vg in range(2):
                    nc.tensor.matmul(ps[vg][:], lhsT=lv,
                                     rhs=wv_sb[:, pr, :, vg * N:(vg + 1) * N],
                                     start=(pr == 0), stop=(pr == NPR - 1),
                                     perf_mode=DR)
            for vg in range(2):
                out_ap = v_sb[kt].rearrange("p (h d) -> p h d", d=65)[
                    :, vg * 8:(vg + 1) * 8, 0:64]
                in_ap = ps[vg].rearrange("p (h d) -> p h d", d=64)[:, :, :]
                nc.vector.tensor_scalar(out_ap, in_ap, 1.0 / SW, None, OP.mult)

        with tc.tile_pool(name="q_ps", bufs=4, space="PSUM") as qps:
            emit_q(0, qps)

        # ---------------- Phase 2b: V projection (DoubleRow fp8) ------------
        v_sb = [p_v.tile([P, H * 65], FP8, tag=f"v{i}", name=f"v{i}")
                for i in range(T2 // P)]
        for kt in range(T2 // P):
            nc.gpsimd.memset(
                v_sb[kt].rearrange("p (h d) -> p h d", d=65)[:, :, 64:65], 1.0)
        def emit_v(kt, vps):
            ps = [vps.tile([P, N], F32, tag="v_ps", name="v_ps") for _ in range(2)]
            for pr in range(NPR):
                lv = xlnT[:, 2 * pr:2 * pr + 2, kt * P:(kt + 1) * P]
                for vg in range(2):
                    nc.tensor.matmul(ps[vg][:], lhsT=lv,
                                     rhs=wv_sb[:, pr, :, vg * N:(vg + 1) * N],
                                     start=(pr == 0), stop=(pr == NPR - 1),
                                     perf_mode=DR)
            for vg in range(2):
                out_ap = v_sb[kt].rearrange("p (h d) -> p h d", d=65)[
                    :, vg * 8:(vg + 1) * 8, 0:64]
                in_ap = ps[vg].rearrange("p (h d) -> p h d", d=64)[:, :, :]
                nc.scalar.activation(out_ap, in_ap, AF.Identity,
                                     bias=0.0, scale=1.0 / SW)

        with tc.tile_pool(name="v_ps", bufs=4, space="PSUM") as vps:
            for kt in (8, 9, 10, 11):
                emit_v(kt, vps)

        # ---------------- Phase 3: attention -------------------------------
        # Per head-pair: 24 causal slots/qb-group; score matmul pair runs
        # concurrently in PE row-groups 0-63/64-127. Each qb group leads with
        # its maskless slots so the previous group's DVE evacuation tail
        # drains before the first causal-mask multiply is needed.
        y_fm = [p_y.tile([P, TQ], BF16, tag=f"y{i}", name=f"y{i}") for i in range(CC)]
        feeds = {}
        for sl, kt in zip(((0, 0, 0), (0, 0, 1), (0, 0, 2), (0, 0, 3),
                           (0, 1, 0), (0, 1, 2), (0, 1, 4), (0, 1, 6),
                           (0, 1, 8), (0, 1, 10), (0, 1, 12), (0, 1, 14)),
                          (0, 1, 2, 3, 4, 5, 6, 7, 12, 13, 14, 15)):
            feeds[sl] = (lambda kt: lambda pool: emit_v_dve(kt, pool))(kt)
        feeds[(0, 1, 9)] = lambda pool: emit_q(1, pool)
        for h in range(1, 7):
            feeds[(h, 0, 1)] = (lambda fc: lambda pool: emit_q(fc, pool))(h + 1)
        with tc.tile_pool(name="att_at", bufs=1) as ap_pool, \
             tc.tile_pool(name="att_sps", bufs=2, space="PSUM") as sps_pool, \
             tc.tile_pool(name="att_yps", bufs=2, space="PSUM") as yps_pool, \
             tc.tile_pool(name="att_vps", bufs=2, space="PSUM") as avps:
            for hp in range(H // 2):
                for qb in (0, 1):
                    slots = SLOTS[qb]
                    yps = [yps_pool.tile([65, N], F32, tag="yps", name="yps")
                           for _ in range(2)]
                    last = len(slots) - 1
                    for i, (kt, kind, m) in enumerate(slots):
                        sp = sps_pool.tile([P, 2, N], F32, tag="sps", name="sps")
                        for j in range(2):
                            ro = j * 64
                            nc.tensor.matmul(
                                sp[:, j, :],
                                lhsT=k_sb[hp][ro:ro + 64, kt * P:(kt + 1) * P],
                                rhs=q_sb[hp][ro:ro + 64, qb * N:(qb + 1) * N],
                                start=True, stop=True)
                        fd = feeds.get((hp, qb, i))
                        if fd is not None:
                            fd(avps)
                        at = ap_pool.tile([P, 2, N], FP8, tag="at", name="at", bufs=8)
                        bias = {"diag": 0.0, "full": 0.0,
                                "gate2": gate2_sb[:, 0:1],
                                "gate3": gate3_sb[:, 0:1]}[kind]
                        nc.scalar.activation(at[:, 0:2, :], sp[:, 0:2, :],
                                             AF.Exp, bias=bias, scale=0.125)
                        if kind == "diag":
                            nc.vector.tensor_tensor(
                                out=at[:, 0:2, :], in0=at[:, 0:2, :],
                                in1=mask_sb[:, m, :, :], op=OP.mult)
                        for j in range(2):
                            h = 2 * hp + j
                            nc.tensor.matmul(yps[j][:],
                                             lhsT=v_sb[kt][:, h * 65:(h + 1) * 65],
                                             rhs=at[:, j, :],
                                             start=(i == 0), stop=(i == last))
                    for j in range(2):
                        nc.vector.tensor_copy(
                            out=y_fm[hp][j * 64:(j + 1) * 64, qb * N:(qb + 1) * N],
                            in_=yps[j][0:64, :])
                        s1 = ap_pool.tile([1, N], F32, tag="s1", name="s1", bufs=4)
                        nc.vector.tensor_copy(out=s1[:], in_=yps[j][64:65, :])
                        nc.sync.dma_start(
                            out=sums_d[2 * hp + j:2 * hp + j + 1, qb * N:(qb + 1) * N],
                            in_=s1[:])

        # ---------------- Phase 4: normalize y -> y8 (x SY, + SY*b_v) ------
        # one lane-parallel reciprocal over all 16 heads' sums; partition
        # repack via DRAM roundtrip (engine partition bases are 0/32/64 only)
        y8 = p_y8.tile([P, CC, TQ], FP8, tag="y8", name="y8")
        with tc.tile_pool(name="att_rp", bufs=2) as rp, \
             tc.tile_pool(name="att_rps", bufs=2, space="PSUM") as rps_pool:
            s16 = rp.tile([16, TQ], F32, tag="s16", name="s16", bufs=1)
            nc.sync.dma_start(out=s16[:], in_=sums_d[:])
            recip16 = rp.tile([16, TQ], F32, tag="recip16", name="recip16", bufs=1)
            nc.vector.reciprocal(out=recip16[:], in_=s16[:])
            reciprr = rp.tile([16, TQ], F32R, tag="reciprr", name="reciprr", bufs=1)
            with nc.allow_low_precision(reason="f32r view of f32 recip"):
                nc.vector.tensor_scalar(reciprr[:], recip16[:], SY, None, OP.mult)
            for yt in range(CC):
                recip_r = rp.tile([2, TQ], F32R, tag="recipr", name="recipr", bufs=4)
                nc.sync.dma_start(out=recip_r[:], in_=reciprr[2 * yt:2 * yt + 2, :])
                for tt in range(2):
                    rps = rps_pool.tile([P, N], F32, tag="rps", name="rps")
                    nc.tensor.matmul(rps[:], lhsT=sel_sb[:],
                                     rhs=recip_r[:, tt * N:(tt + 1) * N],
                                     start=True, stop=True)
                    nc.vector.tensor_tensor(out=y8[:, yt, tt * N:(tt + 1) * N],
                                            in0=y_fm[yt][:, tt * N:(tt + 1) * N],
                                            in1=rps[:], op=OP.mult)
                    if apply_bias:
                        nc.vector.tensor_scalar(y8[:, yt, tt * N:(tt + 1) * N],
                                                y8[:, yt, tt * N:(tt + 1) * N],
                                                bv_sb[:, yt:yt + 1], None, OP.add)

        es_wv.close()
        es_v.close()
        es_qk.close()
        es_y.close()

        # ---------------- Phase 5: proj + residual + LN2 (fused) ------------
        # proj evacuation, residual add, LN2 and its transposes all run
        # per-128-token tile so PE pipelines the next tile's proj matmuls
        # under this tile's DVE/ACT work.
        x_mid = [p_mid.tile([P, C], BF16, tag=f"xm{i}", name=f"xm{i}")
                 for i in range(TQ // P)]
        xln2T = p_x2.tile([P, CC, TQ], FP8, tag="xln2T", name="xln2T")
        with tc.tile_pool(name="pj_sp", bufs=3) as sp, \
             tc.tile_pool(name="pj_cp", bufs=1) as cp, \
             tc.tile_pool(name="pj_st", bufs=6) as st, \
             tc.tile_pool(name="pj_ps", bufs=2, space="PSUM") as pps, \
             tc.tile_pool(name="ln2_ps", bufs=3, space="PSUM") as tps:
            if apply_lnwb:
                w2 = cp.tile([P, C], F32, tag="w2", name="w2")
                nc.sync.dma_start(out=w2[:], in_=ln2w[:])
                b2 = cp.tile([P, C], F32, tag="b2", name="b2")
                nc.sync.dma_start(out=b2[:], in_=ln2b[:])
            for t8 in range(TQ // P):
                xo = sp.tile([P, C], F32, tag="xo", name="xo")
                nc.sync.dma_start(out=xo[:], in_=x_seq[t8 * P:(t8 + 1) * P, :])
                ps2 = pps.tile([P, 2, N], F32, tag="pj_ps", name="pj_ps")
                for pr in range(NPR):
                    ly = y8[:, 2 * pr:2 * pr + 2, t8 * P:(t8 + 1) * P]
                    for ft in range(2):
                        nc.tensor.matmul(ps2[:, ft, :], lhsT=ly,
                                         rhs=wpj_sb[:, pr, :, ft * N:(ft + 1) * N],
                                         start=(pr == 0), stop=(pr == NPR - 1),
                                         perf_mode=DR)
                xt = x_mid[t8]
                nc.scalar.activation(xt[:], ps2[:, 0:2, :], AF.Identity,
                                     bias=0.0, scale=1.0 / (SW * SY))
                nc.vector.tensor_tensor(out=xt[:], in0=xt[:], in1=xo[:], op=OP.add)
                if apply_bias:
                    nc.gpsimd.tensor_tensor(out=xt[:], in0=xt[:], in1=bpj_sb[:], op=OP.add)
                # --- LN2 for this token tile ---
                stats = st.tile([P, 2, 6], F32, tag="st2", name="st2")
                for g in range(2):
                    nc.vector.bn_stats(out=stats[:, g, :], in_=xt[:, g * 512:(g + 1) * 512])
                mv = st.tile([P, 2], F32, tag="mv2", name="mv2")
                nc.vector.bn_aggr(out=mv[:], in_=stats[:])
                rstd = st.tile([P, 1], F32, tag="rstd2", name="rstd2")
                nc.scalar.activation(rstd[:], mv[:, 1:2], AF.Sqrt, bias=eps_sb[:], scale=1.0)
                nc.vector.reciprocal(out=rstd[:], in_=rstd[:])
                xb = sp.tile([P, C], BF16, tag="xb2", name="xb2")
                nmr = st.tile([P, 1], F32, tag="nmr2", name="nmr2")
                nc.vector.tensor_scalar(nmr[:], mv[:, 0:1], rstd[:], -1.0,
                                        OP.mult, OP.mult)
                if apply_lnwb:
                    xc = sp.tile([P, C], F32, tag="xc2", name="xc2")
                    nc.scalar.activation(xc[:], xt[:], AF.Identity,
                                         bias=nmr[:], scale=rstd[:])
                    xw = sp.tile([P, C], F32, tag="xw2", name="xw2")
                    nc.vector.tensor_tensor(out=xw[:], in0=xc[:], in1=w2[:], op=OP.mult)
                    nc.vector.tensor_tensor(out=xb[:], in0=xw[:], in1=b2[:], op=OP.add)
                else:
                    nc.scalar.activation(xb[:], xt[:], AF.Identity,
                                         bias=nmr[:], scale=rstd[:])
                pst = tps.tile([P, CC, P], BF16, tag="trp2", name="trp2")
                for cc in range(CC):
                    nc.tensor.transpose(pst[:, cc, :],
                                        xb[:, cc * P:(cc + 1) * P], ident[:])
                nc.vector.tensor_copy(
                    out=xln2T[:, :, t8 * P:(t8 + 1) * P], in_=pst[:])

        es_y8.close()
        es_wpj.close()

        # ---------------- Phase 7: FC + gelu (DoubleRow fp8) ---------------
        with tc.tile_pool(name="fc_w", bufs=3) as wp, \
             tc.tile_pool(name="fc_ps", bufs=3, space="PSUM") as fps:
            for hg in range(F // N):
                wt = wp.tile([P, NPR, 2, N], FP8, tag="wfc", name="wfc")
                nc.sync.dma_start(out=wt[:], in_=w_fc3[:, :, :, hg * N:(hg + 1) * N])
                for hs in range(4):
                    hf = hg * 4 + hs
                    ps2 = fps.tile([P, 2, N], F32, tag="fc_ps", name="fc_ps")
                    for pr in range(NPR):
                        lw = wt[:, pr, :, hs * P:(hs + 1) * P]
                        for tt in range(2):
                            nc.tensor.matmul(ps2[:, tt, :], lhsT=lw,
                                             rhs=xln2T[:, 2 * pr:2 * pr + 2, tt * N:(tt + 1) * N],
                                             start=(pr == 0), stop=(pr == NPR - 1),
                                             perf_mode=DR)
                    if not SIM_GELU:
                        nc.scalar.activation(h8[:, hf, :], ps2[:, 0:2, :],
                                             AF.Gelu_apprx_tanh,
                                             bias=bfc_sb[:, hf:hf + 1], scale=1.0 / SW)
                    else:
                        import math
                        cst = math.sqrt(2.0 / math.pi)
                        u = wp.tile([P, 2, N], F32, tag="g_u", name="g_u")
                        nc.scalar.activation(u[:], ps2[:, 0:2, :], AF.Identity,
                                             bias=bfc_sb[:, hf:hf + 1], scale=1.0 / SW)
                        u3 = wp.tile([P, 2, N], F32, tag="g_u3", name="g_u3")
                        nc.scalar.activation(u3[:], u[:], AF.Square, bias=0.0, scale=1.0)
                        nc.vector.tensor_tensor(out=u3[:], in0=u3[:], in1=u[:], op=OP.mult)
                        nc.vector.tensor_scalar(u3[:], u3[:], 0.044715, None, OP.mult)
                        nc.vector.tensor_tensor(out=u3[:], in0=u3[:], in1=u[:], op=OP.add)
                        tqh = wp.tile([P, 2, N], F32, tag="g_t", name="g_t")
                        nc.scalar.activation(tqh[:], u3[:], AF.Tanh, bias=0.0, scale=cst)
                        nc.vector.tensor_scalar(tqh[:], tqh[:], 1.0, None, OP.add)
                        nc.vector.tensor_tensor(out=tqh[:], in0=tqh[:], in1=u[:], op=OP.mult)
                        nc.vector.tensor_scalar(h8[:, hf, :], tqh[:], 0.5, None, OP.mult)

        es_x2.close()

        # ---------------- Phase 8: out matmul + residual (DoubleRow fp8) ---
        with tc.tile_pool(name="ot_w", bufs=6) as wp, \
             tc.tile_pool(name="ot_sp", bufs=3) as sp, \
             tc.tile_pool(name="ot_ps", bufs=8, space="PSUM") as ops_pool:
            for half in range(2):
                opss = [ops_pool.tile([P, N], F32, tag="ot_ps", name="ot_ps")
                        for _ in range(8)]
                for pr in range(F // 256):
                    wt = wp.tile([P, 2, C], FP8, tag="wot", name="wot")
                    nc.sync.dma_start(out=wt[:], in_=w_ot3[:, pr, :, :])
                    for tc4 in range(4):
                        t8 = half * 4 + tc4
                        lh = h8[:, 2 * pr:2 * pr + 2, t8 * P:(t8 + 1) * P]
                        for ft in range(2):
                            nc.tensor.matmul(opss[tc4 * 2 + ft][:], lhsT=lh,
                                             rhs=wt[:, :, ft * N:(ft + 1) * N],
                                             start=(pr == 0), stop=(pr == F // 256 - 1),
                                             perf_mode=DR)
                for tc4 in range(4):
                    t8 = half * 4 + tc4
                    ot = sp.tile([P, C], F32, tag="ot", name="ot")
                    for ft in range(2):
                        nc.scalar.activation(ot[:, ft * N:(ft + 1) * N],
                                             opss[tc4 * 2 + ft][:], AF.Identity,
                                             bias=0.0, scale=1.0 / SWO)
                    nc.vector.tensor_tensor(out=ot[:], in0=ot[:],
                                            in1=x_mid[t8][:], op=OP.add)
                    if apply_bias:
                        nc.vector.tensor_tensor(out=ot[:], in0=ot[:], in1=bot_sb[:], op=OP.add)
                    nc.sync.dma_start(out=out_d[t8 * P:(t8 + 1) * P, :], in_=ot[:])

    nc.finalize()
    return nc


def _own_blocks(s):
    return [0, 1, 2, 3, 12, 13, 14, 15] if s == 0 else list(range(4, 12))


def _prep_shared(inputs):
    f8 = ml_dtypes.float8_e4m3

    def pack_dr(wT, npr, scale):
        # wT: [K, M] (contraction-major); -> [P, npr, 2, M] with
        # [p, pr, hf, m] = scale * wT[pr*256 + hf*128 + p, m]
        K, M = wT.shape
        assert K == npr * 256
        a = (wT * scale).reshape(npr, 2, P, M).transpose(2, 0, 1, 3)
        return np.ascontiguousarray(a).astype(f8)

    W_attn = np.asarray(inputs["W_attn"], np.float32)
    shared = {
        "w_qk3": pack_dr(np.ascontiguousarray(W_attn[:2 * C].T), NPR, SW),
        "w_v3": pack_dr(np.ascontiguousarray(W_attn[2 * C:].T), NPR, SW),
        "w_pj3": pack_dr(np.ascontiguousarray(np.asarray(inputs["W_proj"], np.float32).T), NPR, SW),
        "w_fc3": pack_dr(np.ascontiguousarray(np.asarray(inputs["W_fc"], np.float32).T), NPR, SW),
        "w_ot3": pack_dr(np.ascontiguousarray(np.asarray(inputs["W_out"], np.float32).T), F // 256, SWO),
        "ln1w": np.ascontiguousarray(np.broadcast_to(np.asarray(inputs["ln1_w"], np.float32), (P, C))),
        "ln1b": np.ascontiguousarray(np.broadcast_to(np.asarray(inputs["ln1_b"], np.float32), (P, C))),
        "ln2w": np.ascontiguousarray(np.broadcast_to(np.asarray(inputs["ln2_w"], np.float32), (P, C))),
        "ln2b": np.ascontiguousarray(np.broadcast_to(np.asarray(inputs["ln2_b"], np.float32), (P, C))),
        "b_q": np.ascontiguousarray(np.asarray(inputs["b_attn"], np.float32)[:C].reshape(CC, P).T) * SW,
        "b_k": np.ascontiguousarray(np.asarray(inputs["b_attn"], np.float32)[C:2 * C].reshape(CC, P).T),
        "b_v": np.ascontiguousarray(np.asarray(inputs["b_attn"], np.float32)[2 * C:].reshape(CC, P).T) * SY,
        "b_pj": np.ascontiguousarray(np.broadcast_to(np.asarray(inputs["b_proj"], np.float32), (P, C))),
        "b_fc": np.ascontiguousarray(np.asarray(inputs["b_fc"], np.float32).reshape(F // P, P).T),
        "b_ot": np.ascontiguousarray(np.broadcast_to(np.asarray(inputs["b_out"], np.float32), (P, C))),
    }
    # mask4[p, m*N + qf] = 1 if qf >= m*128 + p else 0
    pp = np.arange(P)[:, None]
    qf = np.arange(N)[None, :]
    mask = np.zeros((P, 4, 2, N), np.float32)
    for m in range(4):
        mask[:, m, 0, :] = (qf >= m * P + pp)
        mask[:, m, 1, :] = mask[:, m, 0, :]
    shared["mask4"] = mask.astype(ml_dtypes.bfloat16)
    sel = np.zeros((2, P), np.float32)
    sel[0, :64] = 1.0
    sel[1, 64:] = 1.0
    shared["sel2"] = sel
    return shared


def _make_in_maps(inputs):
    x = np.asarray(inputs["x"], np.float32)
    shared = _prep_shared(inputs)
    in_maps = []
    for c in range(8):
        b, s = c // 2, c % 2
        own = _own_blocks(s)
        other = _own_blocks(1 - s)
        xb = x[b].reshape(16, P, C)
        m = dict(shared)
        m["x_seq"] = np.ascontiguousarray(
            np.concatenate([xb[own], xb[other]], axis=0).reshape(T2, C))
        m["gate2"] = np.full((P, 1), 0.0 if s == 1 else -1e30, np.float32)
        m["gate3"] = np.full((P, 1), 0.0 if s == 0 else -1e30, np.float32)
        in_maps.append(m)
    return in_maps


def _get_nc(apply_lnwb=True, apply_bias=True):
    key = ("nc", apply_lnwb, apply_bias, SIM_GELU)
    if key not in _CACHE:
        _CACHE[key] = _build_nc(apply_lnwb, apply_bias)
    return _CACHE[key]


def run_cores(inputs, profile=False):
    """Run the SPMD program; returns list of per-core result dicts."""
    global last_exec_time_ns
    apply_lnwb = not (
        np.allclose(np.asarray(inputs["ln1_w"]), 1.0)
        and np.allclose(np.asarray(inputs["ln1_b"]), 0.0)
        and np.allclose(np.asarray(inputs["ln2_w"]), 1.0)
        and np.allclose(np.asarray(inputs["ln2_b"]), 0.0))
    apply_bias = not (
        np.allclose(np.asarray(inputs["b_attn"]), 0.0)
        and np.allclose(np.asarray(inputs["b_proj"]), 0.0)
        and np.allclose(np.asarray(inputs["b_out"]), 0.0))
    nc = _get_nc(apply_lnwb, apply_bias)
    in_maps = _make_in_maps(inputs)
    if profile:
        import concourse.bass_utils as bass_utils
        bass_utils.upload_artifacts = lambda tmpdir: "local://" + tmpdir
        try:
            from trn_agent_boot.trn_boot import _ntff_profile_via_ctypes
            import antenv.axon_hooks as hooks
            if hooks.get_axon_ntff_profile_hook() is None:
                hooks.set_axon_ntff_profile_hook(
                    _ntff_profile_via_ctypes("/opt/axon/libaxon_pjrt.so"))
        except Exception:
            pass
        res = bass_utils.run_bass_kernel_spmd(nc, in_maps, list(range(8)), trace=True)
        last_exec_time_ns = res.exec_time_ns
        return res.results
    return _cached_runner(nc)(in_maps)


def _cached_runner(nc):
    """Per-process cached jit of the SPMD executable so repeated kernel()
    calls don't recompile (mirrors bass2jax.run_bass_via_pjrt's multi-core
    branch)."""
    key = ("runner", id(nc))
    if key in _CACHE:
        return _CACHE[key]
    import jax
    import numpy as _np
    from jax.sharding import Mesh, PartitionSpec
    from jax.experimental.shard_map import shard_map
    from concourse import bass2jax, mybir as _mybir
    bass2jax.install_neuronx_cc_hook()

    part_name = nc.partition_id_tensor.name if nc.partition_id_tensor else None
    in_names, out_names, out_avals, zero_outs = [], [], [], []
    for alloc in nc.m.functions[0].allocations:
        if not isinstance(alloc, _mybir.MemoryLocationSet):
            continue
        name = alloc.memorylocations[0].name
        if alloc.kind == "ExternalInput":
            if name != part_name:
                in_names.append(name)
        elif alloc.kind == "ExternalOutput":
            out_names.append(name)
            shape = tuple(alloc.tensor_shape)
            dtype = _mybir.dt.np(alloc.dtype)
            out_avals.append(jax.core.ShapedArray(shape, dtype))
            zero_outs.append(_np.zeros(shape, dtype))
    n_params = len(in_names)
    all_names = in_names + out_names
    if part_name is not None:
        all_names = all_names + [part_name]
    donate = tuple(range(n_params, n_params + len(out_names)))
    if jax.default_backend() == "cpu":
        donate = ()  # cpu sim path can't alias donated outputs

    def _body(*args):
        operands = list(args)
        if part_name is not None:
            operands.append(bass2jax.partition_id_tensor())
        outs = bass2jax._bass_exec_p.bind(
            *operands, out_avals=tuple(out_avals), in_names=tuple(all_names),
            out_names=tuple(out_names), lowering_input_output_aliases=(),
            sim_require_finite=True, sim_require_nnan=True, nc=nc)
        return tuple(outs)

    devices = jax.devices()[:8]
    mesh = Mesh(_np.asarray(devices), ("core",))
    spec = (PartitionSpec("core"),) * (n_params + len(out_names))
    sharded = jax.jit(
        shard_map(_body, mesh=mesh, in_specs=spec,
                  out_specs=(PartitionSpec("core"),) * len(out_names),
                  check_rep=False),
        donate_argnums=donate, keep_unused=True)

    def run(in_maps):
        concat_in = [
            _np.concatenate([_np.asarray(in_maps[c][nm]) for c in range(8)], axis=0)
            for nm in in_names]
        concat_zero = [_np.zeros((8 * z.shape[0], *z.shape[1:]), z.dtype)
                       for z in zero_outs]
        out_arrs = sharded(*concat_in, *concat_zero)
        return [
            {nm: _np.asarray(out_arrs[i]).reshape(8, *out_avals[i].shape)[c]
             for i, nm in enumerate(out_names)}
            for c in range(8)]

    _CACHE[key] = run
    return run


def kernel(**inputs) -> np.ndarray:
    results = run_cores(inputs, profile=PROFILE)
    out = np.empty((B, T, C), np.float32)
    for c in range(8):
        b, s = c // 2, c % 2
        res = results[c]["out"]
        for j, blk in enumerate(_own_blocks(s)):
            out[b, blk * P:(blk + 1) * P, :] = res[j * P:(j + 1) * P]
    return out


# revision 55
# speedup vs baseline: 1.3044x; 1.0066x over previous
"""Trainium2 Bass kernel for a GPT-2 style transformer block.

Problem: B=4, T=2048, C=1024, H=16 heads (hd=64), MLP hidden 4096, fp32 I/O.

Sharding: zero-collective 8-way data parallel. Core c handles batch b=c//2;
s=c%2 selects its query set: s=0 owns the OUTER sequence quarters (blocks
0-3 and 12-15 of 128 tokens), s=1 the MIDDLE half (blocks 4-11). This makes
the causal-attention work symmetric across the pair: a uniform 24-tile
slot schedule per head covers both cores' needs, with per-core host-side
mask / gate tables providing the divergence. K/V are computed locally for
all 2048 tokens in own-first order.

Precision: all big weight matmuls (QKV, V, proj, FC, out) run in fp8e4
DoubleRow perf mode (2 contraction rows per PE cell per cycle): weights are
host-prescaled by 64 (W_out by 256) so N(0, 0.02)-scale values land in
e4m3's normal range; the scale is removed in the PSUM evacuation ops.
Attention q/k/v/exp-weights are fp8e4 as well (no DoubleRow; contraction is
only 64/128 deep), scores accumulate in f32 PSUM and softmax runs in f32 on
ScalarE. LayerNorm is f32 (bn_stats), residuals bf16/f32.

Layouts:
  x / residuals / final out: token-major [tok(P), C]
  x_ln transposed to feature-major [feat(P), chunk, tok] fp8 via PE
  transposes (bf16) + DVE convert-copy
  Q [feat(P), TQ], K [feat(P), T2] fp8; head pair hp lives in one tile
  (rows 0-63 head 2hp, 64-127 head 2hp+1) so score matmuls of a pair are
  emitted adjacently and run CONCURRENTLY in distinct PE row-groups
  V token-major [tok(P), h*65] fp8 with a built-in ones column per head
  (softmax row sums ride the AV matmul); softmax needs no max-subtraction
  (scores bounded ~|s|<4) and no transposes anywhere in attention
  normalization + v-bias deferred to after AV via a tiny K=2 f32r selector
  matmul that partition-broadcasts 16/sums
"""

import os
import sys
import types

import numpy as np
import ml_dtypes

for _p in ("/opt/trn_rl_repo", "/root/.axon_site/_ro/trn_rl_repo"):
    if os.path.isdir(_p) and _p not in sys.path:
        sys.path.append(_p)

# antenv.axon_hooks is absent in this image; bass_utils imports it when
# tracing under axon. Provide the trivial get/set holder it expects.
if "antenv.axon_hooks" not in sys.modules:
    try:
        import antenv

        _m = types.ModuleType("antenv.axon_hooks")
        _m._hook = None

        def _set_hook(h):
            _m._hook = h

        def _get_hook():
            return _m._hook

        _m.set_axon_ntff_profile_hook = _set_hook
        _m.get_axon_ntff_profile_hook = _get_hook
        sys.modules["antenv.axon_hooks"] = _m
        antenv.axon_hooks = _m
    except ImportError:
        pass

import concourse.bacc as bacc
import concourse.tile as tile
from concourse import mybir
from concourse.masks import make_identity

P = 128
B, T, C = 4, 2048, 1024
H, HD = 16, 64
F = 4096
T2 = T  # tokens per core for K/V (full sequence of one batch element)
TQ = T // 2  # own query tokens per core
CC = C // P  # 8 C-chunks
NPR = CC // 2  # 4 DoubleRow contraction pair-chunks (256 each)
N = 512  # moving free dim per matmul

SW = 64.0  # fp8 weight prescale (qkv/v/proj/fc)
SWO = 256.0  # fp8 weight prescale for W_out
SY = 16.0  # attention-output prescale into fp8

F32 = mybir.dt.float32
F32R = mybir.dt.float32r
BF16 = mybir.dt.bfloat16
FP8 = mybir.dt.float8e4
AF = mybir.ActivationFunctionType
OP = mybir.AluOpType
DR = mybir.MatmulPerfMode.DoubleRow

PROFILE = False
SIM_GELU = False  # CoreSim lacks the Gelu LUT; emulate with Tanh + DVE ops
last_exec_time_ns = None

_CACHE = {}

# per-(g,kt) attention slot schedule, uniform across cores.
# kinds: 'diag' (mask m), 'full', 'gate2' (live iff s==1), 'gate3' (iff s==0)
SLOTS0 = [(8, "gate2", 0), (9, "gate2", 0), (10, "gate2", 0), (11, "gate2", 0),
          (0, "diag", 0), (1, "diag", 1), (2, "diag", 2), (3, "diag", 3)]
SLOTS1 = [(0, "full", 0), (1, "full", 0), (2, "full", 0), (3, "full", 0),
          (4, "diag", 0), (5, "diag", 1), (6, "diag", 2), (7, "diag", 3),
          (8, "full", 0), (9, "full", 0), (10, "full", 0), (11, "full", 0),
          (12, "gate3", 0), (13, "gate3", 0), (14, "gate3", 0), (15, "gate3", 0)]
SLOTS = (SLOTS0, SLOTS1)


def _build_nc(apply_lnwb: bool = True, apply_bias: bool = True):
    nc = bacc.Bacc("TRN2", target_bir_lowering=False, debug=False, num_devices=8)

    x_seq = nc.dram_tensor("x_seq", [T2, C], F32, kind="ExternalInput")
    w_qk3 = nc.dram_tensor("w_qk3", [P, NPR, 2, 2 * C], FP8, kind="ExternalInput")
    w_v3 = nc.dram_tensor("w_v3", [P, NPR, 2, C], FP8, kind="ExternalInput")
    w_pj3 = nc.dram_tensor("w_pj3", [P, NPR, 2, C], FP8, kind="ExternalInput")
    w_fc3 = nc.dram_tensor("w_fc3", [P, NPR, 2, F], FP8, kind="ExternalInput")
    w_ot3 = nc.dram_tensor("w_ot3", [P, F // 256, 2, C], FP8, kind="ExternalInput")
    ln1w = nc.dram_tensor("ln1w", [P, C], F32, kind="ExternalInput")
    ln1b = nc.dram_tensor("ln1b", [P, C], F32, kind="ExternalInput")
    ln2w = nc.dram_tensor("ln2w", [P, C], F32, kind="ExternalInput")
    ln2b = nc.dram_tensor("ln2b", [P, C], F32, kind="ExternalInput")
    b_q = nc.dram_tensor("b_q", [P, CC], F32, kind="ExternalInput")  # x64
    b_k = nc.dram_tensor("b_k", [P, CC], F32, kind="ExternalInput")  # x64
    b_v = nc.dram_tensor("b_v", [P, CC], F32, kind="ExternalInput")  # x16, col per chunk
    b_pj = nc.dram_tensor("b_pj", [P, C], F32, kind="ExternalInput")
    b_fc = nc.dram_tensor("b_fc", [P, F // P], F32, kind="ExternalInput")
    b_ot = nc.dram_tensor("b_ot", [P, C], F32, kind="ExternalInput")
    mask4 = nc.dram_tensor("mask4", [P, 4, 2, N], BF16, kind="ExternalInput")
    gate2 = nc.dram_tensor("gate2", [P, 1], F32, kind="ExternalInput")
    gate3 = nc.dram_tensor("gate3", [P, 1], F32, kind="ExternalInput")
    sel2 = nc.dram_tensor("sel2", [2, P], F32R, kind="ExternalInput")

    out_d = nc.dram_tensor("out", [TQ, C], F32, kind="ExternalOutput")
    sums_d = nc.dram_tensor("sums_scratch", [16, TQ], F32)

    from contextlib import ExitStack

    with tile.TileContext(nc) as tc, ExitStack() as ctx:
        # pool enter order = reverse of close order (pool stack is LIFO);
        # SBUF is reserved from first tile creation to pool close
        const = ctx.enter_context(tc.tile_pool(name="const", bufs=1))
        p_big = ctx.enter_context(tc.tile_pool(name="p_big", bufs=1))
        es_mid = ctx.enter_context(ExitStack())
        es_x2 = ctx.enter_context(ExitStack())
        es_wpj = ctx.enter_context(ExitStack())
        es_y8 = ctx.enter_context(ExitStack())
        es_y = ctx.enter_context(ExitStack())
        es_qk = ctx.enter_context(ExitStack())
        es_v = ctx.enter_context(ExitStack())
        es_wv = ctx.enter_context(ExitStack())

        ident = const.tile([P, P], BF16, tag="ident", name="ident")
        make_identity(nc, ident)
        eps_sb = const.tile([P, 1], F32, tag="eps", name="eps")
        nc.vector.memset(eps_sb[:], 1e-5)
        mask_sb = const.tile([P, 4, 2, N], BF16, tag="mask", name="mask")
        nc.sync.dma_start(out=mask_sb[:], in_=mask4[:])
        gate2_sb = const.tile([P, 1], F32, tag="g2", name="g2")
        nc.sync.dma_start(out=gate2_sb[:], in_=gate2[:])
        gate3_sb = const.tile([P, 1], F32, tag="g3", name="g3")
        nc.sync.dma_start(out=gate3_sb[:], in_=gate3[:])
        sel_sb = const.tile([2, P], F32R, tag="sel", name="sel")
        nc.sync.dma_start(out=sel_sb[:], in_=sel2[:])
        bq_sb = const.tile([P, CC], F32, tag="bq", name="bq")
        nc.sync.dma_start(out=bq_sb[:], in_=b_q[:])
        bk_sb = const.tile([P, CC], F32, tag="bk", name="bk")
        nc.sync.dma_start(out=bk_sb[:], in_=b_k[:])
        bv_sb = const.tile([P, CC], F32, tag="bv", name="bv")
        nc.sync.dma_start(out=bv_sb[:], in_=b_v[:])
        bfc_sb = const.tile([P, F // P], F32, tag="bfc", name="bfc")
        nc.sync.dma_start(out=bfc_sb[:], in_=b_fc[:])
        bpj_sb = const.tile([P, C], F32, tag="bpj", name="bpj")
        nc.sync.dma_start(out=bpj_sb[:], in_=b_pj[:])
        bot_sb = const.tile([P, C], F32, tag="bot", name="bot")
        nc.sync.dma_start(out=bot_sb[:], in_=b_ot[:])

        p_mid = es_mid.enter_context(tc.tile_pool(name="p_mid", bufs=1))
        p_x2 = es_x2.enter_context(tc.tile_pool(name="p_x2", bufs=1))
        p_wpj = es_wpj.enter_context(tc.tile_pool(name="p_wpj", bufs=1))
        p_y8 = es_y8.enter_context(tc.tile_pool(name="p_y8", bufs=1))
        p_y = es_y.enter_context(tc.tile_pool(name="p_y", bufs=1))
        p_qk = es_qk.enter_context(tc.tile_pool(name="p_qk", bufs=1))
        p_v = es_v.enter_context(tc.tile_pool(name="p_v", bufs=1))
        p_wv = es_wv.enter_context(tc.tile_pool(name="p_wv", bufs=1))

        # one 32 KiB/partition fp8 buffer triple-aliased across disjoint
        # lifetimes: [xlnT | wqk] (phases 1-3) then h8 (phases 7-8)
        buf32 = p_big.tile([P, 2 * CC * T2], FP8, tag="buf32", name="buf32")
        xlnT = buf32.rearrange("p (a c t) -> p a c t", a=2, t=T2)[:, 0]
        wqk_sb = buf32.rearrange("p (a pr hf f) -> p a pr hf f",
                                 a=2, pr=NPR, hf=2)[:, 1]
        h8 = buf32.rearrange("p (f t) -> p f t", t=TQ)
        wv_sb = p_wv.tile([P, NPR, 2, C], FP8, tag="wv", name="wv")
        wpj_sb = p_wpj.tile([P, NPR, 2, C], FP8, tag="wpj", name="wpj")

        # ---------------- Phase 1: LN1 + transpose + K (fused) --------------
        # K matmuls for token-block pairs are emitted as soon as their
        # transposes land, filling the PE during the DVE/ACT-bound LN loop.
        # Weight-stationary over 2 blocks so each 256-col DoubleRow weight
        # load amortizes over 2 matmuls; evacuations ((psum+64b)/64 -> bf16)
        # run on ScalarE (idle here) via the free affine: ps/64 + b_true.
        q_sb = [p_qk.tile([P, TQ], BF16, tag=f"q{i}", name=f"q{i}") for i in range(CC)]
        k_sb = [p_qk.tile([P, T2], BF16, tag=f"k{i}", name=f"k{i}") for i in range(CC)]

        def emit_k(blk, kps):
            for fc in range(CC):
                psK = kps.tile([P, N], F32, tag="k_ps", name="k_ps")
                for pr in range(NPR):
                    lk = wqk_sb[:, pr, :, C + fc * P:C + (fc + 1) * P]
                    nc.tensor.matmul(psK[:], lhsT=lk,
                                     rhs=xlnT[:, 2 * pr:2 * pr + 2, blk * N:(blk + 1) * N],
                                     start=(pr == 0), stop=(pr == NPR - 1),
                                     perf_mode=DR)
                nc.scalar.activation(k_sb[fc][:, blk * N:(blk + 1) * N],
                                     psK[:], AF.Identity,
                                     bias=bk_sb[:, fc:fc + 1], scale=1.0 / SW)

        with tc.tile_pool(name="qk_ps", bufs=4, space="PSUM") as kps:
            with tc.tile_pool(name="ln1_sp", bufs=3) as sp, \
                 tc.tile_pool(name="ln1_cp", bufs=1) as cp, \
                 tc.tile_pool(name="ln1_st", bufs=6) as st, \
                 tc.tile_pool(name="ln1_ps", bufs=3, space="PSUM") as tps:
                if apply_lnwb:
                    w1 = cp.tile([P, C], F32, tag="w1", name="w1")
                    nc.sync.dma_start(out=w1[:], in_=ln1w[:])
                    b1 = cp.tile([P, C], F32, tag="b1", name="b1")
                    nc.sync.dma_start(out=b1[:], in_=ln1b[:])
                for tt in range(T2 // P):
                    xt = sp.tile([P, C], F32, tag="xs", name="xs")
                    nc.sync.dma_start(out=xt[:], in_=x_seq[tt * P:(tt + 1) * P, :])
                    if tt == 2:
                        nc.sync.dma_start(out=wqk_sb[:], in_=w_qk3[:])
                        nc.sync.dma_start(out=wv_sb[:], in_=w_v3[:])
                        nc.sync.dma_start(out=wpj_sb[:], in_=w_pj3[:])
                    stats = st.tile([P, 2, 6], F32, tag="st", name="st")
                    for g in range(2):
                        nc.vector.bn_stats(out=stats[:, g, :], in_=xt[:, g * 512:(g + 1) * 512])
                    mv = st.tile([P, 2], F32, tag="mv", name="mv")
                    nc.vector.bn_aggr(out=mv[:], in_=stats[:])
                    rstd = st.tile([P, 1], F32, tag="rstd", name="rstd")
                    nc.scalar.activation(rstd[:], mv[:, 1:2], AF.Sqrt, bias=eps_sb[:], scale=1.0)
                    nc.vector.reciprocal(out=rstd[:], in_=rstd[:])
                    xb = sp.tile([P, C], BF16, tag="xb", name="xb")
                    nmr = st.tile([P, 1], F32, tag="nmr", name="nmr")
                    nc.vector.tensor_scalar(nmr[:], mv[:, 0:1], rstd[:], -1.0,
                                            OP.mult, OP.mult)
                    if apply_lnwb:
                        xc = sp.tile([P, C], F32, tag="xc", name="xc")
                        nc.scalar.activation(xc[:], xt[:], AF.Identity,
                                             bias=nmr[:], scale=rstd[:])
                        xw = sp.tile([P, C], F32, tag="xw", name="xw")
                        nc.vector.tensor_tensor(out=xw[:], in0=xc[:], in1=w1[:], op=OP.mult)
                        nc.vector.tensor_tensor(out=xb[:], in0=xw[:], in1=b1[:], op=OP.add)
                    else:
                        nc.scalar.activation(xb[:], xt[:], AF.Identity,
                                             bias=nmr[:], scale=rstd[:])
                    pst = tps.tile([P, CC, P], BF16, tag="trp", name="trp")
                    for cc in range(CC):
                        nc.tensor.transpose(pst[:, cc, :],
                                            xb[:, cc * P:(cc + 1) * P], ident[:])
                    nc.vector.tensor_copy(
                        out=xlnT[:, :, tt * P:(tt + 1) * P], in_=pst[:])
                    if tt % 4 == 3 and tt < 15:
                        emit_k(tt // 4, kps)
            emit_k(3, kps)

        # ---------------- Phase 2: Q projection, fc=0 only ------------------
        # remaining Q chunks and V tiles are fed into attention idle PE slots
        def emit_q(fc, qps):
            psQ = [qps.tile([P, N], F32, tag="v_ps", name="v_ps") for _ in range(2)]
            for pr in range(NPR):
                lq = wqk_sb[:, pr, :, fc * P:(fc + 1) * P]
                for blk in range(2):
                    nc.tensor.matmul(psQ[blk][:], lhsT=lq,
                                     rhs=xlnT[:, 2 * pr:2 * pr + 2, blk * N:(blk + 1) * N],
                                     start=(pr == 0), stop=(pr == NPR - 1),
                                     perf_mode=DR)
            for blk in range(2):
                nc.vector.tensor_scalar(q_sb[fc][:, blk * N:(blk + 1) * N],
                                        psQ[blk][:], bq_sb[:, fc:fc + 1],
                                        1.0 / SW, OP.add, OP.mult)

        def emit_v_dve(kt, vps):
            ps = [vps.tile([P, N], F32, tag="v_ps", name="v_ps") for _ in range(2)]
            for pr in range(NPR):
                lv = xlnT[:, 2 * pr:2 * pr + 2, kt * P:(kt + 1) * P]
                for vg in range(2):
                    nc.tensor.matmul(ps[vg][:], lhsT=lv,
                                     rhs=wv_sb[:, pr, :, vg * N:(vg + 1) * N],
                                     start=(pr == 0), stop=(pr == NPR - 1),
                                     perf_mode=DR)
            for vg in range(2):
                out_ap = v_sb[kt].rearrange("p (h d) -> p h d", d=65)[
                    :, vg * 8:(vg + 1) * 8, 0:64]
                in_ap = ps[vg].rearrange("p (h d) -> p h d", d=64)[:, :, :]
                nc.vector.tensor_scalar(out_ap, in_ap, 1.0 / SW, None, OP.mult)

        with tc.tile_pool(name="q_ps", bufs=4, space="PSUM") as qps:
            emit_q(0, qps)

        # ---------------- Phase 2b: V projection (DoubleRow fp8) ------------
        v_sb = [p_v.tile([P, H * 65], FP8, tag=f"v{i}", name=f"v{i}")
                for i in range(T2 // P)]
        for kt in range(T2 // P):
            nc.gpsimd.memset(
                v_sb[kt].rearrange("p (h d) -> p h d", d=65)[:, :, 64:65], 1.0)
        def emit_v(kt, vps):
            ps = [vps.tile([P, N], F32, tag="v_ps", name="v_ps") for _ in range(2)]
            for pr in range(NPR):
                lv = xlnT[:, 2 * pr:2 * pr + 2, kt * P:(kt + 1) * P]
                for vg in range(2):
                    nc.tensor.matmul(ps[vg][:], lhsT=lv,
                                     rhs=wv_sb[:, pr, :, vg * N:(vg + 1) * N],
                                     start=(pr == 0), stop=(pr == NPR - 1),
                                     perf_mode=DR)
            for vg in range(2):
                out_ap = v_sb[kt].rearrange("p (h d) -> p h d", d=65)[
                    :, vg * 8:(vg + 1) * 8, 0:64]
                in_ap = ps[vg].rearrange("p (h d) -> p h d", d=64)[:, :, :]
                nc.scalar.activation(out_ap, in_ap, AF.Identity,
                                     bias=0.0, scale=1.0 / SW)

        with tc.tile_pool(name="v_ps", bufs=4, space="PSUM") as vps:
            for kt in (8, 9, 10, 11):
                emit_v(kt, vps)

        # ---------------- Phase 3: attention -------------------------------
        # Per head-pair: 24 causal slots/qb-group; score matmul pair runs
        # concurrently in PE row-groups 0-63/64-127. Each qb group leads with
        # its maskless slots so the previous group's DVE evacuation tail
        # drains before the first causal-mask multiply is needed.
        y_fm = [p_y.tile([P, TQ], BF16, tag=f"y{i}", name=f"y{i}") for i in range(CC)]
        feeds = {}
        for sl, kt in zip(((0, 0, 0), (0, 0, 1), (0, 0, 2), (0, 0, 3),
                           (0, 1, 0), (0, 1, 2), (0, 1, 4), (0, 1, 6),
                           (0, 1, 8), (0, 1, 10), (0, 1, 12), (0, 1, 14)),
                          (0, 1, 2, 3, 4, 5, 6, 7, 12, 13, 14, 15)):
            feeds[sl] = (lambda kt: lambda pool: emit_v_dve(kt, pool))(kt)
        feeds[(0, 1, 9)] = lambda pool: emit_q(1, pool)
        for h in range(1, 7):
            feeds[(h, 0, 1)] = (lambda fc: lambda pool: emit_q(fc, pool))(h + 1)
        with tc.tile_pool(name="att_at", bufs=1) as ap_pool, \
             tc.tile_pool(name="att_sps", bufs=2, space="PSUM") as sps_pool, \
             tc.tile_pool(name="att_yps", bufs=2, space="PSUM") as yps_pool, \
             tc.tile_pool(name="att_vps", bufs=2, space="PSUM") as avps:
            for hp in range(H // 2):
                for qb in (0, 1):
                    slots = SLOTS[qb]
                    yps = [yps_pool.tile([65, N], F32, tag="yps", name="yps")
                           for _ in range(2)]
                    last = len(slots) - 1
                    for i, (kt, kind, m) in enumerate(slots):
                        sp = sps_pool.tile([P, 2, N], F32, tag="sps", name="sps")
                        for j in range(2):
                            ro = j * 64
                            nc.tensor.matmul(
                                sp[:, j, :],
                                lhsT=k_sb[hp][ro:ro + 64, kt * P:(kt + 1) * P],
                                rhs=q_sb[hp][ro:ro + 64, qb * N:(qb + 1) * N],
                                start=True, stop=True)
                        fd = feeds.get((hp, qb, i))
                        if fd is not None:
                            fd(avps)
                        at = ap_pool.tile([P, 2, N], FP8, tag="at", name="at", bufs=8)
                        bias = {"diag": 0.0, "full": 0.0,
                                "gate2": gate2_sb[:, 0:1],
                                "gate3": gate3_sb[:, 0:1]}[kind]
                        nc.scalar.activation(at[:, 0:2, :], sp[:, 0:2, :],
                                             AF.Exp, bias=bias, scale=0.125)
                        if kind == "diag":
                            nc.vector.tensor_tensor(
                                out=at[:, 0:2, :], in0=at[:, 0:2, :],
                                in1=mask_sb[:, m, :, :], op=OP.mult)
                        for j in range(2):
                            h = 2 * hp + j
                            nc.tensor.matmul(yps[j][:],
                                             lhsT=v_sb[kt][:, h * 65:(h + 1) * 65],
                                             rhs=at[:, j, :],
                                             start=(i == 0), stop=(i == last))
                    for j in range(2):
                        nc.vector.tensor_copy(
                            out=y_fm[hp][j * 64:(j + 1) * 64, qb * N:(qb + 1) * N],
                            in_=yps[j][0:64, :])
                        s1 = ap_pool.tile([1, N], F32, tag="s1", name="s1", bufs=4)
                        nc.vector.tensor_copy(out=s1[:], in_=yps[j][64:65, :])
                        nc.sync.dma_start(
                            out=sums_d[2 * hp + j:2 * hp + j + 1, qb * N:(qb + 1) * N],
                            in_=s1[:])

        es_wv.close()
        es_v.close()
        es_qk.close()

        # ---------------- Phase 5: normalize + proj + residual + LN2 -------
        # softmax-sum normalize is split by query half: half 0 feeds proj
        # tiles 0-3 while half 1 normalizes, hiding the sums roundtrip
        y8 = p_y8.tile([P, CC, TQ], FP8, tag="y8", name="y8")
        x_mid = [p_mid.tile([P, C], BF16, tag=f"xm{i}", name=f"xm{i}")
                 for i in range(TQ // P)]
        xln2T = p_x2.tile([P, CC, TQ], FP8, tag="xln2T", name="xln2T")

        def normalize_qb(qb, rp, rps_pool):
            s16 = rp.tile([16, N], F32, tag="s16", name="s16")
            nc.sync.dma_start(out=s16[:], in_=sums_d[:, qb * N:(qb + 1) * N])
            recip16 = rp.tile([16, N], F32, tag="recip16", name="recip16")
            nc.vector.reciprocal(out=recip16[:], in_=s16[:])
            reciprr = rp.tile([16, N], F32R, tag="reciprr", name="reciprr")
            with nc.allow_low_precision(reason="f32r view of f32 recip"):
                nc.vector.tensor_scalar(reciprr[:], recip16[:], SY, None, OP.mult)
            for yt in range(CC):
                recip_r = rp.tile([2, N], F32R, tag="recipr", name="recipr", bufs=4)
                nc.sync.dma_start(out=recip_r[:], in_=reciprr[2 * yt:2 * yt + 2, :])
                rps = rps_pool.tile([P, N], F32, tag="rps", name="rps")
                nc.tensor.matmul(rps[:], lhsT=sel_sb[:], rhs=recip_r[:],
                                 start=True, stop=True)
                nc.vector.tensor_tensor(out=y8[:, yt, qb * N:(qb + 1) * N],
                                        in0=y_fm[yt][:, qb * N:(qb + 1) * N],
                                        in1=rps[:], op=OP.mult)
                if apply_bias:
                    nc.vector.tensor_scalar(y8[:, yt, qb * N:(qb + 1) * N],
                                            y8[:, yt, qb * N:(qb + 1) * N],
                                            bv_sb[:, yt:yt + 1], None, OP.add)

        with tc.tile_pool(name="pj_sp", bufs=3) as sp, \
             tc.tile_pool(name="pj_cp", bufs=1) as cp, \
             tc.tile_pool(name="pj_st", bufs=6) as st, \
             tc.tile_pool(name="att_rp", bufs=2) as rp, \
             tc.tile_pool(name="att_rps", bufs=2, space="PSUM") as rps_pool, \
             tc.tile_pool(name="pj_ps", bufs=2, space="PSUM") as pps, \
             tc.tile_pool(name="ln2_ps", bufs=2, space="PSUM") as tps:
            if apply_lnwb:
                w2 = cp.tile([P, C], F32, tag="w2", name="w2")
                nc.sync.dma_start(out=w2[:], in_=ln2w[:])
                b2 = cp.tile([P, C], F32, tag="b2", name="b2")
                nc.sync.dma_start(out=b2[:], in_=ln2b[:])
            normalize_qb(0, rp, rps_pool)
            for t8 in range(TQ // P):
                if t8 == 4:
                    normalize_qb(1, rp, rps_pool)
                xo = sp.tile([P, C], F32, tag="xo", name="xo")
                nc.sync.dma_start(out=xo[:], in_=x_seq[t8 * P:(t8 + 1) * P, :])
                ps2 = pps.tile([P, 2, N], F32, tag="pj_ps", name="pj_ps")
                for pr in range(NPR):
                    ly = y8[:, 2 * pr:2 * pr + 2, t8 * P:(t8 + 1) * P]
                    for ft in range(2):
                        nc.tensor.matmul(ps2[:, ft, :], lhsT=ly,
                                         rhs=wpj_sb[:, pr, :, ft * N:(ft + 1) * N],
                                         start=(pr == 0), stop=(pr == NPR - 1),
                                         perf_mode=DR)
                xt = x_mid[t8]
                nc.scalar.activation(xt[:], ps2[:, 0:2, :], AF.Identity,
                                     bias=0.0, scale=1.0 / (SW * SY))
                nc.vector.tensor_tensor(out=xt[:], in0=xt[:], in1=xo[:], op=OP.add)
                if apply_bias:
                    nc.gpsimd.tensor_tensor(out=xt[:], in0=xt[:], in1=bpj_sb[:], op=OP.add)
                # --- LN2 for this token tile ---
                stats = st.tile([P, 2, 6], F32, tag="st2", name="st2")
                for g in range(2):
                    nc.vector.bn_stats(out=stats[:, g, :], in_=xt[:, g * 512:(g + 1) * 512])
                mv = st.tile([P, 2], F32, tag="mv2", name="mv2")
                nc.vector.bn_aggr(out=mv[:], in_=stats[:])
                rstd = st.tile([P, 1], F32, tag="rstd2", name="rstd2")
                nc.scalar.activation(rstd[:], mv[:, 1:2], AF.Sqrt, bias=eps_sb[:], scale=1.0)
                nc.vector.reciprocal(out=rstd[:], in_=rstd[:])
                xb = sp.tile([P, C], BF16, tag="xb2", name="xb2")
                nmr = st.tile([P, 1], F32, tag="nmr2", name="nmr2")
                nc.vector.tensor_scalar(nmr[:], mv[:, 0:1], rstd[:], -1.0,
                                        OP.mult, OP.mult)
                if apply_lnwb:
                    xc = sp.tile([P, C], F32, tag="xc2", name="xc2")
                    nc.scalar.activation(xc[:], xt[:], AF.Identity,
                                         bias=nmr[:], scale=rstd[:])
                    xw = sp.tile([P, C], F32, tag="xw2", name="xw2")
                    nc.vector.tensor_tensor(out=xw[:], in0=xc[:], in1=w2[:], op=OP.mult)
                    nc.vector.tensor_tensor(out=xb[:], in0=xw[:], in1=b2[:], op=OP.add)
                else:
                    nc.scalar.activation(xb[:], xt[:], AF.Identity,
                                         bias=nmr[:], scale=rstd[:])
                pst = tps.tile([P, CC, P], BF16, tag="trp2", name="trp2")
                for cc in range(CC):
                    nc.tensor.transpose(pst[:, cc, :],
                                        xb[:, cc * P:(cc + 1) * P], ident[:])
                nc.vector.tensor_copy(
                    out=xln2T[:, :, t8 * P:(t8 + 1) * P], in_=pst[:])

        es_y.close()

        es_y8.close()
        es_wpj.close()

        # ---------------- Phase 7: FC + gelu (DoubleRow fp8) ---------------
        with tc.tile_pool(name="fc_w", bufs=3) as wp, \
             tc.tile_pool(name="fc_ps", bufs=3, space="PSUM") as fps:
            for hg in range(F // N):
                wt = wp.tile([P, NPR, 2, N], FP8, tag="wfc", name="wfc")
                nc.sync.dma_start(out=wt[:], in_=w_fc3[:, :, :, hg * N:(hg + 1) * N])
                for hs in range(4):
                    hf = hg * 4 + hs
                    ps2 = fps.tile([P, 2, N], F32, tag="fc_ps", name="fc_ps")
                    for pr in range(NPR):
                        lw = wt[:, pr, :, hs * P:(hs + 1) * P]
                        for tt in range(2):
                            nc.tensor.matmul(ps2[:, tt, :], lhsT=lw,
                                             rhs=xln2T[:, 2 * pr:2 * pr + 2, tt * N:(tt + 1) * N],
                                             start=(pr == 0), stop=(pr == NPR - 1),
                                             perf_mode=DR)
                    if not SIM_GELU:
                        nc.scalar.activation(h8[:, hf, :], ps2[:, 0:2, :],
                                             AF.Gelu_apprx_tanh,
                                             bias=bfc_sb[:, hf:hf + 1], scale=1.0 / SW)
                    else:
                        import math
                        cst = math.sqrt(2.0 / math.pi)
                        u = wp.tile([P, 2, N], F32, tag="g_u", name="g_u")
                        nc.scalar.activation(u[:], ps2[:, 0:2, :], AF.Identity,
                                             bias=bfc_sb[:, hf:hf + 1], scale=1.0 / SW)
                        u3 = wp.tile([P, 2, N], F32, tag="g_u3", name="g_u3")
                        nc.scalar.activation(u3[:], u[:], AF.Square, bias=0.0, scale=1.0)
                        nc.vector.tensor_tensor(out=u3[:], in0=u3[:], in1=u[:], op=OP.mult)
                        nc.vector.tensor_scalar(u3[:], u3[:], 0.044715, None, OP.mult)
                        nc.vector.tensor_tensor(out=u3[:], in0=u3[:], in1=u[:], op=OP.add)
                        tqh = wp.tile([P, 2, N], F32, tag="g_t", name="g_t")
                        nc.scalar.activation(tqh[:], u3[:], AF.Tanh, bias=0.0, scale=cst)
                        nc.vector.tensor_scalar(tqh[:], tqh[:], 1.0, None, OP.add)
                        nc.vector.tensor_tensor(out=tqh[:], in0=tqh[:], in1=u[:], op=OP.mult)
                        nc.vector.tensor_scalar(h8[:, hf, :], tqh[:], 0.5, None, OP.mult)

        es_x2.close()

        # ---------------- Phase 8: out matmul + residual (DoubleRow fp8) ---
        with tc.tile_pool(name="ot_w", bufs=6) as wp, \
             tc.tile_pool(name="ot_sp", bufs=3) as sp, \
             tc.tile_pool(name="ot_ps", bufs=8, space="PSUM") as ops_pool:
            for half in range(2):
                opss = [ops_pool.tile([P, N], F32, tag="ot_ps", name="ot_ps")
                        for _ in range(8)]
                for pr in range(F // 256):
                    wt = wp.tile([P, 2, C], FP8, tag="wot", name="wot")
                    nc.sync.dma_start(out=wt[:], in_=w_ot3[:, pr, :, :])
                    for tc4 in range(4):
                        t8 = half * 4 + tc4
                        lh = h8[:, 2 * pr:2 * pr + 2, t8 * P:(t8 + 1) * P]
                        for ft in range(2):
                            nc.tensor.matmul(opss[tc4 * 2 + ft][:], lhsT=lh,
                                             rhs=wt[:, :, ft * N:(ft + 1) * N],
                                             start=(pr == 0), stop=(pr == F // 256 - 1),
                                             perf_mode=DR)
                for tc4 in range(4):
                    t8 = half * 4 + tc4
                    ot = sp.tile([P, C], F32, tag="ot", name="ot")
                    for ft in range(2):
                        nc.scalar.activation(ot[:, ft * N:(ft + 1) * N],
                                             opss[tc4 * 2 + ft][:], AF.Identity,
                                             bias=0.0, scale=1.0 / SWO)
                    nc.vector.tensor_tensor(out=ot[:], in0=ot[:],
                                            in1=x_mid[t8][:], op=OP.add)
                    if apply_bias:
                        nc.vector.tensor_tensor(out=ot[:], in0=ot[:], in1=bot_sb[:], op=OP.add)
                    nc.sync.dma_start(out=out_d[t8 * P:(t8 + 1) * P, :], in_=ot[:])

    nc.finalize()
    return nc


def _own_blocks(s):
    return [0, 1, 2, 3, 12, 13, 14, 15] if s == 0 else list(range(4, 12))


def _prep_shared(inputs):
    f8 = ml_dtypes.float8_e4m3

    def pack_dr(wT, npr, scale):
        # wT: [K, M] (contraction-major); -> [P, npr, 2, M] with
        # [p, pr, hf, m] = scale * wT[pr*256 + hf*128 + p, m]
        K, M = wT.shape
        assert K == npr * 256
        a = (wT * scale).reshape(npr, 2, P, M).transpose(2, 0, 1, 3)
        return np.ascontiguousarray(a).astype(f8)

    W_attn = np.asarray(inputs["W_attn"], np.float32)
    shared = {
        "w_qk3": pack_dr(np.ascontiguousarray(W_attn[:2 * C].T), NPR, SW),
        "w_v3": pack_dr(np.ascontiguousarray(W_attn[2 * C:].T), NPR, SW),
        "w_pj3": pack_dr(np.ascontiguousarray(np.asarray(inputs["W_proj"], np.float32).T), NPR, SW),
        "w_fc3": pack_dr(np.ascontiguousarray(np.asarray(inputs["W_fc"], np.float32).T), NPR, SW),
        "w_ot3": pack_dr(np.ascontiguousarray(np.asarray(inputs["W_out"], np.float32).T), F // 256, SWO),
        "ln1w": np.ascontiguousarray(np.broadcast_to(np.asarray(inputs["ln1_w"], np.float32), (P, C))),
        "ln1b": np.ascontiguousarray(np.broadcast_to(np.asarray(inputs["ln1_b"], np.float32), (P, C))),
        "ln2w": np.ascontiguousarray(np.broadcast_to(np.asarray(inputs["ln2_w"], np.float32), (P, C))),
        "ln2b": np.ascontiguousarray(np.broadcast_to(np.asarray(inputs["ln2_b"], np.float32), (P, C))),
        "b_q": np.ascontiguousarray(np.asarray(inputs["b_attn"], np.float32)[:C].reshape(CC, P).T) * SW,
        "b_k": np.ascontiguousarray(np.asarray(inputs["b_attn"], np.float32)[C:2 * C].reshape(CC, P).T),
        "b_v": np.ascontiguousarray(np.asarray(inputs["b_attn"], np.float32)[2 * C:].reshape(CC, P).T) * SY,
        "b_pj": np.ascontiguousarray(np.broadcast_to(np.asarray(inputs["b_proj"], np.float32), (P, C))),
        "b_fc": np.ascontiguousarray(np.asarray(inputs["b_fc"], np.float32).reshape(F // P, P).T),
        "b_ot": np.ascontiguousarray(np.broadcast_to(np.asarray(inputs["b_out"], np.float32), (P, C))),
    }
    # mask4[p, m*N + qf] = 1 if qf >= m*128 + p else 0
    pp = np.arange(P)[:, None]
    qf = np.arange(N)[None, :]
    mask = np.zeros((P, 4, 2, N), np.float32)
    for m in range(4):
        mask[:, m, 0, :] = (qf >= m * P + pp)
        mask[:, m, 1, :] = mask[:, m, 0, :]
    shared["mask4"] = mask.astype(ml_dtypes.bfloat16)
    sel = np.zeros((2, P), np.float32)
    sel[0, :64] = 1.0
    sel[1, 64:] = 1.0
    shared["sel2"] = sel
    return shared


def _make_in_maps(inputs):
    x = np.asarray(inputs["x"], np.float32)
    shared = _prep_shared(inputs)
    in_maps = []
    for c in range(8):
        b, s = c // 2, c % 2
        own = _own_blocks(s)
        other = _own_blocks(1 - s)
        xb = x[b].reshape(16, P, C)
        m = dict(shared)
        m["x_seq"] = np.ascontiguousarray(
            np.concatenate([xb[own], xb[other]], axis=0).reshape(T2, C))
        m["gate2"] = np.full((P, 1), 0.0 if s == 1 else -1e30, np.float32)
        m["gate3"] = np.full((P, 1), 0.0 if s == 0 else -1e30, np.float32)
        in_maps.append(m)
    return in_maps


def _get_nc(apply_lnwb=True, apply_bias=True):
    key = ("nc", apply_lnwb, apply_bias, SIM_GELU)
    if key not in _CACHE:
        _CACHE[key] = _build_nc(apply_lnwb, apply_bias)
    return _CACHE[key]


def run_cores(inputs, profile=False):
    """Run the SPMD program; returns list of per-core result dicts."""
    global last_exec_time_ns
    apply_lnwb = not (
        np.allclose(np.asarray(inputs["ln1_w"]), 1.0)
        and np.allclose(np.asarray(inputs["ln1_b"]), 0.0)
        and np.allclose(np.asarray(inputs["ln2_w"]), 1.0)
        and np.allclose(np.asarray(inputs["ln2_b"]), 0.0))
    apply_bias = not (
        np.allclose(np.asarray(inputs["b_attn"]), 0.0)
        and np.allclose(np.asarray(inputs["b_proj"]), 0.0)
        and np.allclose(np.asarray(inputs["b_out"]), 0.0))
    nc = _get_nc(apply_lnwb, apply_bias)
    in_maps = _make_in_maps(inputs)
    if profile:
        import concourse.bass_utils as bass_utils
        bass_utils.upload_artifacts = lambda tmpdir: "local://" + tmpdir
        try:
            from trn_agent_boot.trn_boot import _ntff_profile_via_ctypes
            import antenv.axon_hooks as hooks
            if hooks.get_axon_ntff_profile_hook() is None:
                hooks.set_axon_ntff_profile_hook(
                    _ntff_profile_via_ctypes("/opt/axon/libaxon_pjrt.so"))
        except Exception:
            pass
        res = bass_utils.run_bass_kernel_spmd(nc, in_maps, list(range(8)), trace=True)
        last_exec_time_ns = res.exec_time_ns
        return res.results
    return _cached_runner(nc)(in_maps)


def _cached_runner(nc):
    """Per-process cached jit of the SPMD executable so repeated kernel()
    calls don't recompile (mirrors bass2jax.run_bass_via_pjrt's multi-core
    branch)."""
    key = ("runner", id(nc))
    if key in _CACHE:
        return _CACHE[key]
    import jax
    import numpy as _np
    from jax.sharding import Mesh, PartitionSpec
    from jax.experimental.shard_map import shard_map
    from concourse import bass2jax, mybir as _mybir
    bass2jax.install_neuronx_cc_hook()

    part_name = nc.partition_id_tensor.name if nc.partition_id_tensor else None
    in_names, out_names, out_avals, zero_outs = [], [], [], []
    for alloc in nc.m.functions[0].allocations:
        if not isinstance(alloc, _mybir.MemoryLocationSet):
            continue
        name = alloc.memorylocations[0].name
        if alloc.kind == "ExternalInput":
            if name != part_name:
                in_names.append(name)
        elif alloc.kind == "ExternalOutput":
            out_names.append(name)
            shape = tuple(alloc.tensor_shape)
            dtype = _mybir.dt.np(alloc.dtype)
            out_avals.append(jax.core.ShapedArray(shape, dtype))
            zero_outs.append(_np.zeros(shape, dtype))
    n_params = len(in_names)
    all_names = in_names + out_names
    if part_name is not None:
        all_names = all_names + [part_name]
    donate = tuple(range(n_params, n_params + len(out_names)))
    if jax.default_backend() == "cpu":
        donate = ()  # cpu sim path can't alias donated outputs

    def _body(*args):
        operands = list(args)
        if part_name is not None:
            operands.append(bass2jax.partition_id_tensor())
        outs = bass2jax._bass_exec_p.bind(
            *operands, out_avals=tuple(out_avals), in_names=tuple(all_names),
            out_names=tuple(out_names), lowering_input_output_aliases=(),
            sim_require_finite=True, sim_require_nnan=True, nc=nc)
        return tuple(outs)

    devices = jax.devices()[:8]
    mesh = Mesh(_np.asarray(devices), ("core",))
    spec = (PartitionSpec("core"),) * (n_params + len(out_names))
    sharded = jax.jit(
        shard_map(_body, mesh=mesh, in_specs=spec,
                  out_specs=(PartitionSpec("core"),) * len(out_names),
                  check_rep=False),
        donate_argnums=donate, keep_unused=True)

    def run(in_maps):
        concat_in = [
            _np.concatenate([_np.asarray(in_maps[c][nm]) for c in range(8)], axis=0)
            for nm in in_names]
        concat_zero = [_np.zeros((8 * z.shape[0], *z.shape[1:]), z.dtype)
                       for z in zero_outs]
        out_arrs = sharded(*concat_in, *concat_zero)
        return [
            {nm: _np.asarray(out_arrs[i]).reshape(8, *out_avals[i].shape)[c]
             for i, nm in enumerate(out_names)}
            for c in range(8)]

    _CACHE[key] = run
    return run


def kernel(**inputs) -> np.ndarray:
    results = run_cores(inputs, profile=PROFILE)
    out = np.empty((B, T, C), np.float32)
    for c in range(8):
        b, s = c // 2, c % 2
        res = results[c]["out"]
        for j, blk in enumerate(_own_blocks(s)):
            out[b, blk * P:(blk + 1) * P, :] = res[j * P:(j + 1) * P]
    return out


# revision 57
# speedup vs baseline: 1.3287x; 1.0187x over previous
"""Trainium2 Bass kernel for a GPT-2 style transformer block.

Problem: B=4, T=2048, C=1024, H=16 heads (hd=64), MLP hidden 4096, fp32 I/O.

Sharding: zero-collective 8-way data parallel. Core c handles batch b=c//2;
s=c%2 selects its query set: s=0 owns the OUTER sequence quarters (blocks
0-3 and 12-15 of 128 tokens), s=1 the MIDDLE half (blocks 4-11). This makes
the causal-attention work symmetric across the pair: a uniform 24-tile
slot schedule per head covers both cores' needs, with per-core host-side
mask / gate tables providing the divergence. K/V are computed locally for
all 2048 tokens in own-first order.

Precision: all big weight matmuls (QKV, V, proj, FC, out) run in fp8e4
DoubleRow perf mode (2 contraction rows per PE cell per cycle): weights are
host-prescaled by 64 (W_out by 256) so N(0, 0.02)-scale values land in
e4m3's normal range; the scale is removed in the PSUM evacuation ops.
Attention q/k/v/exp-weights are fp8e4 as well (no DoubleRow; contraction is
only 64/128 deep), scores accumulate in f32 PSUM and softmax runs in f32 on
ScalarE. LayerNorm is f32 (bn_stats), residuals bf16/f32.

Layouts:
  x / residuals / final out: token-major [tok(P), C]
  x_ln transposed to feature-major [feat(P), chunk, tok] fp8 via PE
  transposes (bf16) + DVE convert-copy
  Q [feat(P), TQ], K [feat(P), T2] fp8; head pair hp lives in one tile
  (rows 0-63 head 2hp, 64-127 head 2hp+1) so score matmuls of a pair are
  emitted adjacently and run CONCURRENTLY in distinct PE row-groups
  V token-major [tok(P), h*65] fp8 with a built-in ones column per head
  (softmax row sums ride the AV matmul); softmax needs no max-subtraction
  (scores bounded ~|s|<4) and no transposes anywhere in attention
  normalization + v-bias deferred to after AV via a tiny K=2 f32r selector
  matmul that partition-broadcasts 16/sums
"""

import os
import sys
import types

import numpy as np
import ml_dtypes

for _p in ("/opt/trn_rl_repo", "/root/.axon_site/_ro/trn_rl_repo"):
    if os.path.isdir(_p) and _p not in sys.path:
        sys.path.append(_p)

# antenv.axon_hooks is absent in this image; bass_utils imports it when
# tracing under axon. Provide the trivial get/set holder it expects.
if "antenv.axon_hooks" not in sys.modules:
    try:
        import antenv

        _m = types.ModuleType("antenv.axon_hooks")
        _m._hook = None

        def _set_hook(h):
            _m._hook = h

        def _get_hook():
            return _m._hook

        _m.set_axon_ntff_profile_hook = _set_hook
        _m.get_axon_ntff_profile_hook = _get_hook
        sys.modules["antenv.axon_hooks"] = _m
        antenv.axon_hooks = _m
    except ImportError:
        pass

import concourse.bacc as bacc
import concourse.tile as tile
from concourse import mybir
from concourse.masks import make_identity

P = 128
B, T, C = 4, 2048, 1024
H, HD = 16, 64
F = 4096
T2 = T  # tokens per core for K/V (full sequence of one batch element)
TQ = T // 2  # own query tokens per core
CC = C // P  # 8 C-chunks
NPR = CC // 2  # 4 DoubleRow contraction pair-chunks (256 each)
N = 512  # moving free dim per matmul

SW = 64.0  # fp8 weight prescale (qkv/v/proj/fc)
SWO = 256.0  # fp8 weight prescale for W_out
SY = 16.0  # attention-output prescale into fp8

F32 = mybir.dt.float32
F32R = mybir.dt.float32r
BF16 = mybir.dt.bfloat16
FP8 = mybir.dt.float8e4
AF = mybir.ActivationFunctionType
OP = mybir.AluOpType
DR = mybir.MatmulPerfMode.DoubleRow

PROFILE = False
SIM_GELU = False  # CoreSim lacks the Gelu LUT; emulate with Tanh + DVE ops
last_exec_time_ns = None

_CACHE = {}

# per-(g,kt) attention slot schedule, uniform across cores.
# kinds: 'diag' (mask m), 'full', 'gate2' (live iff s==1), 'gate3' (iff s==0)
SLOTS0 = [(8, "gate2", 0), (9, "gate2", 0), (10, "gate2", 0), (11, "gate2", 0),
          (0, "diag", 0), (1, "diag", 1), (2, "diag", 2), (3, "diag", 3)]
SLOTS1 = [(0, "full", 0), (1, "full", 0), (2, "full", 0), (3, "full", 0),
          (4, "diag", 0), (5, "diag", 1), (6, "diag", 2), (7, "diag", 3),
          (8, "full", 0), (9, "full", 0), (10, "full", 0), (11, "full", 0),
          (12, "gate3", 0), (13, "gate3", 0), (14, "gate3", 0), (15, "gate3", 0)]
SLOTS = (SLOTS0, SLOTS1)


def _build_nc(apply_lnwb: bool = True, apply_bias: bool = True):
    nc = bacc.Bacc("TRN2", target_bir_lowering=False, debug=False, num_devices=8)

    x_seq = nc.dram_tensor("x_seq", [T2, C], F32, kind="ExternalInput")
    w_qk3 = nc.dram_tensor("w_qk3", [P, NPR, 2, 2 * C], FP8, kind="ExternalInput")
    w_v3 = nc.dram_tensor("w_v3", [P, NPR, 2, C], FP8, kind="ExternalInput")
    w_pj3 = nc.dram_tensor("w_pj3", [P, NPR, 2, C], FP8, kind="ExternalInput")
    w_fc3 = nc.dram_tensor("w_fc3", [P, NPR, 2, F], FP8, kind="ExternalInput")
    w_ot3 = nc.dram_tensor("w_ot3", [P, F // 256, 2, C], FP8, kind="ExternalInput")
    ln1w = nc.dram_tensor("ln1w", [P, C], F32, kind="ExternalInput")
    ln1b = nc.dram_tensor("ln1b", [P, C], F32, kind="ExternalInput")
    ln2w = nc.dram_tensor("ln2w", [P, C], F32, kind="ExternalInput")
    ln2b = nc.dram_tensor("ln2b", [P, C], F32, kind="ExternalInput")
    b_q = nc.dram_tensor("b_q", [P, CC], F32, kind="ExternalInput")  # x64
    b_k = nc.dram_tensor("b_k", [P, CC], F32, kind="ExternalInput")  # x64
    b_v = nc.dram_tensor("b_v", [P, CC], F32, kind="ExternalInput")  # x16, col per chunk
    b_pj = nc.dram_tensor("b_pj", [P, C], F32, kind="ExternalInput")
    b_fc = nc.dram_tensor("b_fc", [P, F // P], F32, kind="ExternalInput")
    b_ot = nc.dram_tensor("b_ot", [P, C], F32, kind="ExternalInput")
    mask4 = nc.dram_tensor("mask4", [P, 4, 2, N], BF16, kind="ExternalInput")
    gate2 = nc.dram_tensor("gate2", [P, 1], F32, kind="ExternalInput")
    gate3 = nc.dram_tensor("gate3", [P, 1], F32, kind="ExternalInput")
    sel2 = nc.dram_tensor("sel2", [2, P], F32R, kind="ExternalInput")

    out_d = nc.dram_tensor("out", [TQ, C], F32, kind="ExternalOutput")
    sums_d = nc.dram_tensor("sums_scratch", [16, TQ], F32)

    from contextlib import ExitStack

    with tile.TileContext(nc) as tc, ExitStack() as ctx:
        # pool enter order = reverse of close order (pool stack is LIFO);
        # SBUF is reserved from first tile creation to pool close
        const = ctx.enter_context(tc.tile_pool(name="const", bufs=1))
        p_big = ctx.enter_context(tc.tile_pool(name="p_big", bufs=1))
        es_mid = ctx.enter_context(ExitStack())
        es_x2 = ctx.enter_context(ExitStack())
        es_wpj = ctx.enter_context(ExitStack())
        es_y8 = ctx.enter_context(ExitStack())
        es_y = ctx.enter_context(ExitStack())
        es_qk = ctx.enter_context(ExitStack())
        es_v = ctx.enter_context(ExitStack())
        es_wv = ctx.enter_context(ExitStack())

        ident = const.tile([P, P], BF16, tag="ident", name="ident")
        make_identity(nc, ident)
        eps_sb = const.tile([P, 1], F32, tag="eps", name="eps")
        nc.vector.memset(eps_sb[:], 1e-5)
        mask_sb = const.tile([P, 4, 2, N], BF16, tag="mask", name="mask")
        nc.sync.dma_start(out=mask_sb[:], in_=mask4[:])
        gate2_sb = const.tile([P, 1], F32, tag="g2", name="g2")
        nc.sync.dma_start(out=gate2_sb[:], in_=gate2[:])
        gate3_sb = const.tile([P, 1], F32, tag="g3", name="g3")
        nc.sync.dma_start(out=gate3_sb[:], in_=gate3[:])
        sel_sb = const.tile([2, P], F32R, tag="sel", name="sel")
        nc.sync.dma_start(out=sel_sb[:], in_=sel2[:])
        bq_sb = const.tile([P, CC], F32, tag="bq", name="bq")
        nc.sync.dma_start(out=bq_sb[:], in_=b_q[:])
        bk_sb = const.tile([P, CC], F32, tag="bk", name="bk")
        nc.sync.dma_start(out=bk_sb[:], in_=b_k[:])
        bk64_sb = const.tile([P, CC], F32, tag="bk64", name="bk64")
        nc.vector.tensor_scalar(bk64_sb[:], bk_sb[:], SW, None, OP.mult)
        bv_sb = const.tile([P, CC], F32, tag="bv", name="bv")
        nc.sync.dma_start(out=bv_sb[:], in_=b_v[:])
        bfc_sb = const.tile([P, F // P], F32, tag="bfc", name="bfc")
        nc.sync.dma_start(out=bfc_sb[:], in_=b_fc[:])
        bpj_sb = const.tile([P, C], F32, tag="bpj", name="bpj")
        nc.sync.dma_start(out=bpj_sb[:], in_=b_pj[:])
        bot_sb = const.tile([P, C], F32, tag="bot", name="bot")
        nc.sync.dma_start(out=bot_sb[:], in_=b_ot[:])

        p_mid = es_mid.enter_context(tc.tile_pool(name="p_mid", bufs=1))
        p_x2 = es_x2.enter_context(tc.tile_pool(name="p_x2", bufs=1))
        p_wpj = es_wpj.enter_context(tc.tile_pool(name="p_wpj", bufs=1))
        p_y8 = es_y8.enter_context(tc.tile_pool(name="p_y8", bufs=1))
        p_y = es_y.enter_context(tc.tile_pool(name="p_y", bufs=1))
        p_qk = es_qk.enter_context(tc.tile_pool(name="p_qk", bufs=1))
        p_v = es_v.enter_context(tc.tile_pool(name="p_v", bufs=1))
        p_wv = es_wv.enter_context(tc.tile_pool(name="p_wv", bufs=1))

        # one 32 KiB/partition fp8 buffer triple-aliased across disjoint
        # lifetimes: [xlnT | wqk] (phases 1-3) then h8 (phases 7-8)
        buf32 = p_big.tile([P, 2 * CC * T2], FP8, tag="buf32", name="buf32")
        xlnT = buf32.rearrange("p (a c t) -> p a c t", a=2, t=T2)[:, 0]
        wqk_sb = buf32.rearrange("p (a pr hf f) -> p a pr hf f",
                                 a=2, pr=NPR, hf=2)[:, 1]
        h8 = buf32.rearrange("p (f t) -> p f t", t=TQ)
        wv_sb = p_wv.tile([P, NPR, 2, C], FP8, tag="wv", name="wv")
        wpj_sb = p_wpj.tile([P, NPR, 2, C], FP8, tag="wpj", name="wpj")

        # ---------------- Phase 1: LN1 + transpose + K (fused) --------------
        # K matmuls for token-block pairs are emitted as soon as their
        # transposes land, filling the PE during the DVE/ACT-bound LN loop.
        # Weight-stationary over 2 blocks so each 256-col DoubleRow weight
        # load amortizes over 2 matmuls; evacuations ((psum+64b)/64 -> bf16)
        # run on ScalarE (idle here) via the free affine: ps/64 + b_true.
        q_sb = [p_qk.tile([P, TQ], BF16, tag=f"q{i}", name=f"q{i}") for i in range(CC)]
        k_sb = [p_qk.tile([P, T2], BF16, tag=f"k{i}", name=f"k{i}") for i in range(CC)]

        def emit_k(blk, kps):
            for fc in range(CC):
                psK = kps.tile([P, N], F32, tag="k_ps", name="k_ps")
                for pr in range(NPR):
                    lk = wqk_sb[:, pr, :, C + fc * P:C + (fc + 1) * P]
                    nc.tensor.matmul(psK[:], lhsT=lk,
                                     rhs=xlnT[:, 2 * pr:2 * pr + 2, blk * N:(blk + 1) * N],
                                     start=(pr == 0), stop=(pr == NPR - 1),
                                     perf_mode=DR)
                nc.scalar.activation(k_sb[fc][:, blk * N:(blk + 1) * N],
                                     psK[:], AF.Identity,
                                     bias=bk_sb[:, fc:fc + 1], scale=1.0 / SW)

        with tc.tile_pool(name="qk_ps", bufs=4, space="PSUM") as kps:
            with tc.tile_pool(name="ln1_sp", bufs=3) as sp, \
                 tc.tile_pool(name="ln1_cp", bufs=1) as cp, \
                 tc.tile_pool(name="ln1_st", bufs=6) as st, \
                 tc.tile_pool(name="ln1_ps", bufs=3, space="PSUM") as tps:
                if apply_lnwb:
                    w1 = cp.tile([P, C], F32, tag="w1", name="w1")
                    nc.sync.dma_start(out=w1[:], in_=ln1w[:])
                    b1 = cp.tile([P, C], F32, tag="b1", name="b1")
                    nc.sync.dma_start(out=b1[:], in_=ln1b[:])
                for tt in range(T2 // P):
                    xt = sp.tile([P, C], F32, tag="xs", name="xs")
                    nc.sync.dma_start(out=xt[:], in_=x_seq[tt * P:(tt + 1) * P, :])
                    if tt == 2:
                        nc.sync.dma_start(out=wqk_sb[:], in_=w_qk3[:])
                        nc.sync.dma_start(out=wv_sb[:], in_=w_v3[:])
                        nc.sync.dma_start(out=wpj_sb[:], in_=w_pj3[:])
                    stats = st.tile([P, 2, 6], F32, tag="st", name="st")
                    for g in range(2):
                        nc.vector.bn_stats(out=stats[:, g, :], in_=xt[:, g * 512:(g + 1) * 512])
                    mv = st.tile([P, 2], F32, tag="mv", name="mv")
                    nc.vector.bn_aggr(out=mv[:], in_=stats[:])
                    rstd = st.tile([P, 1], F32, tag="rstd", name="rstd")
                    nc.scalar.activation(rstd[:], mv[:, 1:2], AF.Sqrt, bias=eps_sb[:], scale=1.0)
                    nc.vector.reciprocal(out=rstd[:], in_=rstd[:])
                    xb = sp.tile([P, C], BF16, tag="xb", name="xb")
                    nmr = st.tile([P, 1], F32, tag="nmr", name="nmr")
                    nc.vector.tensor_scalar(nmr[:], mv[:, 0:1], rstd[:], -1.0,
                                            OP.mult, OP.mult)
                    if apply_lnwb:
                        xc = sp.tile([P, C], F32, tag="xc", name="xc")
                        nc.scalar.activation(xc[:], xt[:], AF.Identity,
                                             bias=nmr[:], scale=rstd[:])
                        xw = sp.tile([P, C], F32, tag="xw", name="xw")
                        nc.vector.tensor_tensor(out=xw[:], in0=xc[:], in1=w1[:], op=OP.mult)
                        nc.vector.tensor_tensor(out=xb[:], in0=xw[:], in1=b1[:], op=OP.add)
                    else:
                        nc.scalar.activation(xb[:], xt[:], AF.Identity,
                                             bias=nmr[:], scale=rstd[:])
                    pst = tps.tile([P, CC, P], BF16, tag="trp", name="trp")
                    for cc in range(CC):
                        nc.tensor.transpose(pst[:, cc, :],
                                            xb[:, cc * P:(cc + 1) * P], ident[:])
                    nc.vector.tensor_copy(
                        out=xlnT[:, :, tt * P:(tt + 1) * P], in_=pst[:])
                    if tt % 4 == 3 and tt < 15:
                        emit_k(tt // 4, kps)

        # ---------------- Phase 2: Q projection, fc=0 only ------------------
        # remaining Q chunks and V tiles are fed into attention idle PE slots
        def emit_q(fc, qps):
            psQ = [qps.tile([P, N], F32, tag="v_ps", name="v_ps") for _ in range(2)]
            for pr in range(NPR):
                lq = wqk_sb[:, pr, :, fc * P:(fc + 1) * P]
                for blk in range(2):
                    nc.tensor.matmul(psQ[blk][:], lhsT=lq,
                                     rhs=xlnT[:, 2 * pr:2 * pr + 2, blk * N:(blk + 1) * N],
                                     start=(pr == 0), stop=(pr == NPR - 1),
                                     perf_mode=DR)
            for blk in range(2):
                nc.vector.tensor_scalar(q_sb[fc][:, blk * N:(blk + 1) * N],
                                        psQ[blk][:], bq_sb[:, fc:fc + 1],
                                        1.0 / SW, OP.add, OP.mult)

        def emit_k_dve(fc2, vps):
            for fc in (2 * fc2, 2 * fc2 + 1):
                psK = vps.tile([P, N], F32, tag="v_ps", name="v_ps")
                for pr in range(NPR):
                    lk = wqk_sb[:, pr, :, C + fc * P:C + (fc + 1) * P]
                    nc.tensor.matmul(psK[:], lhsT=lk,
                                     rhs=xlnT[:, 2 * pr:2 * pr + 2, 3 * N:4 * N],
                                     start=(pr == 0), stop=(pr == NPR - 1),
                                     perf_mode=DR)
                nc.vector.tensor_scalar(k_sb[fc][:, 3 * N:4 * N], psK[:],
                                        bk64_sb[:, fc:fc + 1], 1.0 / SW,
                                        OP.add, OP.mult)

        def emit_v_dve(kt, vps):
            ps = [vps.tile([P, N], F32, tag="v_ps", name="v_ps") for _ in range(2)]
            for pr in range(NPR):
                lv = xlnT[:, 2 * pr:2 * pr + 2, kt * P:(kt + 1) * P]
                for vg in range(2):
                    nc.tensor.matmul(ps[vg][:], lhsT=lv,
                                     rhs=wv_sb[:, pr, :, vg * N:(vg + 1) * N],
                                     start=(pr == 0), stop=(pr == NPR - 1),
                                     perf_mode=DR)
            for vg in range(2):
                out_ap = v_sb[kt].rearrange("p (h d) -> p h d", d=65)[
                    :, vg * 8:(vg + 1) * 8, 0:64]
                in_ap = ps[vg].rearrange("p (h d) -> p h d", d=64)[:, :, :]
                nc.vector.tensor_scalar(out_ap, in_ap, 1.0 / SW, None, OP.mult)

        with tc.tile_pool(name="q_ps", bufs=4, space="PSUM") as qps:
            emit_q(0, qps)

        # ---------------- Phase 2b: V projection (DoubleRow fp8) ------------
        v_sb = [p_v.tile([P, H * 65], FP8, tag=f"v{i}", name=f"v{i}")
                for i in range(T2 // P)]
        for kt in range(T2 // P):
            nc.gpsimd.memset(
                v_sb[kt].rearrange("p (h d) -> p h d", d=65)[:, :, 64:65], 1.0)
        def emit_v(kt, vps):
            ps = [vps.tile([P, N], F32, tag="v_ps", name="v_ps") for _ in range(2)]
            for pr in range(NPR):
                lv = xlnT[:, 2 * pr:2 * pr + 2, kt * P:(kt + 1) * P]
                for vg in range(2):
                    nc.tensor.matmul(ps[vg][:], lhsT=lv,
                                     rhs=wv_sb[:, pr, :, vg * N:(vg + 1) * N],
                                     start=(pr == 0), stop=(pr == NPR - 1),
                                     perf_mode=DR)
            for vg in range(2):
                out_ap = v_sb[kt].rearrange("p (h d) -> p h d", d=65)[
                    :, vg * 8:(vg + 1) * 8, 0:64]
                in_ap = ps[vg].rearrange("p (h d) -> p h d", d=64)[:, :, :]
                nc.scalar.activation(out_ap, in_ap, AF.Identity,
                                     bias=0.0, scale=1.0 / SW)

        with tc.tile_pool(name="v_ps", bufs=4, space="PSUM") as vps:
            for kt in (8, 9, 10, 11):
                emit_v(kt, vps)

        # ---------------- Phase 3: attention -------------------------------
        # Per head-pair: 24 causal slots/qb-group; score matmul pair runs
        # concurrently in PE row-groups 0-63/64-127. Each qb group leads with
        # its maskless slots so the previous group's DVE evacuation tail
        # drains before the first causal-mask multiply is needed.
        y_fm = [p_y.tile([P, TQ], BF16, tag=f"y{i}", name=f"y{i}") for i in range(CC)]
        feeds = {}
        for sl, kt in zip(((0, 0, 0), (0, 0, 1), (0, 0, 2), (0, 0, 3),
                           (0, 1, 0), (0, 1, 2), (0, 1, 4), (0, 1, 6),
                           (0, 1, 8), (0, 1, 10), (0, 1, 12), (0, 1, 14)),
                          (0, 1, 2, 3, 4, 5, 6, 7, 12, 13, 14, 15)):
            feeds[sl] = (lambda kt: lambda pool: emit_v_dve(kt, pool))(kt)
        for sl, fc2 in zip(((0, 0, 4), (0, 0, 6), (0, 1, 1), (0, 1, 3)), range(4)):
            feeds[sl] = (lambda f: lambda pool: emit_k_dve(f, pool))(fc2)
        feeds[(0, 1, 9)] = lambda pool: emit_q(1, pool)
        for h in range(1, 7):
            feeds[(h, 0, 1)] = (lambda fc: lambda pool: emit_q(fc, pool))(h + 1)
        with tc.tile_pool(name="att_at", bufs=1) as ap_pool, \
             tc.tile_pool(name="att_sps", bufs=2, space="PSUM") as sps_pool, \
             tc.tile_pool(name="att_yps", bufs=2, space="PSUM") as yps_pool, \
             tc.tile_pool(name="att_vps", bufs=2, space="PSUM") as avps:
            for hp in range(H // 2):
                for qb in (0, 1):
                    slots = SLOTS[qb]
                    yps = [yps_pool.tile([65, N], F32, tag="yps", name="yps")
                           for _ in range(2)]
                    last = len(slots) - 1
                    for i, (kt, kind, m) in enumerate(slots):
                        sp = sps_pool.tile([P, 2, N], F32, tag="sps", name="sps")
                        for j in range(2):
                            ro = j * 64
                            nc.tensor.matmul(
                                sp[:, j, :],
                                lhsT=k_sb[hp][ro:ro + 64, kt * P:(kt + 1) * P],
                                rhs=q_sb[hp][ro:ro + 64, qb * N:(qb + 1) * N],
                                start=True, stop=True)
                        fd = feeds.get((hp, qb, i))
                        if fd is not None:
                            fd(avps)
                        at = ap_pool.tile([P, 2, N], FP8, tag="at", name="at", bufs=10)
                        bias = {"diag": 0.0, "full": 0.0,
                                "gate2": gate2_sb[:, 0:1],
                                "gate3": gate3_sb[:, 0:1]}[kind]
                        nc.scalar.activation(at[:, 0:2, :], sp[:, 0:2, :],
                                             AF.Exp, bias=bias, scale=0.125)
                        if kind == "diag":
                            nc.vector.tensor_tensor(
                                out=at[:, 0:2, :], in0=at[:, 0:2, :],
                                in1=mask_sb[:, m, :, :], op=OP.mult)
                        for j in range(2):
                            h = 2 * hp + j
                            nc.tensor.matmul(yps[j][:],
                                             lhsT=v_sb[kt][:, h * 65:(h + 1) * 65],
                                             rhs=at[:, j, :],
                                             start=(i == 0), stop=(i == last))
                    for j in range(2):
                        nc.vector.tensor_copy(
                            out=y_fm[hp][j * 64:(j + 1) * 64, qb * N:(qb + 1) * N],
                            in_=yps[j][0:64, :])
                        s1 = ap_pool.tile([1, N], F32, tag="s1", name="s1", bufs=4)
                        nc.vector.tensor_copy(out=s1[:], in_=yps[j][64:65, :])
                        nc.sync.dma_start(
                            out=sums_d[2 * hp + j:2 * hp + j + 1, qb * N:(qb + 1) * N],
                            in_=s1[:])

        es_wv.close()
        es_v.close()
        es_qk.close()

        # ---------------- Phase 5: normalize + proj + residual + LN2 -------
        # softmax-sum normalize is split by query half: half 0 feeds proj
        # tiles 0-3 while half 1 normalizes, hiding the sums roundtrip
        y8 = p_y8.tile([P, CC, TQ], FP8, tag="y8", name="y8")
        x_mid = [p_mid.tile([P, C], BF16, tag=f"xm{i}", name=f"xm{i}")
                 for i in range(TQ // P)]
        xln2T = p_x2.tile([P, CC, TQ], FP8, tag="xln2T", name="xln2T")

        def normalize_qb(qb, rp, rps_pool):
            s16 = rp.tile([16, N], F32, tag="s16", name="s16")
            nc.sync.dma_start(out=s16[:], in_=sums_d[:, qb * N:(qb + 1) * N])
            recip16 = rp.tile([16, N], F32, tag="recip16", name="recip16")
            nc.vector.reciprocal(out=recip16[:], in_=s16[:])
            reciprr = rp.tile([16, N], F32R, tag="reciprr", name="reciprr")
            with nc.allow_low_precision(reason="f32r view of f32 recip"):
                nc.vector.tensor_scalar(reciprr[:], recip16[:], SY, None, OP.mult)
            for yt in range(CC):
                recip_r = rp.tile([2, N], F32R, tag="recipr", name="recipr", bufs=4)
                nc.sync.dma_start(out=recip_r[:], in_=reciprr[2 * yt:2 * yt + 2, :])
                rps = rps_pool.tile([P, N], F32, tag="rps", name="rps")
                nc.tensor.matmul(rps[:], lhsT=sel_sb[:], rhs=recip_r[:],
                                 start=True, stop=True)
                nc.vector.tensor_tensor(out=y8[:, yt, qb * N:(qb + 1) * N],
                                        in0=y_fm[yt][:, qb * N:(qb + 1) * N],
                                        in1=rps[:], op=OP.mult)
                if apply_bias:
                    nc.vector.tensor_scalar(y8[:, yt, qb * N:(qb + 1) * N],
                                            y8[:, yt, qb * N:(qb + 1) * N],
                                            bv_sb[:, yt:yt + 1], None, OP.add)

        with tc.tile_pool(name="pj_sp", bufs=3) as sp, \
             tc.tile_pool(name="pj_cp", bufs=1) as cp, \
             tc.tile_pool(name="pj_st", bufs=6) as st, \
             tc.tile_pool(name="att_rp", bufs=2) as rp, \
             tc.tile_pool(name="att_rps", bufs=2, space="PSUM") as rps_pool, \
             tc.tile_pool(name="pj_ps", bufs=2, space="PSUM") as pps, \
             tc.tile_pool(name="ln2_ps", bufs=2, space="PSUM") as tps:
            if apply_lnwb:
                w2 = cp.tile([P, C], F32, tag="w2", name="w2")
                nc.sync.dma_start(out=w2[:], in_=ln2w[:])
                b2 = cp.tile([P, C], F32, tag="b2", name="b2")
                nc.sync.dma_start(out=b2[:], in_=ln2b[:])
            normalize_qb(0, rp, rps_pool)
            for t8 in range(TQ // P):
                if t8 == 4:
                    normalize_qb(1, rp, rps_pool)
                xo = sp.tile([P, C], F32, tag="xo", name="xo")
                nc.sync.dma_start(out=xo[:], in_=x_seq[t8 * P:(t8 + 1) * P, :])
                ps2 = pps.tile([P, 2, N], F32, tag="pj_ps", name="pj_ps")
                for pr in range(NPR):
                    ly = y8[:, 2 * pr:2 * pr + 2, t8 * P:(t8 + 1) * P]
                    for ft in range(2):
                        nc.tensor.matmul(ps2[:, ft, :], lhsT=ly,
                                         rhs=wpj_sb[:, pr, :, ft * N:(ft + 1) * N],
                                         start=(pr == 0), stop=(pr == NPR - 1),
                                         perf_mode=DR)
                xt = x_mid[t8]
                nc.scalar.activation(xt[:], ps2[:, 0:2, :], AF.Identity,
                                     bias=0.0, scale=1.0 / (SW * SY))
                nc.vector.tensor_tensor(out=xt[:], in0=xt[:], in1=xo[:], op=OP.add)
                if apply_bias:
                    nc.gpsimd.tensor_tensor(out=xt[:], in0=xt[:], in1=bpj_sb[:], op=OP.add)
                # --- LN2 for this token tile ---
                stats = st.tile([P, 2, 6], F32, tag="st2", name="st2")
                for g in range(2):
                    nc.vector.bn_stats(out=stats[:, g, :], in_=xt[:, g * 512:(g + 1) * 512])
                mv = st.tile([P, 2], F32, tag="mv2", name="mv2")
                nc.vector.bn_aggr(out=mv[:], in_=stats[:])
                rstd = st.tile([P, 1], F32, tag="rstd2", name="rstd2")
                nc.scalar.activation(rstd[:], mv[:, 1:2], AF.Sqrt, bias=eps_sb[:], scale=1.0)
                nc.vector.reciprocal(out=rstd[:], in_=rstd[:])
                xb = sp.tile([P, C], BF16, tag="xb2", name="xb2")
                nmr = st.tile([P, 1], F32, tag="nmr2", name="nmr2")
                nc.vector.tensor_scalar(nmr[:], mv[:, 0:1], rstd[:], -1.0,
                                        OP.mult, OP.mult)
                if apply_lnwb:
                    xc = sp.tile([P, C], F32, tag="xc2", name="xc2")
                    nc.scalar.activation(xc[:], xt[:], AF.Identity,
                                         bias=nmr[:], scale=rstd[:])
                    xw = sp.tile([P, C], F32, tag="xw2", name="xw2")
                    nc.vector.tensor_tensor(out=xw[:], in0=xc[:], in1=w2[:], op=OP.mult)
                    nc.vector.tensor_tensor(out=xb[:], in0=xw[:], in1=b2[:], op=OP.add)
                else:
                    nc.scalar.activation(xb[:], xt[:], AF.Identity,
                                         bias=nmr[:], scale=rstd[:])
                pst = tps.tile([P, CC, P], BF16, tag="trp2", name="trp2")
                for cc in range(CC):
                    nc.tensor.transpose(pst[:, cc, :],
                                        xb[:, cc * P:(cc + 1) * P], ident[:])
                nc.vector.tensor_copy(
                    out=xln2T[:, :, t8 * P:(t8 + 1) * P], in_=pst[:])

        es_y.close()

        es_y8.close()
        es_wpj.close()

        # ---------------- Phase 7: FC + gelu (DoubleRow fp8) ---------------
        with tc.tile_pool(name="fc_w", bufs=3) as wp, \
             tc.tile_pool(name="fc_ps", bufs=3, space="PSUM") as fps:
            for hg in range(F // N):
                wt = wp.tile([P, NPR, 2, N], FP8, tag="wfc", name="wfc")
                nc.sync.dma_start(out=wt[:], in_=w_fc3[:, :, :, hg * N:(hg + 1) * N])
                for hs in range(4):
                    hf = hg * 4 + hs
                    ps2 = fps.tile([P, 2, N], F32, tag="fc_ps", name="fc_ps")
                    for pr in range(NPR):
                        lw = wt[:, pr, :, hs * P:(hs + 1) * P]
                        for tt in range(2):
                            nc.tensor.matmul(ps2[:, tt, :], lhsT=lw,
                                             rhs=xln2T[:, 2 * pr:2 * pr + 2, tt * N:(tt + 1) * N],
                                             start=(pr == 0), stop=(pr == NPR - 1),
                                             perf_mode=DR)
                    if not SIM_GELU:
                        nc.scalar.activation(h8[:, hf, :], ps2[:, 0:2, :],
                                             AF.Gelu_apprx_tanh,
                                             bias=bfc_sb[:, hf:hf + 1], scale=1.0 / SW)
                    else:
                        import math
                        cst = math.sqrt(2.0 / math.pi)
                        u = wp.tile([P, 2, N], F32, tag="g_u", name="g_u")
                        nc.scalar.activation(u[:], ps2[:, 0:2, :], AF.Identity,
                                             bias=bfc_sb[:, hf:hf + 1], scale=1.0 / SW)
                        u3 = wp.tile([P, 2, N], F32, tag="g_u3", name="g_u3")
                        nc.scalar.activation(u3[:], u[:], AF.Square, bias=0.0, scale=1.0)
                        nc.vector.tensor_tensor(out=u3[:], in0=u3[:], in1=u[:], op=OP.mult)
                        nc.vector.tensor_scalar(u3[:], u3[:], 0.044715, None, OP.mult)
                        nc.vector.tensor_tensor(out=u3[:], in0=u3[:], in1=u[:], op=OP.add)
                        tqh = wp.tile([P, 2, N], F32, tag="g_t", name="g_t")
                        nc.scalar.activation(tqh[:], u3[:], AF.Tanh, bias=0.0, scale=cst)
                        nc.vector.tensor_scalar(tqh[:], tqh[:], 1.0, None, OP.add)
                        nc.vector.tensor_tensor(out=tqh[:], in0=tqh[:], in1=u[:], op=OP.mult)
                        nc.vector.tensor_scalar(h8[:, hf, :], tqh[:], 0.5, None, OP.mult)

        es_x2.close()

        # ---------------- Phase 8: out matmul + residual (DoubleRow fp8) ---
        with tc.tile_pool(name="ot_w", bufs=6) as wp, \
             tc.tile_pool(name="ot_sp", bufs=3) as sp, \
             tc.tile_pool(name="ot_ps", bufs=8, space="PSUM") as ops_pool:
            for half in range(2):
                opss = [ops_pool.tile([P, N], F32, tag="ot_ps", name="ot_ps")
                        for _ in range(8)]
                for pr in range(F // 256):
                    wt = wp.tile([P, 2, C], FP8, tag="wot", name="wot")
                    nc.sync.dma_start(out=wt[:], in_=w_ot3[:, pr, :, :])
                    for tc4 in range(4):
                        t8 = half * 4 + tc4
                        lh = h8[:, 2 * pr:2 * pr + 2, t8 * P:(t8 + 1) * P]
                        for ft in range(2):
                            nc.tensor.matmul(opss[tc4 * 2 + ft][:], lhsT=lh,
                                             rhs=wt[:, :, ft * N:(ft + 1) * N],
                                             start=(pr == 0), stop=(pr == F // 256 - 1),
                                             perf_mode=DR)
                for tc4 in range(4):
                    t8 = half * 4 + tc4
                    ot = sp.tile([P, C], F32, tag="ot", name="ot")
                    for ft in range(2):
                        nc.scalar.activation(ot[:, ft * N:(ft + 1) * N],
                                             opss[tc4 * 2 + ft][:], AF.Identity,
                                             bias=0.0, scale=1.0 / SWO)
                    nc.vector.tensor_tensor(out=ot[:], in0=ot[:],
                                            in1=x_mid[t8][:], op=OP.add)
                    if apply_bias:
                        nc.vector.tensor_tensor(out=ot[:], in0=ot[:], in1=bot_sb[:], op=OP.add)
                    nc.sync.dma_start(out=out_d[t8 * P:(t8 + 1) * P, :], in_=ot[:])

    nc.finalize()
    return nc


def _own_blocks(s):
    return [0, 1, 2, 3, 12, 13, 14, 15] if s == 0 else list(range(4, 12))


def _prep_shared(inputs):
    f8 = ml_dtypes.float8_e4m3

    def pack_dr(wT, npr, scale):
        # wT: [K, M] (contraction-major); -> [P, npr, 2, M] with
        # [p, pr, hf, m] = scale * wT[pr*256 + hf*128 + p, m]
        K, M = wT.shape
        assert K == npr * 256
        a = (wT * scale).reshape(npr, 2, P, M).transpose(2, 0, 1, 3)
        return np.ascontiguousarray(a).astype(f8)

    W_attn = np.asarray(inputs["W_attn"], np.float32)
    shared = {
        "w_qk3": pack_dr(np.ascontiguousarray(W_attn[:2 * C].T), NPR, SW),
        "w_v3": pack_dr(np.ascontiguousarray(W_attn[2 * C:].T), NPR, SW),
        "w_pj3": pack_dr(np.ascontiguousarray(np.asarray(inputs["W_proj"], np.float32).T), NPR, SW),
        "w_fc3": pack_dr(np.ascontiguousarray(np.asarray(inputs["W_fc"], np.float32).T), NPR, SW),
        "w_ot3": pack_dr(np.ascontiguousarray(np.asarray(inputs["W_out"], np.float32).T), F // 256, SWO),
        "ln1w": np.ascontiguousarray(np.broadcast_to(np.asarray(inputs["ln1_w"], np.float32), (P, C))),
        "ln1b": np.ascontiguousarray(np.broadcast_to(np.asarray(inputs["ln1_b"], np.float32), (P, C))),
        "ln2w": np.ascontiguousarray(np.broadcast_to(np.asarray(inputs["ln2_w"], np.float32), (P, C))),
        "ln2b": np.ascontiguousarray(np.broadcast_to(np.asarray(inputs["ln2_b"], np.float32), (P, C))),
        "b_q": np.ascontiguousarray(np.asarray(inputs["b_attn"], np.float32)[:C].reshape(CC, P).T) * SW,
        "b_k": np.ascontiguousarray(np.asarray(inputs["b_attn"], np.float32)[C:2 * C].reshape(CC, P).T),
        "b_v": np.ascontiguousarray(np.asarray(inputs["b_attn"], np.float32)[2 * C:].reshape(CC, P).T) * SY,
        "b_pj": np.ascontiguousarray(np.broadcast_to(np.asarray(inputs["b_proj"], np.float32), (P, C))),
        "b_fc": np.ascontiguousarray(np.asarray(inputs["b_fc"], np.float32).reshape(F // P, P).T),
        "b_ot": np.ascontiguousarray(np.broadcast_to(np.asarray(inputs["b_out"], np.float32), (P, C))),
    }
    # mask4[p, m*N + qf] = 1 if qf >= m*128 + p else 0
    pp = np.arange(P)[:, None]
    qf = np.arange(N)[None, :]
    mask = np.zeros((P, 4, 2, N), np.float32)
    for m in range(4):
        mask[:, m, 0, :] = (qf >= m * P + pp)
        mask[:, m, 1, :] = mask[:, m, 0, :]
    shared["mask4"] = mask.astype(ml_dtypes.bfloat16)
    sel = np.zeros((2, P), np.float32)
    sel[0, :64] = 1.0
    sel[1, 64:] = 1.0
    shared["sel2"] = sel
    return shared


def _make_in_maps(inputs):
    x = np.asarray(inputs["x"], np.float32)
    shared = _prep_shared(inputs)
    in_maps = []
    for c in range(8):
        b, s = c // 2, c % 2
        own = _own_blocks(s)
        other = _own_blocks(1 - s)
        xb = x[b].reshape(16, P, C)
        m = dict(shared)
        m["x_seq"] = np.ascontiguousarray(
            np.concatenate([xb[own], xb[other]], axis=0).reshape(T2, C))
        m["gate2"] = np.full((P, 1), 0.0 if s == 1 else -1e30, np.float32)
        m["gate3"] = np.full((P, 1), 0.0 if s == 0 else -1e30, np.float32)
        in_maps.append(m)
    return in_maps


def _get_nc(apply_lnwb=True, apply_bias=True):
    key = ("nc", apply_lnwb, apply_bias, SIM_GELU)
    if key not in _CACHE:
        _CACHE[key] = _build_nc(apply_lnwb, apply_bias)
    return _CACHE[key]


def run_cores(inputs, profile=False):
    """Run the SPMD program; returns list of per-core result dicts."""
    global last_exec_time_ns
    apply_lnwb = not (
        np.allclose(np.asarray(inputs["ln1_w"]), 1.0)
        and np.allclose(np.asarray(inputs["ln1_b"]), 0.0)
        and np.allclose(np.asarray(inputs["ln2_w"]), 1.0)
        and np.allclose(np.asarray(inputs["ln2_b"]), 0.0))
    apply_bias = not (
        np.allclose(np.asarray(inputs["b_attn"]), 0.0)
        and np.allclose(np.asarray(inputs["b_proj"]), 0.0)
        and np.allclose(np.asarray(inputs["b_out"]), 0.0))
    nc = _get_nc(apply_lnwb, apply_bias)
    in_maps = _make_in_maps(inputs)
    if profile:
        import concourse.bass_utils as bass_utils
        bass_utils.upload_artifacts = lambda tmpdir: "local://" + tmpdir
        try:
            from trn_agent_boot.trn_boot import _ntff_profile_via_ctypes
            import antenv.axon_hooks as hooks
            if hooks.get_axon_ntff_profile_hook() is None:
                hooks.set_axon_ntff_profile_hook(
                    _ntff_profile_via_ctypes("/opt/axon/libaxon_pjrt.so"))
        except Exception:
            pass
        res = bass_utils.run_bass_kernel_spmd(nc, in_maps, list(range(8)), trace=True)
        last_exec_time_ns = res.exec_time_ns
        return res.results
    return _cached_runner(nc)(in_maps)


def _cached_runner(nc):
    """Per-process cached jit of the SPMD executable so repeated kernel()
    calls don't recompile (mirrors bass2jax.run_bass_via_pjrt's multi-core
    branch)."""
    key = ("runner", id(nc))
    if key in _CACHE:
        return _CACHE[key]
    import jax
    import numpy as _np
    from jax.sharding import Mesh, PartitionSpec
    from jax.experimental.shard_map import shard_map
    from concourse import bass2jax, mybir as _mybir
    bass2jax.install_neuronx_cc_hook()

    part_name = nc.partition_id_tensor.name if nc.partition_id_tensor else None
    in_names, out_names, out_avals, zero_outs = [], [], [], []
    for alloc in nc.m.functions[0].allocations:
        if not isinstance(alloc, _mybir.MemoryLocationSet):
            continue
        name = alloc.memorylocations[0].name
        if alloc.kind == "ExternalInput":
            if name != part_name:
                in_names.append(name)
        elif alloc.kind == "ExternalOutput":
            out_names.append(name)
            shape = tuple(alloc.tensor_shape)
            dtype = _mybir.dt.np(alloc.dtype)
            out_avals.append(jax.core.ShapedArray(shape, dtype))
            zero_outs.append(_np.zeros(shape, dtype))
    n_params = len(in_names)
    all_names = in_names + out_names
    if part_name is not None:
        all_names = all_names + [part_name]
    donate = tuple(range(n_params, n_params + len(out_names)))
    if jax.default_backend() == "cpu":
        donate = ()  # cpu sim path can't alias donated outputs

    def _body(*args):
        operands = list(args)
        if part_name is not None:
            operands.append(bass2jax.partition_id_tensor())
        outs = bass2jax._bass_exec_p.bind(
            *operands, out_avals=tuple(out_avals), in_names=tuple(all_names),
            out_names=tuple(out_names), lowering_input_output_aliases=(),
            sim_require_finite=True, sim_require_nnan=True, nc=nc)
        return tuple(outs)

    devices = jax.devices()[:8]
    mesh = Mesh(_np.asarray(devices), ("core",))
    spec = (PartitionSpec("core"),) * (n_params + len(out_names))
    sharded = jax.jit(
        shard_map(_body, mesh=mesh, in_specs=spec,
                  out_specs=(PartitionSpec("core"),) * len(out_names),
                  check_rep=False),
        donate_argnums=donate, keep_unused=True)

    def run(in_maps):
        concat_in = [
            _np.concatenate([_np.asarray(in_maps[c][nm]) for c in range(8)], axis=0)
            for nm in in_names]
        concat_zero = [_np.zeros((8 * z.shape[0], *z.shape[1:]), z.dtype)
                       for z in zero_outs]
        out_arrs = sharded(*concat_in, *concat_zero)
        return [
            {nm: _np.asarray(out_arrs[i]).reshape(8, *out_avals[i].shape)[c]
             for i, nm in enumerate(out_names)}
            for c in range(8)]

    _CACHE[key] = run
    return run


def kernel(**inputs) -> np.ndarray:
    results = run_cores(inputs, profile=PROFILE)
    out = np.empty((B, T, C), np.float32)
    for c in range(8):
        b, s = c // 2, c % 2
        res = results[c]["out"]
        for j, blk in enumerate(_own_blocks(s)):
            out[b, blk * P:(blk + 1) * P, :] = res[j * P:(j + 1) * P]
    return out
